# revision 1
# baseline (speedup 1.0000x reference)
"""Trainium2 Bass kernel for nn_Discriminator (2x TransformerConv GNN + pool + MLP).

v2 design:
- Graphs split 64-per-core (batch sorted => contiguous node ranges per core).
- Per-layer node table [v | p] (256 bf16 cols = 512B rows) in DRAM; p folds the
  attention q-projection: score(e->d) = p[src_e] . r[d] with r the dst-side
  feature block. One plain dma_gather per 128-edge tile (512B/row).
- Layer-1 table computed REPLICATED on every core from the full x (no
  collective). Layer-2 table shard computed in the layer-1 block epilogue and
  allgathered (the only collective).
- Per tile: PE transpose of the p-part, S = pT^T @ rT_block (PE), batched exp
  (scalar engine), W = onehot(dst) * exp(S) in one DVE op, attention aggregate
  + denominator via PE matmuls.

Self-contained: hardcodes problem shapes; layout computed from runtime inputs.
"""
import numpy as np
import ml_dtypes

import concourse.bass as bass
import concourse.bacc as bacc
import concourse.mybir as mybir
from concourse.tile import TileContext
from concourse.masks import make_identity
from concourse.bass_utils import run_bass_kernel_spmd

BF = ml_dtypes.bfloat16
N, E, G = 50000, 800000, 512
F_IN, H, SEQ = 64, 128, 256
FE = F_IN + 1                 # x extended with ones column
NCORES = 8
GPC = G // NCORES             # graphs per core
P = 128
SGB = 2                       # blocks per supergroup
GMAX = 8                      # tiles per gather call (1024 idxs; runtime ucode cap)
TB = 4                        # tiles per S/exp batch (one PSUM bank)
TTB = 8                       # tiles per transpose/copy batch (one bf16 bank)
SCALE = 1.0 / np.sqrt(np.float32(H))
EPS = 1e-30

FP32 = mybir.dt.float32
BF16 = mybir.dt.bfloat16
I16 = mybir.dt.int16
AF = mybir.ActivationFunctionType
OP = mybir.AluOpType


# ---------------------------------------------------------------- host prep

def _pack_idx(idx_stream):
    """idx_stream [ntot*128] -> [128, ntot*8] int16 (16-partition wrap, x8)."""
    n = idx_stream.shape[0]
    s = n // 16
    out = np.zeros((128, s), dtype=np.int16)
    arr = idx_stream.reshape(s, 16).T.astype(np.int16)
    for g in range(8):
        out[g * 16:(g + 1) * 16, :] = arr
    return out


def preprocess(inputs):
    batch = np.asarray(inputs['batch']).astype(np.int64)
    ei = np.asarray(inputs['edge_index']).astype(np.int64)
    src_g, dst_g = ei[0], ei[1]

    gstart = np.searchsorted(batch, np.arange(NCORES) * GPC)
    gend = np.searchsorted(batch, np.arange(NCORES) * GPC + GPC)
    nloc = gend - gstart
    ncap = int(np.ceil(nloc.max() / (2 * P)) * (2 * P))   # even block count
    NB = ncap // P
    split = (NCORES // 2) * ncap          # lo/hi table split row

    node_core = batch // GPC
    node_local = np.arange(N) - gstart[node_core]
    table_idx = node_core * ncap + node_local            # row in [8*ncap] table
    node_half = (table_idx >= split).astype(np.int64)
    half_idx = table_idx - node_half * split             # row within half

    edge_core = node_core[dst_g]
    per_core = []
    for c in range(NCORES):
        em = np.where(edge_core == c)[0]
        e_src, e_dst = src_g[em], dst_g[em]
        order = np.argsort(e_dst, kind='stable')
        e_src, e_dst = e_src[order], e_dst[order]
        dst_loc = e_dst - gstart[c]
        tsrc = half_idx[e_src]
        is_hi = node_half[e_src]
        blk = dst_loc // P
        buckets = {}
        for b in range(NB):
            bm = np.where(blk == b)[0]
            bh = is_hi[bm]
            for half in (0, 1):
                hm = bm[bh == half]
                buckets[(b, half)] = (tsrc[hm], dst_loc[hm] - b * P)
        per_core.append(buckets)

    # uniform tile counts per (b, half)
    tcount = {}
    for b in range(NB):
        for half in (0, 1):
            mx = max(len(per_core[c][(b, half)][0]) for c in range(NCORES))
            tcount[(b, half)] = (mx + P - 1) // P

    # supergroups and stream layout (shared across cores)
    sgs = []           # (t0, Tlo, Tsg, blocks)
    tile_block = []    # per tile: block id
    t0 = 0
    for s0 in range(0, NB, SGB):
        blocks = list(range(s0, min(s0 + SGB, NB)))
        lo = sum(tcount[(b, 0)] for b in blocks)
        hi = sum(tcount[(b, 1)] for b in blocks)
        for half in (0, 1):
            for b in blocks:
                tile_block += [b] * tcount[(b, half)]
        sgs.append((t0, lo, lo + hi, blocks))
        t0 += lo + hi
    ntot = t0
    tile_block = np.array(tile_block)
    blk_first = {b: int(np.where(tile_block == b)[0][0]) for b in range(NB)}
    blk_last = {b: int(np.where(tile_block == b)[0][-1]) for b in range(NB)}

    # per-core streams
    dls = []
    kvis = []
    for c in range(NCORES):
        kvi = np.zeros(ntot * P, np.int64)
        dl = np.full(ntot * P, -1.0, np.float32)
        pos = 0
        for (t0_, lo, tsg, blocks) in sgs:
            for half in (0, 1):
                for b in blocks:
                    k, d = per_core[c][(b, half)]
                    ntile = tcount[(b, half)]
                    cnt = len(k)
                    kvi[pos:pos + cnt] = k
                    dl[pos:pos + cnt] = d
                    pos += ntile * P
        assert pos == ntot * P
        dls.append(dl.reshape(ntot, P))
        kvis.append(kvi)

    # per-tile dst band: union across cores (program structure is shared)
    bands = []
    for tt in range(ntot):
        lo128, hi = P, -1
        for c in range(NCORES):
            v = dls[c][tt]
            v = v[v >= 0]
            if len(v):
                lo128 = min(lo128, int(v.min()))
                hi = max(hi, int(v.max()))
        if hi < 0:
            lo128, hi = 0, 0
        bands.append((lo128, hi - lo128 + 1))

    cores = []
    for c in range(NCORES):
        dl = dls[c]
        dlb = dl.copy()
        for tt in range(ntot):
            m = dlb[tt] >= 0
            dlb[tt][m] -= bands[tt][0]
        gl = np.full(ncap, -1.0, np.float32)
        gl[:nloc[c]] = (batch[gstart[c]:gend[c]] - c * GPC).astype(np.float32)
        cores.append({
            'kvidx': _pack_idx(kvis[c]),
            'dstl': dlb.T.astype(np.float32),                  # [128, ntot]
            'glocal': gl.reshape(NB, P).T.astype(np.float32),  # [128, NB]
        })

    return {
        'ncap': ncap, 'NB': NB, 'split': split, 'ntot': ntot,
        'gstart': gstart, 'gend': gend, 'nloc': nloc,
        'table_idx': table_idx, 'sgs': sgs,
        'tile_block': tile_block, 'blk_first': blk_first, 'blk_last': blk_last,
        'bands': bands, 'cores': cores,
    }


def make_inputs(inputs, meta):
    ncap = meta['ncap']
    x = np.asarray(inputs['x'], np.float32)
    xte = np.zeros((FE, NCORES * ncap), np.float32)
    xte[F_IN, :] = 1.0
    xte[:F_IN, meta['table_idx']] = x.T
    xte_bf = xte.astype(BF)

    f32 = lambda v: np.asarray(v, np.float32)

    # layer-1 folded weights.  score = x_ext[src] A1 x_ext[dst]^T
    wk1 = np.concatenate([f32(inputs['k1_w']), f32(inputs['k1_b'])[None, :]], 0)
    wq1 = np.concatenate([f32(inputs['q1_w']), f32(inputs['q1_b'])[None, :]], 0)
    A1 = wk1 @ wq1.T                                     # [FE, FE]
    wtab1 = np.zeros((FE, 2 * H), np.float32)
    wtab1[:F_IN, 0:H] = f32(inputs['v1_w'])              # v1 (bias folded below)
    wtab1[:, H:H + FE] = A1
    ws1 = np.zeros((FE, H), np.float32)
    ws1[:F_IN, :] = f32(inputs['s1_w'])
    ws1[F_IN, :] = f32(inputs['s1_b']) + f32(inputs['v1_b'])  # skip + v bias

    # layer-2: score = h1[src] A2 h1[dst]^T + u.h1[src] (+ dst terms cancel)
    A2 = f32(inputs['k2_w']).astype(np.float64) @ f32(inputs['q2_w']).T.astype(np.float64)
    u = f32(inputs['k2_w']).astype(np.float64) @ f32(inputs['q2_b']).astype(np.float64)
    U, S, Vt = np.linalg.svd(A2)
    US = (U[:, :H - 1] * S[:H - 1]).astype(np.float32)   # [H, 127]
    V127 = Vt[:H - 1, :].T.astype(np.float32)            # [H, 127]
    wtab2 = np.zeros((H, 2 * H), np.float32)
    wtab2[:, 0:H] = f32(inputs['v2_w'])
    wtab2[:, H:2 * H - 1] = US
    wtab2[:, 2 * H - 1] = u.astype(np.float32)
    ws2 = f32(inputs['s2_w'])
    b2col = (f32(inputs['s2_b']) + f32(inputs['v2_b']))[:, None]  # [H,1]

    seqc = np.asarray(inputs['sequence_character'], np.float32)

    shared = {
        'xte': np.ascontiguousarray(xte_bf),
        'wtab1': wtab1.astype(BF),
        'ws1': ws1.astype(BF),
        'wtab2': wtab2.astype(BF),
        'v127': np.ascontiguousarray(V127.astype(BF)),
        'ws2': ws2.astype(BF),
        'b2col': b2col.astype(np.float32),
        'seqw': f32(inputs['seq_w']).astype(BF),
        'seqb': f32(inputs['seq_b'])[None, :].astype(BF),
        'fc1w': f32(inputs['fc1_w']).astype(BF),
        'fc1b': f32(inputs['fc1_b'])[None, :].astype(BF),
        'fc2w': f32(inputs['fc2_w']).astype(BF),
        'fc2b': f32(inputs['fc2_b'])[None, :].astype(BF),
        'iota': np.tile(np.arange(P, dtype=np.float32)[None, :], (P, 1)).astype(BF),
    }

    in_maps = []
    for c in range(NCORES):
        m = dict(shared)
        m['xte_own'] = np.ascontiguousarray(xte_bf[:, c * ncap:(c + 1) * ncap])
        m['seqT'] = np.ascontiguousarray(
            seqc[c * GPC:(c + 1) * GPC].T.astype(BF))            # [256, 64]
        mc = meta['cores'][c]
        m['kvidx'] = mc['kvidx']
        m['dstl'] = mc['dstl']
        m['glocal'] = mc['glocal']
        in_maps.append(m)
    return in_maps


# ---------------------------------------------------------------- program

def build_program(meta, dbg=None):
    ncap, NB, ntot = meta['ncap'], meta['NB'], meta['ntot']
    split = meta['split']
    sgs = meta['sgs']
    bands = meta['bands']
    tile_block = meta['tile_block']
    blk_first, blk_last = meta['blk_first'], meta['blk_last']
    NT = NCORES * ncap // P          # node tiles in the full table

    nc = bacc.Bacc("TRN2", target_bir_lowering=False, debug=False,
                   enable_asserts=False, num_devices=NCORES,
                   num_swdge_queues=4)

    def din(name, shape, dt):
        return nc.dram_tensor(name, shape, dt, kind="ExternalInput").ap()

    xte = din('xte', [FE, NCORES * ncap], BF16)
    xte_own = din('xte_own', [FE, ncap], BF16)
    wtab1 = din('wtab1', [FE, 2 * H], BF16)
    ws1 = din('ws1', [FE, H], BF16)
    wtab2 = din('wtab2', [H, 2 * H], BF16)
    v127 = din('v127', [H, H - 1], BF16)
    ws2 = din('ws2', [H, H], BF16)
    b2col = din('b2col', [H, 1], FP32)
    seqw = din('seqw', [SEQ, H], BF16)
    seqb = din('seqb', [1, H], BF16)
    fc1w = din('fc1w', [2 * H, H], BF16)
    fc1b = din('fc1b', [1, H], BF16)
    fc2w = din('fc2w', [H, 1], BF16)
    fc2b = din('fc2b', [1, 1], BF16)
    iota_in = din('iota', [P, P], BF16)
    seqT = din('seqT', [SEQ, GPC], BF16)
    kvidx = din('kvidx', [P, ntot * 8], I16)
    dstl = din('dstl', [P, ntot], FP32)
    glocal = din('glocal', [P, NB], FP32)

    out_g = nc.dram_tensor('out_g', [GPC, 1], FP32, kind="ExternalOutput").ap()
    if dbg:
        dbg_o = nc.dram_tensor('dbg_o', [P, NB * P], FP32,
                               kind="ExternalOutput").ap()

    tab1_lo = nc.dram_tensor('tab1_lo', [split, 2 * H], BF16,
                             kind="Internal").ap()
    tab1_hi = nc.dram_tensor('tab1_hi', [NCORES * ncap - split, 2 * H], BF16,
                             kind="Internal").ap()
    tab2_sh = nc.dram_tensor('tab2_sh', [ncap, 2 * H], BF16,
                             kind="Internal").ap()
    tab2 = nc.dram_tensor('tab2', [NCORES * ncap, 2 * H], BF16,
                          kind="Internal", addr_space="Shared").ap()

    from contextlib import ExitStack
    with TileContext(nc, num_cores=NCORES) as tc, ExitStack() as _st:
        cpool = _st.enter_context(tc.tile_pool(name="consts", bufs=1))
        pool = _st.enter_context(tc.tile_pool(name="work", bufs=3))
        spool = _st.enter_context(tc.tile_pool(name="stage", bufs=5))
        wpool = _st.enter_context(tc.tile_pool(name="wts", bufs=16))
        persist = _st.enter_context(tc.tile_pool(name="persist", bufs=1))
        psS = _st.enter_context(tc.tile_pool(name="psS", bufs=3, space="PSUM"))
        psT = _st.enter_context(tc.tile_pool(name="psT", bufs=2, space="PSUM"))
        psB = _st.enter_context(tc.tile_pool(name="psB", bufs=3, space="PSUM"))

        # ---------------- constants
        iota = cpool.tile([P, P], BF16)
        nc.sync.dma_start(out=iota[:], in_=iota_in)
        ident = cpool.tile([P, P], BF16)
        make_identity(nc, ident[:])
        ones_col = cpool.tile([P, 1], BF16)
        nc.vector.memset(ones_col[:], 1.0)
        ones_row = cpool.tile([1, P], BF16)
        nc.vector.memset(ones_row[:], 1.0)
        ones_row_f = cpool.tile([1, P], FP32)
        nc.vector.memset(ones_row_f[:], 1.0)
        zero_row = cpool.tile([1, 2 * P], BF16)
        nc.vector.memset(zero_row[:], 0.0)

        _cn = [0]

        def const_tile(ap_, shape, dt=BF16):
            _cn[0] += 1
            t = cpool.tile(shape, dt, tag=f"c{_cn[0]}", name=f"c{_cn[0]}")
            nc.sync.dma_start(out=t[:], in_=ap_)
            return t

        wtab1_t = const_tile(wtab1, [FE, 2 * H])
        ws1_t = const_tile(ws1, [FE, H])
        wtab2_t = const_tile(wtab2, [H, 2 * H])
        v127_t = const_tile(v127, [H, H - 1])
        ws2_t = const_tile(ws2, [H, H])
        b2col_t = const_tile(b2col, [H, 1], FP32)
        seqb_t = const_tile(seqb, [1, H])
        fc1b_t = const_tile(fc1b, [1, H])
        fc2w_t = const_tile(fc2w, [H, 1])
        fc2b_t = const_tile(fc2b, [1, 1])
        kvidx_t = const_tile(kvidx, [P, ntot * 8], I16)
        dstl_t = const_tile(dstl, [P, ntot], FP32)
        glocal_t = const_tile(glocal, [P, NB], FP32)

        h1T_own = persist.tile([P, NB * P], BF16)   # h1 transposed, own shard

        # ---------------- prologue: table-1 for ALL nodes (replicated)
        def copy_any(i, out, in_):
            if i % 2 == 0:
                nc.vector.tensor_copy(out=out, in_=in_)
            else:
                nc.scalar.copy(out=out, in_=in_)

        WSTG = 8                                     # node tiles per DRAM write
        for w0 in range(0, NT, WSTG):
            wn = min(WSTG, NT - w0)
            stg = spool.tile([P, WSTG * 2 * H], BF16, tag="t1stg")
            xf = spool.tile([FE, WSTG * P], BF16, tag="xfat")
            nc.sync.dma_start(out=xf[:, 0:wn * P],
                              in_=xte[:, w0 * P:(w0 + wn) * P])
            for j in range(wn):
                nt = w0 + j
                pt = psS.tile([P, 2 * H], FP32, space="PSUM", tag="psS",
                              name=f"pro{nt}")
                nc.tensor.matmul(out=pt[:], lhsT=xf[:, j * P:(j + 1) * P],
                                 rhs=wtab1_t[:], start=True, stop=True)
                copy_any(nt, stg[:, j * 2 * H:(j + 1) * 2 * H], pt[:])
            r0, r1 = w0 * P, (w0 + wn) * P
            dst = (tab1_lo[r0:r1, :] if r1 <= split
                   else tab1_hi[r0 - split:r1 - split, :])
            nc.sync.dma_start(out=dst, in_=stg[:, 0:wn * 2 * H])

        # ---------------- edge phase (shared for both layers)
        # Per-block PSUM bank "blk" [P, 4*P] f32:
        #   [:, 0:P]      attention aggregate (accT)
        #   [0:1, P:2P]   denominator row
        #   [:, 2P:3P]    skip projection
        #   [:, 3P:4P]    scratch (rhs2 vh / epilogue denb)
        def edge_phase(layer, tab_lo_ap, tab_hi_ap, rhs_for_block,
                       skip_for_block, on_block_done):
            """rhs_for_block(b, blk) -> (rhs_tile_ap, cp) SBUF [cp,128] dst feats.
            skip_for_block(b, blk) writes skip into blk[:, 2P:3P].
            on_block_done(b, h_sb) with h_sb [128,128] bf16 = relu'd output^T."""
            _q = [0]
            _blks, _rhs = {}, {}

            pend_T = []   # tiles awaiting transpose-copy: (psum_tile, j, tt)

            ptc = [0]

            def flush_T():
                if not pend_T:
                    return None
                pts, n = pend_T[0][0], len(pend_T)
                sb = spool.tile([P, TTB, P], BF16, tag="pT_sb")
                if ptc[0] % 2 == 0:
                    nc.vector.tensor_copy(out=sb[:, 0:n, :], in_=pts[:, 0:n, :])
                else:
                    nc.scalar.copy(out=sb[:, 0:n, :], in_=pts[:, 0:n, :])
                ptc[0] += 1
                del pend_T[:]
                return sb

            for (t0, lo, tsg, blocks) in sgs:
                kv_t = pool.tile([P, tsg, 2 * H], BF16, tag="kv_g", bufs=5)
                for (h0, h1, hh) in ((0, lo, 0), (lo, tsg, 1)):
                    for ps0 in range(h0, h1, GMAX):
                        pe0 = min(ps0 + GMAX, h1)
                        tab_h = tab_lo_ap if hh == 0 else tab_hi_ap
                        nc.gpsimd.dma_gather(
                            out_ap=kv_t[:, ps0:pe0, :], in_ap=tab_h,
                            idxs_ap=kvidx_t[:, (t0 + ps0) * 8:(t0 + pe0) * 8],
                            num_idxs=(pe0 - ps0) * P, num_idxs_reg=(pe0 - ps0) * P,
                            elem_size=2 * H, queue_num=_q[0] % 4)
                        _q[0] += 1

                # pass A: transpose p-parts (batches of TTB)
                pT_sbs = {}
                for tl in range(tsg):
                    if tl % TTB == 0:
                        psT_t = psT.tile([P, TTB, P], BF16, space="PSUM",
                                         tag="psT")
                    tp = nc.tensor.transpose(
                        out=psT_t[:, tl % TTB, :], in_=kv_t[:, tl, H:2 * H],
                        identity=ident[:])
                    pend_T.append((psT_t, tl % TTB, t0 + tl))
                    if tl % TTB == TTB - 1 or tl == tsg - 1:
                        sb = flush_T()
                        pT_sbs[tl // TTB] = sb

                # pass B: banded S matmuls packed into PSUM strips, exp per strip
                SW = P                         # strip width (quarter bank)
                tile_se = {}                   # tl -> (strip idx, off, w)
                strips = []                    # psum strip tiles
                strip_cols = []                # used cols per strip
                for tl in range(tsg):
                    tt = t0 + tl
                    b = int(tile_block[tt])
                    if b not in _blks:
                        blk = psB.tile([P, 4 * P], FP32, space="PSUM",
                                       tag="blk", name=f"blk{layer}_{b}")
                        _blks[b] = blk
                        # zero acc+den regions, open the accumulation group
                        nc.tensor.matmul(out=blk[:, 0:2 * P],
                                         lhsT=zero_row[:1, 0:P],
                                         rhs=zero_row[:1, :],
                                         start=True, stop=False)
                        _rhs[b] = rhs_for_block(b, blk)
                        skip_for_block(b, blk)
                    rhs_sb, cp = _rhs[b]
                    dlo, w = bands[tt]
                    if not strips or strip_cols[-1] + w > SW:
                        st = psS.tile([P, SW], FP32, space="PSUM", tag="psS",
                                      name=f"st{layer}_{tt}")
                        strips.append(st)
                        strip_cols.append(0)
                    off = strip_cols[-1]
                    strip_cols[-1] += w
                    tile_se[tl] = (len(strips) - 1, off, w)
                    pT_sb = pT_sbs[tl // TTB]
                    nc.tensor.matmul(
                        out=strips[-1][:, off:off + w],
                        lhsT=pT_sb[0:cp, tl % TTB, :],
                        rhs=rhs_sb[:, dlo:dlo + w],
                        start=True, stop=True)

                E_sbs = []
                for si, st in enumerate(strips):
                    esb = spool.tile([P, SW], BF16, tag="E_sb",
                                     name=f"esb{layer}_{t0}_{si}")
                    used = strip_cols[si]
                    nc.scalar.activation(out=esb[:, 0:used], in_=st[:, 0:used],
                                         func=AF.Exp, scale=float(SCALE))
                    E_sbs.append(esb)

                for tl in range(tsg):
                    tt = t0 + tl
                    b = int(tile_block[tt])
                    si, off, w = tile_se[tl]
                    E_sb = E_sbs[si]
                    W = wpool.tile([P, P], BF16, tag="W")
                    nc.vector.scalar_tensor_tensor(
                        out=W[:, 0:w], in0=iota[:, 0:w],
                        scalar=dstl_t[:, tt:tt + 1],
                        in1=E_sb[:, off:off + w],
                        op0=OP.is_equal, op1=OP.mult)
                    blk = _blks[b]
                    dlo = bands[tt][0]
                    nc.tensor.matmul(
                        out=blk[:, dlo:dlo + w], lhsT=kv_t[:, tl, 0:H],
                        rhs=W[:, 0:w], start=False, stop=False)
                    nc.tensor.matmul(
                        out=blk[0:1, P + dlo:P + dlo + w], lhsT=ones_col[:],
                        rhs=W[:, 0:w], start=False, stop=False)
                    if tt == blk_last[b]:
                        # close the accumulation group (flush)
                        nc.tensor.matmul(out=blk[:, 0:2 * P],
                                         lhsT=zero_row[:1, 0:P],
                                         rhs=zero_row[:1, :],
                                         start=False, stop=True)
                        # epilogue: normalize + skip + relu
                        _blks.pop(b)
                        _rhs.pop(b)
                        dv = pool.tile([1, P], FP32, tag="dv")
                        nc.vector.tensor_scalar_add(out=dv[:],
                                                    in0=blk[0:1, P:2 * P],
                                                    scalar1=EPS)
                        rv = pool.tile([1, P], FP32, tag="rv")
                        nc.vector.reciprocal(out=rv[:], in_=dv[:])
                        nc.tensor.matmul(out=blk[:, 3 * P:4 * P],
                                         lhsT=ones_row_f[:],
                                         rhs=rv[:], start=True, stop=True)
                        dnb = pool.tile([P, P], FP32, tag="dnb")
                        nc.scalar.copy(out=dnb[:], in_=blk[:, 3 * P:4 * P])
                        t1 = pool.tile([P, P], FP32, tag="t1")
                        nc.vector.tensor_tensor(out=t1[:], in0=blk[:, 0:P],
                                                in1=dnb[:], op=OP.mult)
                        t2 = pool.tile([P, P], FP32, tag="t2")
                        nc.vector.tensor_tensor(out=t2[:], in0=t1[:],
                                                in1=blk[:, 2 * P:3 * P],
                                                op=OP.add)
                        h_sb = spool.tile([P, P], BF16, tag="h_sb")
                        if layer == 1:
                            nc.scalar.activation(out=h_sb[:], in_=t2[:],
                                                 func=AF.Relu)
                        else:
                            nc.scalar.activation(out=h_sb[:], in_=t2[:],
                                                 func=AF.Relu,
                                                 bias=b2col_t[:, 0:1])
                        on_block_done(b, h_sb, blk)

        # ---------------- layer 1 plumbing
        _rhs1_cache = {}

        def rhs1(b, blk):
            xt = spool.tile([FE, P], BF16, tag="xblk")
            nc.sync.dma_start(out=xt[:], in_=xte_own[:, b * P:(b + 1) * P])
            _rhs1_cache[b] = xt
            return (xt[:], FE)

        def skip1(b, blk):
            nc.tensor.matmul(out=blk[:, 2 * P:3 * P], lhsT=ws1_t[:],
                             rhs=_rhs1_cache.pop(b)[:],
                             start=True, stop=True)

        def l1_done(b, h_sb, blk):
            nc.vector.tensor_copy(out=h1T_own[:, b * P:(b + 1) * P], in_=h_sb[:])
            # layer-2 table shard rows for this block (reuse blk skip+spare)
            nc.tensor.matmul(out=blk[:, 2 * P:4 * P], lhsT=h_sb[:],
                             rhs=wtab2_t[:], start=True, stop=True)
            tsb = spool.tile([P, 2 * H], BF16, tag="t2row")
            nc.scalar.copy(out=tsb[:], in_=blk[:, 2 * P:4 * P])
            nc.sync.dma_start(out=tab2_sh[b * P:(b + 1) * P, :], in_=tsb[:])

        edge_phase(1, tab1_lo, tab1_hi, rhs1, skip1, l1_done)

        if dbg == 'h1':
            d = pool.tile([P, NB * P], FP32, tag="dbg")
            nc.vector.tensor_copy(out=d[:], in_=h1T_own[:])
            nc.sync.dma_start(out=dbg_o, in_=d[:])
            do = pool.tile([GPC, 1], FP32, tag="dbgo")
            nc.vector.memset(do[:], 0.5)
            nc.sync.dma_start(out=out_g, in_=do[:])

        # ---------------- collective: allgather layer-2 table
        nc.gpsimd.collective_compute(
            kind="AllGather", op=OP.bypass,
            replica_groups=[list(range(NCORES))],
            ins=[tab2_sh], outs=[tab2])

        if dbg is None or dbg == 'full':
            # ---------------- layer 2 plumbing
            pool_sb = persist.tile([GPC, H + 1], FP32)
            nc.vector.memset(pool_sb[:], 0.0)

            # precompute all dst-side feature blocks during the collective:
            # vh_all[:, b] = [V127^T h1_b ; ones-row]
            vh_all = persist.tile([H, NB * P], BF16)
            nc.vector.memset(vh_all[:], 1.0)
            for b in range(NB):
                vps = psS.tile([H - 1, P], FP32, space="PSUM", tag="psS",
                               name=f"vh{b}")
                nc.tensor.matmul(out=vps[:], lhsT=v127_t[:],
                                 rhs=h1T_own[:, b * P:(b + 1) * P],
                                 start=True, stop=True)
                if b % 2 == 0:
                    nc.vector.tensor_copy(
                        out=vh_all[0:H - 1, b * P:(b + 1) * P], in_=vps[:])
                else:
                    nc.scalar.copy(
                        out=vh_all[0:H - 1, b * P:(b + 1) * P], in_=vps[:])

            def rhs2(b, blk):
                return (vh_all[:, b * P:(b + 1) * P], H)

            def skip2(b, blk):
                nc.tensor.matmul(out=blk[:, 2 * P:3 * P], lhsT=ws2_t[:],
                                 rhs=h1T_own[:, b * P:(b + 1) * P],
                                 start=True, stop=True)

            def l2_done(b, h_sb, blk):
                # transpose h2^T -> h2 [d, h], then pool matmul
                tp = psT.tile([P, P], BF16, space="PSUM", tag="psT",
                              name=f"h2tp{b}")
                nc.tensor.transpose(out=tp[:], in_=h_sb[:], identity=ident[:])
                h2x = pool.tile([P, H + 1], BF16, tag="h2x")
                nc.scalar.copy(out=h2x[:, 0:H], in_=tp[:])
                nc.vector.memset(h2x[:, H:H + 1], 1.0)
                gh = pool.tile([P, GPC], BF16, tag="gh")
                nc.vector.tensor_scalar(
                    out=gh[:], in0=iota[:, 0:GPC], scalar1=glocal_t[:, b:b + 1],
                    scalar2=None, op0=OP.is_equal)
                nc.tensor.matmul(out=blk[0:GPC, 2 * P:2 * P + H + 1],
                                 lhsT=gh[:], rhs=h2x[:],
                                 start=True, stop=True)
                nc.vector.tensor_tensor(out=pool_sb[:], in0=pool_sb[:],
                                        in1=blk[0:GPC, 2 * P:2 * P + H + 1],
                                        op=OP.add)

            # seq branch computed during the collective window
            seqw_t0 = const_tile(seqw[0:P, :], [P, H])
            seqw_t1 = const_tile(seqw[P:SEQ, :], [P, H])
            fc1w_t0 = const_tile(fc1w[0:P, :], [P, H])
            fc1w_t1 = const_tile(fc1w[P:2 * H, :], [P, H])
            seqT0 = const_tile(seqT[0:P, :], [P, GPC])
            seqT1 = const_tile(seqT[P:SEQ, :], [P, GPC])
            z = pool.tile([GPC, 2 * H], BF16, tag="z")
            pseq = psS.tile([GPC, H], FP32, space="PSUM", tag="psS",
                            name="pseq")
            nc.tensor.matmul(out=pseq[:], lhsT=seqT0[:], rhs=seqw_t0[:],
                             start=True, stop=False)
            nc.tensor.matmul(out=pseq[:], lhsT=seqT1[:], rhs=seqw_t1[:],
                             start=False, stop=False)
            nc.tensor.matmul(out=pseq[:], lhsT=ones_row[:1, 0:GPC],
                             rhs=seqb_t[:1, :], start=False, stop=True)
            nc.scalar.activation(out=z[:, H:2 * H], in_=pseq[:], func=AF.Relu)

            edge_phase(2, tab2[0:split, :], tab2[split:NCORES * ncap, :],
                       rhs2, skip2, l2_done)

            # ---------------- tail: pooled -> MLP -> sigmoid
            cnt = pool.tile([GPC, 1], FP32, tag="cnt")
            nc.vector.tensor_scalar_add(out=cnt[:], in0=pool_sb[:, H:H + 1],
                                        scalar1=EPS)
            rc = pool.tile([GPC, 1], FP32, tag="rc")
            nc.vector.reciprocal(out=rc[:], in_=cnt[:])
            nc.vector.tensor_scalar(out=z[:, 0:H], in0=pool_sb[:, 0:H],
                                    scalar1=rc[:, 0:1], scalar2=None,
                                    op0=OP.mult)

            zT = []
            for i in range(2):
                tzp = psT.tile([P, GPC], BF16, space="PSUM", tag="psT",
                               name=f"tzp{i}")
                nc.tensor.transpose(out=tzp[:], in_=z[:, i * H:(i + 1) * H],
                                    identity=ident[0:GPC, 0:GPC])
                zt = pool.tile([P, GPC], BF16, tag=f"zT{i}")
                nc.vector.tensor_copy(out=zt[:], in_=tzp[:])
                zT.append(zt)
            pfc1 = psS.tile([GPC, H], FP32, space="PSUM", tag="psS",
                            name="pfc1")
            nc.tensor.matmul(out=pfc1[:], lhsT=zT[0][:], rhs=fc1w_t0[:],
                             start=True, stop=False)
            nc.tensor.matmul(out=pfc1[:], lhsT=zT[1][:], rhs=fc1w_t1[:],
                             start=False, stop=False)
            nc.tensor.matmul(out=pfc1[:], lhsT=ones_row[:1, 0:GPC],
                             rhs=fc1b_t[:1, :], start=False, stop=True)
            z1 = pool.tile([GPC, H], BF16, tag="z1")
            nc.scalar.activation(out=z1[:], in_=pfc1[:], func=AF.Relu)
            tz1 = psT.tile([P, GPC], BF16, space="PSUM", tag="psT",
                           name="tz1")
            nc.tensor.transpose(out=tz1[:], in_=z1[:],
                                identity=ident[0:GPC, 0:GPC])
            z1T = pool.tile([P, GPC], BF16, tag="z1T")
            nc.vector.tensor_copy(out=z1T[:], in_=tz1[:])
            pfc2 = psS.tile([GPC, 1], FP32, space="PSUM", tag="psS",
                            name="pfc2")
            nc.tensor.matmul(out=pfc2[:], lhsT=z1T[:], rhs=fc2w_t[:],
                             start=True, stop=False)
            nc.tensor.matmul(out=pfc2[:], lhsT=ones_row[:1, 0:GPC],
                             rhs=fc2b_t[:1, :], start=False, stop=True)
            outs = pool.tile([GPC, 1], FP32, tag="outs")
            nc.scalar.activation(out=outs[:], in_=pfc2[:], func=AF.Sigmoid)
            nc.sync.dma_start(out=out_g, in_=outs[:])

    nc.compile()
    return nc


# ---------------------------------------------------------------- entry

_CACHE = {}


def kernel(**inputs):
    meta = preprocess(inputs)
    key = (meta['ncap'], meta['ntot'], tuple(meta['tile_block'].tolist()))
    if key not in _CACHE:
        _CACHE[key] = build_program(meta)
    nc = _CACHE[key]
    in_maps = make_inputs(inputs, meta)
    res = run_bass_kernel_spmd(nc, in_maps, core_ids=list(range(NCORES)))
    out = np.concatenate([res.results[c]['out_g'] for c in range(NCORES)], 0)
    return out.astype(np.float32)



# revision 5
# speedup vs baseline: 1.0452x; 1.0452x over previous
"""Trainium2 Bass kernel for nn_Discriminator (2x TransformerConv GNN + pool + MLP).

v3 design (raw-feature gathers, no node-table prologue):
- Graphs split 64-per-core (batch sorted => contiguous node ranges per core).
- L1 gathers RAW x_ext rows ([x|1|0pad] -> 128 bf16 cols, 256B) straight from
  a host-built row table; the score projection A1 is applied per dst-block
  (B1_b = A1 @ x_d^T), and the value projection v1 is applied per block AFTER
  aggregation (linearity). The ones column of x_ext yields the softmax
  denominator for free in the aggregation matmul.
- L2 allgathers RAW h1 ([N,128] bf16, half the bytes of a v|p table) and
  applies A2/u per block (R_b = A2 h1_d + u) and v2 after aggregation.
- Per tile: PE transpose of the gathered rows, S = X_sT^T @ B_b (PE), batched
  exp (scalar engine), W = onehot(dst) * exp(S) in one DVE op, aggregation
  via one PE matmul (plus a denominator matmul for L2 only).

Self-contained: hardcodes problem shapes; layout computed from runtime inputs.
"""
import numpy as np
import ml_dtypes

import concourse.bass as bass
import concourse.bacc as bacc
import concourse.mybir as mybir
from concourse.tile import TileContext
from concourse.masks import make_identity
from concourse.bass_utils import run_bass_kernel_spmd

BF = np.float16
N, E, G = 50000, 800000, 512
F_IN, H, SEQ = 64, 128, 256
FE = F_IN + 1                 # x extended with ones column
NCORES = 8
GPC = G // NCORES             # graphs per core
P = 128
SGB = 2                       # blocks per supergroup
GMAX = 8                      # tiles per gather call (1024 idxs; runtime ucode cap)
TTB = 8                       # tiles per transpose/copy batch (one bf16 bank)
SCALE = 1.0 / np.sqrt(np.float32(H))
EPS = 1e-30

FP32 = mybir.dt.float32
BF16 = mybir.dt.float16
I16 = mybir.dt.int16
AF = mybir.ActivationFunctionType
OP = mybir.AluOpType


# ---------------------------------------------------------------- host prep

def _pack_idx(idx_stream):
    """idx_stream [ntot*128] -> [128, ntot*8] int16 (16-partition wrap, x8)."""
    n = idx_stream.shape[0]
    s = n // 16
    out = np.zeros((128, s), dtype=np.int16)
    arr = idx_stream.reshape(s, 16).T.astype(np.int16)
    for g in range(8):
        out[g * 16:(g + 1) * 16, :] = arr
    return out


def preprocess(inputs):
    batch = np.asarray(inputs['batch']).astype(np.int64)
    ei = np.asarray(inputs['edge_index']).astype(np.int64)
    src_g, dst_g = ei[0], ei[1]

    gstart = np.searchsorted(batch, np.arange(NCORES) * GPC)
    gend = np.searchsorted(batch, np.arange(NCORES) * GPC + GPC)
    nloc = gend - gstart
    ncap = int(np.ceil(nloc.max() / (2 * P)) * (2 * P))   # even block count
    NB = ncap // P
    split = (NCORES // 2) * ncap          # lo/hi table split row

    node_core = batch // GPC
    node_local = np.arange(N) - gstart[node_core]
    table_idx = node_core * ncap + node_local            # row in [8*ncap] table
    node_half = (table_idx >= split).astype(np.int64)
    half_idx = table_idx - node_half * split             # row within half

    edge_core = node_core[dst_g]
    per_core = []
    for c in range(NCORES):
        em = np.where(edge_core == c)[0]
        e_src, e_dst = src_g[em], dst_g[em]
        order = np.argsort(e_dst, kind='stable')
        e_src, e_dst = e_src[order], e_dst[order]
        dst_loc = e_dst - gstart[c]
        tsrc = half_idx[e_src]
        is_hi = node_half[e_src]
        blk = dst_loc // P
        buckets = {}
        for b in range(NB):
            bm = np.where(blk == b)[0]
            bh = is_hi[bm]
            for half in (0, 1):
                hm = bm[bh == half]
                buckets[(b, half)] = (tsrc[hm], dst_loc[hm] - b * P)
        per_core.append(buckets)

    # uniform tile counts per (b, half)
    tcount = {}
    for b in range(NB):
        for half in (0, 1):
            mx = max(len(per_core[c][(b, half)][0]) for c in range(NCORES))
            tcount[(b, half)] = (mx + P - 1) // P

    # supergroups and stream layout (shared across cores)
    sgs = []           # (t0, Tlo, Tsg, blocks)
    tile_block = []    # per tile: block id
    t0 = 0
    for s0 in range(0, NB, SGB):
        blocks = list(range(s0, min(s0 + SGB, NB)))
        lo = sum(tcount[(b, 0)] for b in blocks)
        hi = sum(tcount[(b, 1)] for b in blocks)
        for half in (0, 1):
            for b in blocks:
                tile_block += [b] * tcount[(b, half)]
        sgs.append((t0, lo, lo + hi, blocks))
        t0 += lo + hi
    ntot = t0
    tile_block = np.array(tile_block)
    blk_first = {b: int(np.where(tile_block == b)[0][0]) for b in range(NB)}
    blk_last = {b: int(np.where(tile_block == b)[0][-1]) for b in range(NB)}

    # per-core streams
    dls = []
    kvis = []
    for c in range(NCORES):
        kvi = np.zeros(ntot * P, np.int64)
        dl = np.full(ntot * P, -1.0, np.float32)
        pos = 0
        for (t0_, lo, tsg, blocks) in sgs:
            for half in (0, 1):
                for b in blocks:
                    k, d = per_core[c][(b, half)]
                    ntile = tcount[(b, half)]
                    cnt = len(k)
                    kvi[pos:pos + cnt] = k
                    dl[pos:pos + cnt] = d
                    pos += ntile * P
        assert pos == ntot * P
        dls.append(dl.reshape(ntot, P))
        kvis.append(kvi)

    # per-tile dst band: union across cores (program structure is shared)
    bands = []
    for tt in range(ntot):
        lo128, hi = P, -1
        for c in range(NCORES):
            v = dls[c][tt]
            v = v[v >= 0]
            if len(v):
                lo128 = min(lo128, int(v.min()))
                hi = max(hi, int(v.max()))
        if hi < 0:
            lo128, hi = 0, 0
        bands.append((lo128, hi - lo128 + 1))

    cores = []
    for c in range(NCORES):
        dl = dls[c]
        dlb = dl.copy()
        for tt in range(ntot):
            m = dlb[tt] >= 0
            dlb[tt][m] -= bands[tt][0]
        gl = np.full(ncap, -1.0, np.float32)
        gl[:nloc[c]] = (batch[gstart[c]:gend[c]] - c * GPC).astype(np.float32)
        cores.append({
            'kvidx': _pack_idx(kvis[c]),
            'dstl': dlb.T.astype(np.float32),                  # [128, ntot]
            'glocal': gl.reshape(NB, P).T.astype(np.float32),  # [128, NB]
        })

    return {
        'ncap': ncap, 'NB': NB, 'split': split, 'ntot': ntot,
        'gstart': gstart, 'gend': gend, 'nloc': nloc,
        'table_idx': table_idx, 'sgs': sgs,
        'tile_block': tile_block, 'blk_first': blk_first, 'blk_last': blk_last,
        'bands': bands, 'cores': cores,
    }


def make_inputs(inputs, meta):
    ncap = meta['ncap']
    x = np.asarray(inputs['x'], np.float32)
    f32 = lambda v: np.asarray(v, np.float32)

    # raw x_ext row table [8*ncap, 128]: [x | 1 | 0pad], gathered by L1
    xrows = np.zeros((NCORES * ncap, P), np.float32)
    xrows[meta['table_idx'], 0:F_IN] = x
    xrows[:, F_IN] = 1.0
    xrows_bf = xrows.astype(BF)

    # column-major x_ext per core (dst side): [FE, ncap]
    xte = np.zeros((FE, NCORES * ncap), np.float32)
    xte[F_IN, :] = 1.0
    xte[:F_IN, meta['table_idx']] = x.T
    xte_bf = xte.astype(BF)

    # layer-1: score = x_ext[s] A1 x_ext[d]^T; A1 = wk1 wq1^T
    wk1 = np.concatenate([f32(inputs['k1_w']), f32(inputs['k1_b'])[None, :]], 0)
    wq1 = np.concatenate([f32(inputs['q1_w']), f32(inputs['q1_b'])[None, :]], 0)
    A1 = wk1 @ wq1.T                                     # [FE, FE]
    v1ext = np.zeros((FE, H), np.float32)
    v1ext[:F_IN, :] = f32(inputs['v1_w'])
    v1ext[F_IN, :] = f32(inputs['v1_b'])
    ws1ext = np.zeros((FE, H), np.float32)
    ws1ext[:F_IN, :] = f32(inputs['s1_w'])
    ws1ext[F_IN, :] = f32(inputs['s1_b'])

    # layer-2: score = h1[s] A2 h1[d] + u.h1[s] (+ dst-only terms cancel)
    A2 = f32(inputs['k2_w']) @ f32(inputs['q2_w']).T     # [H, H]
    u = f32(inputs['k2_w']) @ f32(inputs['q2_b'])        # [H]
    b2col = (f32(inputs['s2_b']) + f32(inputs['v2_b']))[:, None]  # [H,1]

    seqc = np.asarray(inputs['sequence_character'], np.float32)
    split = meta['split']

    shared = {
        'xrows_lo': np.ascontiguousarray(xrows_bf[:split]),
        'xrows_hi': np.ascontiguousarray(xrows_bf[split:]),
        'a1t': np.ascontiguousarray(A1.T.astype(BF)),
        'v1ext': v1ext.astype(BF),
        'ws1ext': ws1ext.astype(BF),
        'a2t': np.ascontiguousarray(A2.T.astype(BF)),
        'urow': u[None, :].astype(BF),
        'v2w': f32(inputs['v2_w']).astype(BF),
        'ws2': f32(inputs['s2_w']).astype(BF),
        'b2col': b2col.astype(np.float32),
        'seqw': f32(inputs['seq_w']).astype(BF),
        'seqb': f32(inputs['seq_b'])[None, :].astype(BF),
        'fc1w': f32(inputs['fc1_w']).astype(BF),
        'fc1b': f32(inputs['fc1_b'])[None, :].astype(BF),
        'fc2w': f32(inputs['fc2_w']).astype(BF),
        'fc2b': f32(inputs['fc2_b'])[None, :].astype(BF),
        'iota': np.tile(np.arange(P, dtype=np.float32)[None, :], (P, 1)).astype(BF),
    }

    in_maps = []
    for c in range(NCORES):
        m = dict(shared)
        m['xte_own'] = np.ascontiguousarray(xte_bf[:, c * ncap:(c + 1) * ncap])
        m['seqT'] = np.ascontiguousarray(
            seqc[c * GPC:(c + 1) * GPC].T.astype(BF))            # [256, 64]
        mc = meta['cores'][c]
        m['kvidx'] = mc['kvidx']
        m['dstl'] = mc['dstl']
        m['glocal'] = mc['glocal']
        in_maps.append(m)
    return in_maps


# ---------------------------------------------------------------- program

def build_program(meta, dbg=None):
    ncap, NB, ntot = meta['ncap'], meta['NB'], meta['ntot']
    split = meta['split']
    sgs = meta['sgs']
    bands = meta['bands']
    tile_block = meta['tile_block']
    blk_first, blk_last = meta['blk_first'], meta['blk_last']

    nc = bacc.Bacc("TRN2", target_bir_lowering=False, debug=False,
                   enable_asserts=False, num_devices=NCORES,
                   num_swdge_queues=4)

    def din(name, shape, dt):
        return nc.dram_tensor(name, shape, dt, kind="ExternalInput").ap()

    xrows_lo = din('xrows_lo', [split, P], BF16)
    xrows_hi = din('xrows_hi', [NCORES * ncap - split, P], BF16)
    xte_own = din('xte_own', [FE, ncap], BF16)
    a1t = din('a1t', [FE, FE], BF16)
    v1ext = din('v1ext', [FE, H], BF16)
    ws1ext = din('ws1ext', [FE, H], BF16)
    a2t = din('a2t', [H, H], BF16)
    urow = din('urow', [1, H], BF16)
    v2w = din('v2w', [H, H], BF16)
    ws2 = din('ws2', [H, H], BF16)
    b2col = din('b2col', [H, 1], FP32)
    seqw = din('seqw', [SEQ, H], BF16)
    seqb = din('seqb', [1, H], BF16)
    fc1w = din('fc1w', [2 * H, H], BF16)
    fc1b = din('fc1b', [1, H], BF16)
    fc2w = din('fc2w', [H, 1], BF16)
    fc2b = din('fc2b', [1, 1], BF16)
    iota_in = din('iota', [P, P], BF16)
    seqT = din('seqT', [SEQ, GPC], BF16)
    kvidx = din('kvidx', [P, ntot * 8], I16)
    dstl = din('dstl', [P, ntot], FP32)
    glocal = din('glocal', [P, NB], FP32)

    out_g = nc.dram_tensor('out_g', [GPC, 1], FP32, kind="ExternalOutput").ap()
    if dbg:
        dbg_o = nc.dram_tensor('dbg_o', [P, NB * P], FP32,
                               kind="ExternalOutput").ap()

    h1_sh = nc.dram_tensor('h1_sh', [ncap, H], BF16, kind="Internal").ap()
    h1_all = nc.dram_tensor('h1_all', [NCORES * ncap, H], BF16,
                            kind="Internal", addr_space="Shared").ap()

    from contextlib import ExitStack
    with TileContext(nc, num_cores=NCORES) as tc, ExitStack() as _st:
        cpool = _st.enter_context(tc.tile_pool(name="consts", bufs=1))
        pool = _st.enter_context(tc.tile_pool(name="work", bufs=3))
        spool = _st.enter_context(tc.tile_pool(name="stage", bufs=5))
        wpool = _st.enter_context(tc.tile_pool(name="wts", bufs=16))
        persist = _st.enter_context(tc.tile_pool(name="persist", bufs=1))
        psS = _st.enter_context(tc.tile_pool(name="psS", bufs=3, space="PSUM"))
        psT = _st.enter_context(tc.tile_pool(name="psT", bufs=2, space="PSUM"))
        psB = _st.enter_context(tc.tile_pool(name="psB", bufs=3, space="PSUM"))

        # ---------------- constants
        iota = cpool.tile([P, P], BF16)
        nc.sync.dma_start(out=iota[:], in_=iota_in)
        ident = cpool.tile([P, P], BF16)
        make_identity(nc, ident[:])
        ones_col = cpool.tile([P, 1], BF16)
        nc.vector.memset(ones_col[:], 1.0)
        ones_row = cpool.tile([1, P], BF16)
        nc.vector.memset(ones_row[:], 1.0)
        ones_row_f = cpool.tile([1, P], FP32)
        nc.vector.memset(ones_row_f[:], 1.0)
        zero_row = cpool.tile([1, 2 * P], BF16)
        nc.vector.memset(zero_row[:], 0.0)

        _cn = [0]

        def const_tile(ap_, shape, dt=BF16):
            _cn[0] += 1
            t = cpool.tile(shape, dt, tag=f"c{_cn[0]}", name=f"c{_cn[0]}")
            nc.sync.dma_start(out=t[:], in_=ap_)
            return t

        a1t_t = const_tile(a1t, [FE, FE])
        v1ext_t = const_tile(v1ext, [FE, H])
        ws1ext_t = const_tile(ws1ext, [FE, H])
        a2t_t = const_tile(a2t, [H, H])
        urow_t = const_tile(urow, [1, H])
        v2w_t = const_tile(v2w, [H, H])
        ws2_t = const_tile(ws2, [H, H])
        b2col_t = const_tile(b2col, [H, 1], FP32)
        seqb_t = const_tile(seqb, [1, H])
        fc1b_t = const_tile(fc1b, [1, H])
        fc2w_t = const_tile(fc2w, [H, 1])
        fc2b_t = const_tile(fc2b, [1, 1])
        kvidx_t = const_tile(kvidx, [P, ntot * 8], I16)
        dstl_t = const_tile(dstl, [P, ntot], FP32)
        glocal_t = const_tile(glocal, [P, NB], FP32)
        xall = const_tile(xte_own, [FE, ncap])      # full own-x, column-major

        h1T_own = persist.tile([P, NB * P], BF16)   # h1 transposed, own shard

        # ---------------- B1 blocks: B1_b = A1 @ x_d^T  (score rhs, layer 1)
        B1_all = persist.tile([FE, NB * P], BF16)
        for b in range(NB):
            bp = psS.tile([FE, P], FP32, space="PSUM", tag="psS",
                          name=f"b1_{b}")
            nc.tensor.matmul(out=bp[:], lhsT=a1t_t[:],
                             rhs=xall[:, b * P:(b + 1) * P],
                             start=True, stop=True)
            if b % 2 == 0:
                nc.vector.tensor_copy(out=B1_all[:, b * P:(b + 1) * P],
                                      in_=bp[:])
            else:
                nc.scalar.copy(out=B1_all[:, b * P:(b + 1) * P], in_=bp[:])

        # ---------------- edge phase (shared for both layers)
        # Per-block PSUM bank "blk" [P, 4*P] f32:
        #   [:, 0:P]      attention aggregate (raw-feature space)
        #   [0:1, P:2P]   denominator row (layer 2 only; layer 1 uses agg row 64)
        #   [:, 2P:3P]    skip + value-projection accumulator
        #   [:, 3P:4P]    scratch (reciprocal broadcast)
        def edge_phase(layer, tab_lo_ap, tab_hi_ap, rhs_for_block,
                       skip_for_block, on_block_done):
            """rhs_for_block(b, blk) -> (rhs_tile_ap, cp) SBUF [cp,128] dst feats.
            skip_for_block(b, blk) opens blk[:, 2P:3P] accumulation with skip.
            on_block_done(b, h_sb, blk) with h_sb [128,128] bf16 = relu'd out^T."""
            _q = [0]
            _blks, _rhs = {}, {}
            cp_agg = FE if layer == 1 else H

            pend_T = []   # tiles awaiting transpose-copy: (psum_tile, j, tt)

            ptc = [0]

            def flush_T():
                if not pend_T:
                    return None
                pts, n = pend_T[0][0], len(pend_T)
                sb = spool.tile([P, TTB, P], BF16, tag="pT_sb")
                if ptc[0] % 2 == 0:
                    nc.vector.tensor_copy(out=sb[:, 0:n, :], in_=pts[:, 0:n, :])
                else:
                    nc.scalar.copy(out=sb[:, 0:n, :], in_=pts[:, 0:n, :])
                ptc[0] += 1
                del pend_T[:]
                return sb

            for (t0, lo, tsg, blocks) in sgs:
                kv_t = pool.tile([P, tsg, H], BF16, tag="kv_g", bufs=5)
                for (h0, h1, hh) in ((0, lo, 0), (lo, tsg, 1)):
                    for ps0 in range(h0, h1, GMAX):
                        pe0 = min(ps0 + GMAX, h1)
                        tab_h = tab_lo_ap if hh == 0 else tab_hi_ap
                        nc.gpsimd.dma_gather(
                            out_ap=kv_t[:, ps0:pe0, :], in_ap=tab_h,
                            idxs_ap=kvidx_t[:, (t0 + ps0) * 8:(t0 + pe0) * 8],
                            num_idxs=(pe0 - ps0) * P, num_idxs_reg=(pe0 - ps0) * P,
                            elem_size=H, queue_num=_q[0] % 4)
                        _q[0] += 1

                # pass A: transpose gathered rows (batches of TTB)
                pT_sbs = {}
                for tl in range(tsg):
                    if tl % TTB == 0:
                        psT_t = psT.tile([P, TTB, P], BF16, space="PSUM",
                                         tag="psT")
                    nc.tensor.transpose(
                        out=psT_t[:, tl % TTB, :], in_=kv_t[:, tl, :],
                        identity=ident[:])
                    pend_T.append((psT_t, tl % TTB, t0 + tl))
                    if tl % TTB == TTB - 1 or tl == tsg - 1:
                        sb = flush_T()
                        pT_sbs[tl // TTB] = sb

                # pass B: banded S matmuls packed into PSUM strips, exp per strip
                SW = P                         # strip width (quarter bank)
                tile_se = {}                   # tl -> (strip idx, off, w)
                strips = []                    # psum strip tiles
                strip_cols = []                # used cols per strip
                for tl in range(tsg):
                    tt = t0 + tl
                    b = int(tile_block[tt])
                    if b not in _blks:
                        blk = psB.tile([P, 4 * P], FP32, space="PSUM",
                                       tag="blk", name=f"blk{layer}_{b}")
                        _blks[b] = blk
                        # zero acc+den regions, open the accumulation group
                        nc.tensor.matmul(out=blk[:, 0:2 * P],
                                         lhsT=zero_row[:1, 0:P],
                                         rhs=zero_row[:1, :],
                                         start=True, stop=False)
                        _rhs[b] = rhs_for_block(b, blk)
                    rhs_sb, cp = _rhs[b]
                    dlo, w = bands[tt]
                    if not strips or strip_cols[-1] + w > SW:
                        st = psS.tile([P, SW], FP32, space="PSUM", tag="psS",
                                      name=f"st{layer}_{tt}")
                        strips.append(st)
                        strip_cols.append(0)
                    off = strip_cols[-1]
                    strip_cols[-1] += w
                    tile_se[tl] = (len(strips) - 1, off, w)
                    pT_sb = pT_sbs[tl // TTB]
                    nc.tensor.matmul(
                        out=strips[-1][:, off:off + w],
                        lhsT=pT_sb[0:cp, tl % TTB, :],
                        rhs=rhs_sb[:, dlo:dlo + w],
                        start=True, stop=True)

                E_sbs = []
                for si, st in enumerate(strips):
                    esb = spool.tile([P, SW], BF16, tag="E_sb",
                                     name=f"esb{layer}_{t0}_{si}")
                    used = strip_cols[si]
                    nc.scalar.activation(out=esb[:, 0:used], in_=st[:, 0:used],
                                         func=AF.Exp, scale=float(SCALE))
                    E_sbs.append(esb)

                for tl in range(tsg):
                    tt = t0 + tl
                    b = int(tile_block[tt])
                    si, off, w = tile_se[tl]
                    E_sb = E_sbs[si]
                    W = wpool.tile([P, P], BF16, tag="W")
                    nc.vector.scalar_tensor_tensor(
                        out=W[:, 0:w], in0=iota[:, 0:w],
                        scalar=dstl_t[:, tt:tt + 1],
                        in1=E_sb[:, off:off + w],
                        op0=OP.is_equal, op1=OP.mult)
                    blk = _blks[b]
                    dlo = bands[tt][0]
                    nc.tensor.matmul(
                        out=blk[0:cp_agg, dlo:dlo + w],
                        lhsT=kv_t[:, tl, 0:cp_agg],
                        rhs=W[:, 0:w], start=False, stop=False)
                    if layer == 2:
                        nc.tensor.matmul(
                            out=blk[0:1, P + dlo:P + dlo + w], lhsT=ones_col[:],
                            rhs=W[:, 0:w], start=False, stop=False)
                    if tt == blk_last[b]:
                        # close the accumulation group (flush)
                        nc.tensor.matmul(out=blk[:, 0:2 * P],
                                         lhsT=zero_row[:1, 0:P],
                                         rhs=zero_row[:1, :],
                                         start=False, stop=True)
                        # epilogue: normalize + project + skip + relu
                        _blks.pop(b)
                        _rhs.pop(b)
                        den_ap = (blk[F_IN:FE, 0:P] if layer == 1
                                  else blk[0:1, P:2 * P])
                        dv = pool.tile([1, P], FP32, tag="dv")
                        nc.vector.tensor_scalar_add(out=dv[:], in0=den_ap,
                                                    scalar1=EPS)
                        rv = pool.tile([1, P], FP32, tag="rv")
                        nc.vector.reciprocal(out=rv[:], in_=dv[:])
                        nc.tensor.matmul(out=blk[0:cp_agg, 3 * P:4 * P],
                                         lhsT=ones_row_f[:1, 0:cp_agg],
                                         rhs=rv[:], start=True, stop=True)
                        dnb = pool.tile([cp_agg, P], FP32, tag="dnb",
                                        name=f"dn{layer}_{b}")
                        nc.scalar.copy(out=dnb[:], in_=blk[0:cp_agg, 3 * P:4 * P])
                        xnorm = pool.tile([cp_agg, P], BF16, tag="xnorm",
                                          name=f"xn{layer}_{b}")
                        nc.vector.tensor_tensor(
                            out=xnorm[:], in0=blk[0:cp_agg, 0:P],
                            in1=dnb[:], op=OP.mult)
                        # skip + value projection accumulate back-to-back
                        skip_for_block(b, blk)
                        vproj = v1ext_t if layer == 1 else v2w_t
                        nc.tensor.matmul(out=blk[:, 2 * P:3 * P],
                                         lhsT=vproj[:], rhs=xnorm[:],
                                         start=False, stop=True)
                        h_sb = spool.tile([P, P], BF16, tag="h_sb")
                        if layer == 1:
                            nc.scalar.activation(out=h_sb[:],
                                                 in_=blk[:, 2 * P:3 * P],
                                                 func=AF.Relu)
                        else:
                            nc.scalar.activation(out=h_sb[:],
                                                 in_=blk[:, 2 * P:3 * P],
                                                 func=AF.Relu,
                                                 bias=b2col_t[:, 0:1])
                        on_block_done(b, h_sb, blk)

        # ---------------- layer 1 plumbing
        def rhs1(b, blk):
            return (B1_all[:, b * P:(b + 1) * P], FE)

        def skip1(b, blk):
            nc.tensor.matmul(out=blk[:, 2 * P:3 * P], lhsT=ws1ext_t[:],
                             rhs=xall[:, b * P:(b + 1) * P],
                             start=True, stop=False)

        def l1_done(b, h_sb, blk):
            nc.vector.tensor_copy(out=h1T_own[:, b * P:(b + 1) * P], in_=h_sb[:])
            # node-major h1 rows for the allgather + layer-2 gathers
            tp = psT.tile([P, P], BF16, space="PSUM", tag="psT",
                          name=f"h1tp{b}")
            nc.tensor.transpose(out=tp[:], in_=h_sb[:], identity=ident[:])
            tsb = spool.tile([P, H], BF16, tag="t2row")
            nc.scalar.copy(out=tsb[:], in_=tp[:])
            nc.sync.dma_start(out=h1_sh[b * P:(b + 1) * P, :], in_=tsb[:])

        edge_phase(1, xrows_lo, xrows_hi, rhs1, skip1, l1_done)

        if dbg == 'h1':
            d = pool.tile([P, NB * P], FP32, tag="dbg")
            nc.vector.tensor_copy(out=d[:], in_=h1T_own[:])
            nc.sync.dma_start(out=dbg_o, in_=d[:])
            do = pool.tile([GPC, 1], FP32, tag="dbgo")
            nc.vector.memset(do[:], 0.5)
            nc.sync.dma_start(out=out_g, in_=do[:])

        # ---------------- collective: allgather raw h1
        nc.gpsimd.collective_compute(
            kind="AllGather", op=OP.bypass,
            replica_groups=[list(range(NCORES))],
            ins=[h1_sh], outs=[h1_all])

        if dbg is None or dbg == 'full':
            # ---------------- layer 2 plumbing
            pool_sb = persist.tile([GPC, H + 1], FP32)
            nc.vector.memset(pool_sb[:], 0.0)

            # score rhs blocks during the collective: R_b = A2 h1_d^T + u
            R_all = persist.tile([H, NB * P], BF16)
            for b in range(NB):
                rp = psS.tile([H, P], FP32, space="PSUM", tag="psS",
                              name=f"r2_{b}")
                nc.tensor.matmul(out=rp[:], lhsT=a2t_t[:],
                                 rhs=h1T_own[:, b * P:(b + 1) * P],
                                 start=True, stop=False)
                nc.tensor.matmul(out=rp[:], lhsT=urow_t[:1, :],
                                 rhs=ones_row[:1, :], start=False, stop=True)
                if b % 2 == 0:
                    nc.vector.tensor_copy(out=R_all[:, b * P:(b + 1) * P],
                                          in_=rp[:])
                else:
                    nc.scalar.copy(out=R_all[:, b * P:(b + 1) * P], in_=rp[:])

            def rhs2(b, blk):
                return (R_all[:, b * P:(b + 1) * P], H)

            def skip2(b, blk):
                nc.tensor.matmul(out=blk[:, 2 * P:3 * P], lhsT=ws2_t[:],
                                 rhs=h1T_own[:, b * P:(b + 1) * P],
                                 start=True, stop=False)

            def l2_done(b, h_sb, blk):
                # transpose h2^T -> h2 [d, h], then pool matmul
                tp = psT.tile([P, P], BF16, space="PSUM", tag="psT",
                              name=f"h2tp{b}")
                nc.tensor.transpose(out=tp[:], in_=h_sb[:], identity=ident[:])
                h2x = pool.tile([P, H + 1], BF16, tag="h2x")
                nc.scalar.copy(out=h2x[:, 0:H], in_=tp[:])
                nc.vector.memset(h2x[:, H:H + 1], 1.0)
                gh = pool.tile([P, GPC], BF16, tag="gh")
                nc.vector.tensor_scalar(
                    out=gh[:], in0=iota[:, 0:GPC], scalar1=glocal_t[:, b:b + 1],
                    scalar2=None, op0=OP.is_equal)
                nc.tensor.matmul(out=blk[0:GPC, 2 * P:2 * P + H + 1],
                                 lhsT=gh[:], rhs=h2x[:],
                                 start=True, stop=True)
                nc.vector.tensor_tensor(out=pool_sb[:], in0=pool_sb[:],
                                        in1=blk[0:GPC, 2 * P:2 * P + H + 1],
                                        op=OP.add)

            # seq branch computed during the collective window
            seqw_t0 = const_tile(seqw[0:P, :], [P, H])
            seqw_t1 = const_tile(seqw[P:SEQ, :], [P, H])
            fc1w_t0 = const_tile(fc1w[0:P, :], [P, H])
            fc1w_t1 = const_tile(fc1w[P:2 * H, :], [P, H])
            seqT0 = const_tile(seqT[0:P, :], [P, GPC])
            seqT1 = const_tile(seqT[P:SEQ, :], [P, GPC])
            z = pool.tile([GPC, 2 * H], BF16, tag="z")
            pseq = psS.tile([GPC, H], FP32, space="PSUM", tag="psS",
                            name="pseq")
            nc.tensor.matmul(out=pseq[:], lhsT=seqT0[:], rhs=seqw_t0[:],
                             start=True, stop=False)
            nc.tensor.matmul(out=pseq[:], lhsT=seqT1[:], rhs=seqw_t1[:],
                             start=False, stop=False)
            nc.tensor.matmul(out=pseq[:], lhsT=ones_row[:1, 0:GPC],
                             rhs=seqb_t[:1, :], start=False, stop=True)
            nc.scalar.activation(out=z[:, H:2 * H], in_=pseq[:], func=AF.Relu)

            edge_phase(2, h1_all[0:split, :], h1_all[split:NCORES * ncap, :],
                       rhs2, skip2, l2_done)

            # ---------------- tail: pooled -> MLP -> sigmoid
            cnt = pool.tile([GPC, 1], FP32, tag="cnt")
            nc.vector.tensor_scalar_add(out=cnt[:], in0=pool_sb[:, H:H + 1],
                                        scalar1=EPS)
            rc = pool.tile([GPC, 1], FP32, tag="rc")
            nc.vector.reciprocal(out=rc[:], in_=cnt[:])
            nc.vector.tensor_scalar(out=z[:, 0:H], in0=pool_sb[:, 0:H],
                                    scalar1=rc[:, 0:1], scalar2=None,
                                    op0=OP.mult)

            zT = []
            for i in range(2):
                tzp = psT.tile([P, GPC], BF16, space="PSUM", tag="psT",
                               name=f"tzp{i}")
                nc.tensor.transpose(out=tzp[:], in_=z[:, i * H:(i + 1) * H],
                                    identity=ident[0:GPC, 0:GPC])
                zt = pool.tile([P, GPC], BF16, tag=f"zT{i}")
                nc.vector.tensor_copy(out=zt[:], in_=tzp[:])
                zT.append(zt)
            pfc1 = psS.tile([GPC, H], FP32, space="PSUM", tag="psS",
                            name="pfc1")
            nc.tensor.matmul(out=pfc1[:], lhsT=zT[0][:], rhs=fc1w_t0[:],
                             start=True, stop=False)
            nc.tensor.matmul(out=pfc1[:], lhsT=zT[1][:], rhs=fc1w_t1[:],
                             start=False, stop=False)
            nc.tensor.matmul(out=pfc1[:], lhsT=ones_row[:1, 0:GPC],
                             rhs=fc1b_t[:1, :], start=False, stop=True)
            z1 = pool.tile([GPC, H], BF16, tag="z1")
            nc.scalar.activation(out=z1[:], in_=pfc1[:], func=AF.Relu)
            tz1 = psT.tile([P, GPC], BF16, space="PSUM", tag="psT",
                           name="tz1")
            nc.tensor.transpose(out=tz1[:], in_=z1[:],
                                identity=ident[0:GPC, 0:GPC])
            z1T = pool.tile([P, GPC], BF16, tag="z1T")
            nc.vector.tensor_copy(out=z1T[:], in_=tz1[:])
            pfc2 = psS.tile([GPC, 1], FP32, space="PSUM", tag="psS",
                            name="pfc2")
            nc.tensor.matmul(out=pfc2[:], lhsT=z1T[:], rhs=fc2w_t[:],
                             start=True, stop=False)
            nc.tensor.matmul(out=pfc2[:], lhsT=ones_row[:1, 0:GPC],
                             rhs=fc2b_t[:1, :], start=False, stop=True)
            outs = pool.tile([GPC, 1], FP32, tag="outs")
            nc.scalar.activation(out=outs[:], in_=pfc2[:], func=AF.Sigmoid)
            nc.sync.dma_start(out=out_g, in_=outs[:])

    nc.compile()
    return nc


# ---------------------------------------------------------------- entry

_CACHE = {}


def kernel(**inputs):
    meta = preprocess(inputs)
    key = (meta['ncap'], meta['ntot'], tuple(meta['tile_block'].tolist()))
    if key not in _CACHE:
        _CACHE[key] = build_program(meta)
    nc = _CACHE[key]
    in_maps = make_inputs(inputs, meta)
    res = run_bass_kernel_spmd(nc, in_maps, core_ids=list(range(NCORES)))
    out = np.concatenate([res.results[c]['out_g'] for c in range(NCORES)], 0)
    return out.astype(np.float32)


# revision 7
# speedup vs baseline: 1.2325x; 1.1792x over previous
"""Trainium2 Bass kernel for nn_Discriminator (2x TransformerConv GNN + pool + MLP).

v3 design (raw-feature gathers, no node-table prologue):
- Graphs split 64-per-core (batch sorted => contiguous node ranges per core).
- L1 gathers RAW x_ext rows ([x|1|0pad] -> 128 bf16 cols, 256B) straight from
  a host-built row table; the score projection A1 is applied per dst-block
  (B1_b = A1 @ x_d^T), and the value projection v1 is applied per block AFTER
  aggregation (linearity). The ones column of x_ext yields the softmax
  denominator for free in the aggregation matmul.
- L2 allgathers RAW h1 ([N,128] bf16, half the bytes of a v|p table) and
  applies A2/u per block (R_b = A2 h1_d + u) and v2 after aggregation.
- Per tile: PE transpose of the gathered rows, S = X_sT^T @ B_b (PE), batched
  exp (scalar engine), W = onehot(dst) * exp(S) in one DVE op, aggregation
  via one PE matmul (plus a denominator matmul for L2 only).

Self-contained: hardcodes problem shapes; layout computed from runtime inputs.
"""
import numpy as np
import ml_dtypes

import concourse.bass as bass
import concourse.bacc as bacc
import concourse.mybir as mybir
from concourse.tile import TileContext
from concourse.masks import make_identity
from concourse.bass_utils import run_bass_kernel_spmd

BF = np.float16
N, E, G = 50000, 800000, 512
F_IN, H, SEQ = 64, 128, 256
FE = F_IN + 1                 # x extended with ones column
NCORES = 8
GPC = G // NCORES             # graphs per core
P = 128
SGB = 2                       # blocks per supergroup
GMAX = 8                      # tiles per gather call (1024 idxs; runtime ucode cap)
TTB = 8                       # tiles per transpose/copy batch (one bf16 bank)
SCALE = 1.0 / np.sqrt(np.float32(H))
EPS = 1e-30

FP32 = mybir.dt.float32
BF16 = mybir.dt.float16
I16 = mybir.dt.int16
AF = mybir.ActivationFunctionType
OP = mybir.AluOpType


# ---------------------------------------------------------------- host prep

def _pack_idx(idx_stream):
    """idx_stream [ntot*128] -> [128, ntot*8] int16 (16-partition wrap, x8)."""
    n = idx_stream.shape[0]
    s = n // 16
    out = np.zeros((128, s), dtype=np.int16)
    arr = idx_stream.reshape(s, 16).T.astype(np.int16)
    for g in range(8):
        out[g * 16:(g + 1) * 16, :] = arr
    return out


def preprocess(inputs):
    batch = np.asarray(inputs['batch']).astype(np.int64)
    ei = np.asarray(inputs['edge_index']).astype(np.int64)
    src_g, dst_g = ei[0], ei[1]

    gstart = np.searchsorted(batch, np.arange(NCORES) * GPC)
    gend = np.searchsorted(batch, np.arange(NCORES) * GPC + GPC)
    nloc = gend - gstart
    ncap = int(np.ceil(nloc.max() / (2 * P)) * (2 * P))   # even block count
    NB = ncap // P
    split = (NCORES // 2) * ncap          # lo/hi table split row

    node_core = batch // GPC
    node_local = np.arange(N) - gstart[node_core]
    table_idx = node_core * ncap + node_local            # row in [8*ncap] table
    node_half = (table_idx >= split).astype(np.int64)
    half_idx = table_idx - node_half * split             # row within half

    edge_core = node_core[dst_g]
    per_core = []
    for c in range(NCORES):
        em = np.where(edge_core == c)[0]
        e_src, e_dst = src_g[em], dst_g[em]
        order = np.argsort(e_dst, kind='stable')
        e_src, e_dst = e_src[order], e_dst[order]
        dst_loc = e_dst - gstart[c]
        tsrc = half_idx[e_src]
        is_hi = node_half[e_src]
        blk = dst_loc // P
        buckets = {}
        for b in range(NB):
            bm = np.where(blk == b)[0]
            bh = is_hi[bm]
            for half in (0, 1):
                hm = bm[bh == half]
                buckets[(b, half)] = (tsrc[hm], dst_loc[hm] - b * P)
        per_core.append(buckets)

    # uniform tile counts per (b, half)
    tcount = {}
    for b in range(NB):
        for half in (0, 1):
            mx = max(len(per_core[c][(b, half)][0]) for c in range(NCORES))
            tcount[(b, half)] = (mx + P - 1) // P

    # supergroups and stream layout (shared across cores)
    sgs = []           # (t0, Tlo, Tsg, blocks)
    tile_block = []    # per tile: block id
    t0 = 0
    for s0 in range(0, NB, SGB):
        blocks = list(range(s0, min(s0 + SGB, NB)))
        lo = sum(tcount[(b, 0)] for b in blocks)
        hi = sum(tcount[(b, 1)] for b in blocks)
        for half in (0, 1):
            for b in blocks:
                tile_block += [b] * tcount[(b, half)]
        sgs.append((t0, lo, lo + hi, blocks))
        t0 += lo + hi
    ntot = t0
    tile_block = np.array(tile_block)
    blk_first = {b: int(np.where(tile_block == b)[0][0]) for b in range(NB)}
    blk_last = {b: int(np.where(tile_block == b)[0][-1]) for b in range(NB)}

    # per-core streams
    dls = []
    kvis = []
    for c in range(NCORES):
        kvi = np.zeros(ntot * P, np.int64)
        dl = np.full(ntot * P, -1.0, np.float32)
        pos = 0
        for (t0_, lo, tsg, blocks) in sgs:
            for half in (0, 1):
                for b in blocks:
                    k, d = per_core[c][(b, half)]
                    ntile = tcount[(b, half)]
                    cnt = len(k)
                    kvi[pos:pos + cnt] = k
                    dl[pos:pos + cnt] = d
                    pos += ntile * P
        assert pos == ntot * P
        dls.append(dl.reshape(ntot, P))
        kvis.append(kvi)

    # per-tile dst band: union across cores (program structure is shared)
    bands = []
    for tt in range(ntot):
        lo128, hi = P, -1
        for c in range(NCORES):
            v = dls[c][tt]
            v = v[v >= 0]
            if len(v):
                lo128 = min(lo128, int(v.min()))
                hi = max(hi, int(v.max()))
        if hi < 0:
            lo128, hi = 0, 0
        bands.append((lo128, hi - lo128 + 1))

    cores = []
    for c in range(NCORES):
        dl = dls[c]
        dlb = dl.copy()
        for tt in range(ntot):
            m = dlb[tt] >= 0
            dlb[tt][m] -= bands[tt][0]
        gl = np.full(ncap, -1.0, np.float32)
        gl[:nloc[c]] = (batch[gstart[c]:gend[c]] - c * GPC).astype(np.float32)
        cores.append({
            'kvidx': _pack_idx(kvis[c]),
            'dstl': dlb.T.astype(np.float32),                  # [128, ntot]
            'glocal': gl.reshape(NB, P).T.astype(np.float32),  # [128, NB]
        })

    return {
        'ncap': ncap, 'NB': NB, 'split': split, 'ntot': ntot,
        'gstart': gstart, 'gend': gend, 'nloc': nloc,
        'table_idx': table_idx, 'sgs': sgs,
        'tile_block': tile_block, 'blk_first': blk_first, 'blk_last': blk_last,
        'bands': bands, 'cores': cores,
    }


def make_inputs(inputs, meta):
    ncap = meta['ncap']
    x = np.asarray(inputs['x'], np.float32)
    f32 = lambda v: np.asarray(v, np.float32)

    # raw x_ext row table [8*ncap, 128]: [x | 1 | 0pad], gathered by L1
    xrows = np.zeros((NCORES * ncap, P), np.float32)
    xrows[meta['table_idx'], 0:F_IN] = x
    xrows[:, F_IN] = 1.0
    xrows_bf = xrows.astype(BF)

    # column-major x_ext per core (dst side): [FE, ncap]
    xte = np.zeros((FE, NCORES * ncap), np.float32)
    xte[F_IN, :] = 1.0
    xte[:F_IN, meta['table_idx']] = x.T
    xte_bf = xte.astype(BF)

    # layer-1: score = x_ext[s] A1 x_ext[d]^T; A1 = wk1 wq1^T
    wk1 = np.concatenate([f32(inputs['k1_w']), f32(inputs['k1_b'])[None, :]], 0)
    wq1 = np.concatenate([f32(inputs['q1_w']), f32(inputs['q1_b'])[None, :]], 0)
    A1 = wk1 @ wq1.T                                     # [FE, FE]
    v1ext = np.zeros((FE, H), np.float32)
    v1ext[:F_IN, :] = f32(inputs['v1_w'])
    v1ext[F_IN, :] = f32(inputs['v1_b'])
    ws1ext = np.zeros((FE, H), np.float32)
    ws1ext[:F_IN, :] = f32(inputs['s1_w'])
    ws1ext[F_IN, :] = f32(inputs['s1_b'])

    # layer-2: score = h1[s] A2 h1[d] + u.h1[s] (+ dst-only terms cancel)
    A2 = f32(inputs['k2_w']) @ f32(inputs['q2_w']).T     # [H, H]
    u = f32(inputs['k2_w']) @ f32(inputs['q2_b'])        # [H]
    b2col = (f32(inputs['s2_b']) + f32(inputs['v2_b']))[:, None]  # [H,1]

    seqc = np.asarray(inputs['sequence_character'], np.float32)
    split = meta['split']

    shared = {
        'xrows_lo': np.ascontiguousarray(xrows_bf[:split]),
        'xrows_hi': np.ascontiguousarray(xrows_bf[split:]),
        'a1t': np.ascontiguousarray(A1.T.astype(BF)),
        'v1ext': v1ext.astype(BF),
        'ws1ext': ws1ext.astype(BF),
        'a2t': np.ascontiguousarray(A2.T.astype(BF)),
        'urow': u[None, :].astype(BF),
        'v2w': f32(inputs['v2_w']).astype(BF),
        'ws2': f32(inputs['s2_w']).astype(BF),
        'b2col': b2col.astype(np.float32),
        'seqw': f32(inputs['seq_w']).astype(BF),
        'seqb': f32(inputs['seq_b'])[None, :].astype(BF),
        'fc1w': f32(inputs['fc1_w']).astype(BF),
        'fc1b': f32(inputs['fc1_b'])[None, :].astype(BF),
        'fc2w': f32(inputs['fc2_w']).astype(BF),
        'fc2b': f32(inputs['fc2_b'])[None, :].astype(BF),
        'iota': np.tile(np.arange(P, dtype=np.float32)[None, :], (P, 1)).astype(BF),
    }

    in_maps = []
    for c in range(NCORES):
        m = dict(shared)
        m['xte_own'] = np.ascontiguousarray(xte_bf[:, c * ncap:(c + 1) * ncap])
        m['seqT'] = np.ascontiguousarray(
            seqc[c * GPC:(c + 1) * GPC].T.astype(BF))            # [256, 64]
        mc = meta['cores'][c]
        m['kvidx'] = mc['kvidx']
        m['dstl'] = mc['dstl']
        m['glocal'] = mc['glocal']
        in_maps.append(m)
    return in_maps


# ---------------------------------------------------------------- program

def build_program(meta, dbg=None):
    ncap, NB, ntot = meta['ncap'], meta['NB'], meta['ntot']
    split = meta['split']
    sgs = meta['sgs']
    bands = meta['bands']
    tile_block = meta['tile_block']
    blk_first, blk_last = meta['blk_first'], meta['blk_last']

    nc = bacc.Bacc("TRN2", target_bir_lowering=False, debug=False,
                   enable_asserts=False, num_devices=NCORES,
                   num_swdge_queues=4)

    def din(name, shape, dt):
        return nc.dram_tensor(name, shape, dt, kind="ExternalInput").ap()

    xrows_lo = din('xrows_lo', [split, P], BF16)
    xrows_hi = din('xrows_hi', [NCORES * ncap - split, P], BF16)
    xte_own = din('xte_own', [FE, ncap], BF16)
    a1t = din('a1t', [FE, FE], BF16)
    v1ext = din('v1ext', [FE, H], BF16)
    ws1ext = din('ws1ext', [FE, H], BF16)
    a2t = din('a2t', [H, H], BF16)
    urow = din('urow', [1, H], BF16)
    v2w = din('v2w', [H, H], BF16)
    ws2 = din('ws2', [H, H], BF16)
    b2col = din('b2col', [H, 1], FP32)
    seqw = din('seqw', [SEQ, H], BF16)
    seqb = din('seqb', [1, H], BF16)
    fc1w = din('fc1w', [2 * H, H], BF16)
    fc1b = din('fc1b', [1, H], BF16)
    fc2w = din('fc2w', [H, 1], BF16)
    fc2b = din('fc2b', [1, 1], BF16)
    iota_in = din('iota', [P, P], BF16)
    seqT = din('seqT', [SEQ, GPC], BF16)
    kvidx = din('kvidx', [P, ntot * 8], I16)
    dstl = din('dstl', [P, ntot], FP32)
    glocal = din('glocal', [P, NB], FP32)

    out_g = nc.dram_tensor('out_g', [GPC, 1], FP32, kind="ExternalOutput").ap()
    if dbg:
        dbg_o = nc.dram_tensor('dbg_o', [P, NB * P], FP32,
                               kind="ExternalOutput").ap()

    h1_sh = nc.dram_tensor('h1_sh', [ncap, H], BF16, kind="Internal").ap()
    h1_all = nc.dram_tensor('h1_all', [NCORES * ncap, H], BF16,
                            kind="Internal", addr_space="Shared").ap()

    from contextlib import ExitStack
    with TileContext(nc, num_cores=NCORES) as tc, ExitStack() as _st:
        cpool = _st.enter_context(tc.tile_pool(name="consts", bufs=1))
        pool = _st.enter_context(tc.tile_pool(name="work", bufs=3))
        spool = _st.enter_context(tc.tile_pool(name="stage", bufs=5))
        epool = _st.enter_context(tc.tile_pool(name="exps", bufs=4))
        wpool = _st.enter_context(tc.tile_pool(name="wts", bufs=16))
        persist = _st.enter_context(tc.tile_pool(name="persist", bufs=1))
        psS = _st.enter_context(tc.tile_pool(name="psS", bufs=3, space="PSUM"))
        psT = _st.enter_context(tc.tile_pool(name="psT", bufs=2, space="PSUM"))
        psB = _st.enter_context(tc.tile_pool(name="psB", bufs=3, space="PSUM"))

        # ---------------- constants
        iota = cpool.tile([P, P], BF16)
        nc.sync.dma_start(out=iota[:], in_=iota_in)
        ident = cpool.tile([P, P], BF16)
        make_identity(nc, ident[:])
        ones_col = cpool.tile([P, 1], BF16)
        nc.vector.memset(ones_col[:], 1.0)
        ones_row = cpool.tile([1, P], BF16)
        nc.vector.memset(ones_row[:], 1.0)
        ones_row_f = cpool.tile([1, P], FP32)
        nc.vector.memset(ones_row_f[:], 1.0)
        zero_row = cpool.tile([1, 2 * P], BF16)
        nc.vector.memset(zero_row[:], 0.0)

        _cn = [0]

        def const_tile(ap_, shape, dt=BF16):
            _cn[0] += 1
            t = cpool.tile(shape, dt, tag=f"c{_cn[0]}", name=f"c{_cn[0]}")
            nc.sync.dma_start(out=t[:], in_=ap_)
            return t

        a1t_t = const_tile(a1t, [FE, FE])
        v1ext_t = const_tile(v1ext, [FE, H])
        ws1ext_t = const_tile(ws1ext, [FE, H])
        a2t_t = const_tile(a2t, [H, H])
        urow_t = const_tile(urow, [1, H])
        v2w_t = const_tile(v2w, [H, H])
        ws2_t = const_tile(ws2, [H, H])
        b2col_t = const_tile(b2col, [H, 1], FP32)
        seqb_t = const_tile(seqb, [1, H])
        fc1b_t = const_tile(fc1b, [1, H])
        fc2w_t = const_tile(fc2w, [H, 1])
        fc2b_t = const_tile(fc2b, [1, 1])
        kvidx_t = const_tile(kvidx, [P, ntot * 8], I16)
        dstl_t = const_tile(dstl, [P, ntot], FP32)
        glocal_t = const_tile(glocal, [P, NB], FP32)
        xall = const_tile(xte_own, [FE, ncap])      # full own-x, column-major

        h1T_own = persist.tile([P, NB * P], BF16)   # h1 transposed, own shard

        # ---------------- B1 blocks: B1_b = A1 @ x_d^T  (score rhs, layer 1)
        B1_all = persist.tile([FE, NB * P], BF16)
        for b in range(NB):
            bp = psS.tile([FE, P], FP32, space="PSUM", tag="psS",
                          name=f"b1_{b}")
            nc.tensor.matmul(out=bp[:], lhsT=a1t_t[:],
                             rhs=xall[:, b * P:(b + 1) * P],
                             start=True, stop=True)
            if b % 2 == 0:
                nc.vector.tensor_copy(out=B1_all[:, b * P:(b + 1) * P],
                                      in_=bp[:])
            else:
                nc.scalar.copy(out=B1_all[:, b * P:(b + 1) * P], in_=bp[:])

        # ---------------- edge phase (shared for both layers)
        # Per-block PSUM bank "blk" [P, 4*P] f32:
        #   [:, 0:P]      attention aggregate (raw-feature space)
        #   [0:1, P:2P]   denominator row (layer 2 only; layer 1 uses agg row 64)
        #   [:, 2P:3P]    skip + value-projection accumulator
        #   [:, 3P:4P]    scratch (reciprocal broadcast)
        def edge_phase(layer, tab_lo_ap, tab_hi_ap, rhs_for_block,
                       skip_for_block, on_block_done):
            """rhs_for_block(b) -> (rhs_tile_ap, cp) SBUF [cp,128] dst feats.
            skip_for_block(b, blk) opens blk[:, 2P:3P] accumulation with skip.
            on_block_done(b, h_sb, blk) with h_sb [128,128] = relu'd out^T.

            Software-pipelined with a one-supergroup skew: iteration k emits
            supergroup k's gather/transpose/S/exp (front) and supergroup k-1's
            W/aggregate/epilogue (back), so the in-order engines stream without
            cross-stage stalls."""
            _q = [0]
            _blks = {}
            cp_agg = FE if layer == 1 else H
            SW = 4 * P                     # strip width (full PSUM bank)
            nsg = len(sgs)
            stash = [None] * nsg

            def front(k):
                (t0, lo, tsg, blocks) = sgs[k]
                kv_t = pool.tile([P, tsg, H], BF16, tag="kv_g", bufs=5)
                for (h0, h1, hh) in ((0, lo, 0), (lo, tsg, 1)):
                    for ps0 in range(h0, h1, GMAX):
                        pe0 = min(ps0 + GMAX, h1)
                        tab_h = tab_lo_ap if hh == 0 else tab_hi_ap
                        nc.gpsimd.dma_gather(
                            out_ap=kv_t[:, ps0:pe0, :], in_ap=tab_h,
                            idxs_ap=kvidx_t[:, (t0 + ps0) * 8:(t0 + pe0) * 8],
                            num_idxs=(pe0 - ps0) * P, num_idxs_reg=(pe0 - ps0) * P,
                            elem_size=H, queue_num=_q[0] % 4)
                        _q[0] += 1

                # transposes (PE) + batch copies (scalar engine)
                pT_sbs = {}
                for j0 in range(0, tsg, TTB):
                    j1 = min(j0 + TTB, tsg)
                    psT_t = psT.tile([P, TTB, P], BF16, space="PSUM",
                                     tag="psT")
                    for tl in range(j0, j1):
                        nc.tensor.transpose(
                            out=psT_t[:, tl - j0, :], in_=kv_t[:, tl, :],
                            identity=ident[:])
                    sb = spool.tile([P, TTB, P], BF16, tag="pT_sb")
                    nc.scalar.copy(out=sb[:, 0:j1 - j0, :],
                                   in_=psT_t[:, 0:j1 - j0, :])
                    pT_sbs[j0 // TTB] = sb

                # banded S matmuls packed into full-bank strips + exp (Act)
                tile_se = {}
                strips, strip_cols = [], []
                for tl in range(tsg):
                    tt = t0 + tl
                    b = int(tile_block[tt])
                    rhs_sb, cp = rhs_for_block(b)
                    dlo, w = bands[tt]
                    if not strips or strip_cols[-1] + w > SW:
                        st = psS.tile([P, SW], FP32, space="PSUM", tag="psS",
                                      name=f"st{layer}_{tt}")
                        strips.append(st)
                        strip_cols.append(0)
                    off = strip_cols[-1]
                    strip_cols[-1] += w
                    tile_se[tl] = (len(strips) - 1, off, w)
                    pT_sb = pT_sbs[tl // TTB]
                    nc.tensor.matmul(
                        out=strips[-1][:, off:off + w],
                        lhsT=pT_sb[0:cp, tl % TTB, :],
                        rhs=rhs_sb[:, dlo:dlo + w],
                        start=True, stop=True)

                E_sbs = []
                for si, st in enumerate(strips):
                    esb = epool.tile([P, SW], BF16, tag="E_sb",
                                     name=f"esb{layer}_{t0}_{si}")
                    used = strip_cols[si]
                    nc.scalar.activation(out=esb[:, 0:used], in_=st[:, 0:used],
                                         func=AF.Exp, scale=float(SCALE))
                    E_sbs.append(esb)
                stash[k] = (t0, tsg, kv_t, tile_se, E_sbs)

            def back(k):
                (t0, tsg, kv_t, tile_se, E_sbs) = stash[k]
                stash[k] = None
                for tl in range(tsg):
                    tt = t0 + tl
                    b = int(tile_block[tt])
                    si, off, w = tile_se[tl]
                    E_sb = E_sbs[si]
                    if b not in _blks:
                        blk = psB.tile([P, 4 * P], FP32, space="PSUM",
                                       tag="blk", name=f"blk{layer}_{b}")
                        _blks[b] = blk
                        # zero acc+den regions, open the accumulation group
                        nc.tensor.matmul(out=blk[:, 0:2 * P],
                                         lhsT=zero_row[:1, 0:P],
                                         rhs=zero_row[:1, :],
                                         start=True, stop=False)
                    W = wpool.tile([P, P], BF16, tag="W")
                    nc.vector.scalar_tensor_tensor(
                        out=W[:, 0:w], in0=iota[:, 0:w],
                        scalar=dstl_t[:, tt:tt + 1],
                        in1=E_sb[:, off:off + w],
                        op0=OP.is_equal, op1=OP.mult)
                    blk = _blks[b]
                    dlo = bands[tt][0]
                    nc.tensor.matmul(
                        out=blk[0:cp_agg, dlo:dlo + w],
                        lhsT=kv_t[:, tl, 0:cp_agg],
                        rhs=W[:, 0:w], start=False, stop=False)
                    if layer == 2:
                        nc.tensor.matmul(
                            out=blk[0:1, P + dlo:P + dlo + w], lhsT=ones_col[:],
                            rhs=W[:, 0:w], start=False, stop=False)
                    if tt == blk_last[b]:
                        # close the accumulation group (flush)
                        nc.tensor.matmul(out=blk[:, 0:2 * P],
                                         lhsT=zero_row[:1, 0:P],
                                         rhs=zero_row[:1, :],
                                         start=False, stop=True)
                        # epilogue: normalize + project + skip + relu
                        _blks.pop(b)
                        den_ap = (blk[F_IN:FE, 0:P] if layer == 1
                                  else blk[0:1, P:2 * P])
                        dv = pool.tile([1, P], FP32, tag="dv")
                        nc.vector.tensor_scalar_add(out=dv[:], in0=den_ap,
                                                    scalar1=EPS)
                        rv = pool.tile([1, P], FP32, tag="rv")
                        nc.vector.reciprocal(out=rv[:], in_=dv[:])
                        nc.tensor.matmul(out=blk[0:cp_agg, 3 * P:4 * P],
                                         lhsT=ones_row_f[:1, 0:cp_agg],
                                         rhs=rv[:], start=True, stop=True)
                        dnb = pool.tile([cp_agg, P], FP32, tag="dnb",
                                        name=f"dn{layer}_{b}")
                        nc.scalar.copy(out=dnb[:], in_=blk[0:cp_agg, 3 * P:4 * P])
                        xnorm = pool.tile([cp_agg, P], BF16, tag="xnorm",
                                          name=f"xn{layer}_{b}")
                        nc.vector.tensor_tensor(
                            out=xnorm[:], in0=blk[0:cp_agg, 0:P],
                            in1=dnb[:], op=OP.mult)
                        # skip + value projection accumulate back-to-back
                        skip_for_block(b, blk)
                        vproj = v1ext_t if layer == 1 else v2w_t
                        nc.tensor.matmul(out=blk[:, 2 * P:3 * P],
                                         lhsT=vproj[:], rhs=xnorm[:],
                                         start=False, stop=True)
                        h_sb = spool.tile([P, P], BF16, tag="h_sb")
                        if layer == 1:
                            nc.scalar.activation(out=h_sb[:],
                                                 in_=blk[:, 2 * P:3 * P],
                                                 func=AF.Relu)
                        else:
                            nc.scalar.activation(out=h_sb[:],
                                                 in_=blk[:, 2 * P:3 * P],
                                                 func=AF.Relu,
                                                 bias=b2col_t[:, 0:1])
                        on_block_done(b, h_sb, blk)

            for k in range(nsg + 1):
                if k < nsg:
                    front(k)
                if k >= 1:
                    back(k - 1)

        # ---------------- layer 1 plumbing
        def rhs1(b):
            return (B1_all[:, b * P:(b + 1) * P], FE)

        def skip1(b, blk):
            nc.tensor.matmul(out=blk[:, 2 * P:3 * P], lhsT=ws1ext_t[:],
                             rhs=xall[:, b * P:(b + 1) * P],
                             start=True, stop=False)

        def l1_done(b, h_sb, blk):
            nc.vector.tensor_copy(out=h1T_own[:, b * P:(b + 1) * P], in_=h_sb[:])
            # node-major h1 rows for the allgather + layer-2 gathers
            tp = psT.tile([P, P], BF16, space="PSUM", tag="psT",
                          name=f"h1tp{b}")
            nc.tensor.transpose(out=tp[:], in_=h_sb[:], identity=ident[:])
            tsb = spool.tile([P, H], BF16, tag="t2row")
            nc.scalar.copy(out=tsb[:], in_=tp[:])
            nc.sync.dma_start(out=h1_sh[b * P:(b + 1) * P, :], in_=tsb[:])

        edge_phase(1, xrows_lo, xrows_hi, rhs1, skip1, l1_done)

        if dbg == 'h1':
            d = pool.tile([P, NB * P], FP32, tag="dbg")
            nc.vector.tensor_copy(out=d[:], in_=h1T_own[:])
            nc.sync.dma_start(out=dbg_o, in_=d[:])
            do = pool.tile([GPC, 1], FP32, tag="dbgo")
            nc.vector.memset(do[:], 0.5)
            nc.sync.dma_start(out=out_g, in_=do[:])

        # ---------------- collective: allgather raw h1
        nc.gpsimd.collective_compute(
            kind="AllGather", op=OP.bypass,
            replica_groups=[list(range(NCORES))],
            ins=[h1_sh], outs=[h1_all])

        if dbg is None or dbg == 'full':
            # ---------------- layer 2 plumbing
            pool_sb = persist.tile([GPC, H + 1], FP32)
            nc.vector.memset(pool_sb[:], 0.0)

            # score rhs blocks during the collective: R_b = A2 h1_d^T + u
            R_all = persist.tile([H, NB * P], BF16)
            for b in range(NB):
                rp = psS.tile([H, P], FP32, space="PSUM", tag="psS",
                              name=f"r2_{b}")
                nc.tensor.matmul(out=rp[:], lhsT=a2t_t[:],
                                 rhs=h1T_own[:, b * P:(b + 1) * P],
                                 start=True, stop=False)
                nc.tensor.matmul(out=rp[:], lhsT=urow_t[:1, :],
                                 rhs=ones_row[:1, :], start=False, stop=True)
                if b % 2 == 0:
                    nc.vector.tensor_copy(out=R_all[:, b * P:(b + 1) * P],
                                          in_=rp[:])
                else:
                    nc.scalar.copy(out=R_all[:, b * P:(b + 1) * P], in_=rp[:])

            def rhs2(b):
                return (R_all[:, b * P:(b + 1) * P], H)

            def skip2(b, blk):
                nc.tensor.matmul(out=blk[:, 2 * P:3 * P], lhsT=ws2_t[:],
                                 rhs=h1T_own[:, b * P:(b + 1) * P],
                                 start=True, stop=False)

            def l2_done(b, h_sb, blk):
                # transpose h2^T -> h2 [d, h], then pool matmul
                tp = psT.tile([P, P], BF16, space="PSUM", tag="psT",
                              name=f"h2tp{b}")
                nc.tensor.transpose(out=tp[:], in_=h_sb[:], identity=ident[:])
                h2x = pool.tile([P, H + 1], BF16, tag="h2x")
                nc.scalar.copy(out=h2x[:, 0:H], in_=tp[:])
                nc.vector.memset(h2x[:, H:H + 1], 1.0)
                gh = pool.tile([P, GPC], BF16, tag="gh")
                nc.vector.tensor_scalar(
                    out=gh[:], in0=iota[:, 0:GPC], scalar1=glocal_t[:, b:b + 1],
                    scalar2=None, op0=OP.is_equal)
                nc.tensor.matmul(out=blk[0:GPC, 2 * P:2 * P + H + 1],
                                 lhsT=gh[:], rhs=h2x[:],
                                 start=True, stop=True)
                nc.vector.tensor_tensor(out=pool_sb[:], in0=pool_sb[:],
                                        in1=blk[0:GPC, 2 * P:2 * P + H + 1],
                                        op=OP.add)

            # seq branch computed during the collective window
            seqw_t0 = const_tile(seqw[0:P, :], [P, H])
            seqw_t1 = const_tile(seqw[P:SEQ, :], [P, H])
            fc1w_t0 = const_tile(fc1w[0:P, :], [P, H])
            fc1w_t1 = const_tile(fc1w[P:2 * H, :], [P, H])
            seqT0 = const_tile(seqT[0:P, :], [P, GPC])
            seqT1 = const_tile(seqT[P:SEQ, :], [P, GPC])
            z = pool.tile([GPC, 2 * H], BF16, tag="z")
            pseq = psS.tile([GPC, H], FP32, space="PSUM", tag="psS",
                            name="pseq")
            nc.tensor.matmul(out=pseq[:], lhsT=seqT0[:], rhs=seqw_t0[:],
                             start=True, stop=False)
            nc.tensor.matmul(out=pseq[:], lhsT=seqT1[:], rhs=seqw_t1[:],
                             start=False, stop=False)
            nc.tensor.matmul(out=pseq[:], lhsT=ones_row[:1, 0:GPC],
                             rhs=seqb_t[:1, :], start=False, stop=True)
            nc.scalar.activation(out=z[:, H:2 * H], in_=pseq[:], func=AF.Relu)

            edge_phase(2, h1_all[0:split, :], h1_all[split:NCORES * ncap, :],
                       rhs2, skip2, l2_done)

            # ---------------- tail: pooled -> MLP -> sigmoid
            cnt = pool.tile([GPC, 1], FP32, tag="cnt")
            nc.vector.tensor_scalar_add(out=cnt[:], in0=pool_sb[:, H:H + 1],
                                        scalar1=EPS)
            rc = pool.tile([GPC, 1], FP32, tag="rc")
            nc.vector.reciprocal(out=rc[:], in_=cnt[:])
            nc.vector.tensor_scalar(out=z[:, 0:H], in0=pool_sb[:, 0:H],
                                    scalar1=rc[:, 0:1], scalar2=None,
                                    op0=OP.mult)

            zT = []
            for i in range(2):
                tzp = psT.tile([P, GPC], BF16, space="PSUM", tag="psT",
                               name=f"tzp{i}")
                nc.tensor.transpose(out=tzp[:], in_=z[:, i * H:(i + 1) * H],
                                    identity=ident[0:GPC, 0:GPC])
                zt = pool.tile([P, GPC], BF16, tag=f"zT{i}")
                nc.vector.tensor_copy(out=zt[:], in_=tzp[:])
                zT.append(zt)
            pfc1 = psS.tile([GPC, H], FP32, space="PSUM", tag="psS",
                            name="pfc1")
            nc.tensor.matmul(out=pfc1[:], lhsT=zT[0][:], rhs=fc1w_t0[:],
                             start=True, stop=False)
            nc.tensor.matmul(out=pfc1[:], lhsT=zT[1][:], rhs=fc1w_t1[:],
                             start=False, stop=False)
            nc.tensor.matmul(out=pfc1[:], lhsT=ones_row[:1, 0:GPC],
                             rhs=fc1b_t[:1, :], start=False, stop=True)
            z1 = pool.tile([GPC, H], BF16, tag="z1")
            nc.scalar.activation(out=z1[:], in_=pfc1[:], func=AF.Relu)
            tz1 = psT.tile([P, GPC], BF16, space="PSUM", tag="psT",
                           name="tz1")
            nc.tensor.transpose(out=tz1[:], in_=z1[:],
                                identity=ident[0:GPC, 0:GPC])
            z1T = pool.tile([P, GPC], BF16, tag="z1T")
            nc.vector.tensor_copy(out=z1T[:], in_=tz1[:])
            pfc2 = psS.tile([GPC, 1], FP32, space="PSUM", tag="psS",
                            name="pfc2")
            nc.tensor.matmul(out=pfc2[:], lhsT=z1T[:], rhs=fc2w_t[:],
                             start=True, stop=False)
            nc.tensor.matmul(out=pfc2[:], lhsT=ones_row[:1, 0:GPC],
                             rhs=fc2b_t[:1, :], start=False, stop=True)
            outs = pool.tile([GPC, 1], FP32, tag="outs")
            nc.scalar.activation(out=outs[:], in_=pfc2[:], func=AF.Sigmoid)
            nc.sync.dma_start(out=out_g, in_=outs[:])

    nc.compile()
    return nc


# ---------------------------------------------------------------- entry

_CACHE = {}


def kernel(**inputs):
    meta = preprocess(inputs)
    key = (meta['ncap'], meta['ntot'], tuple(meta['tile_block'].tolist()))
    if key not in _CACHE:
        _CACHE[key] = build_program(meta)
    nc = _CACHE[key]
    in_maps = make_inputs(inputs, meta)
    res = run_bass_kernel_spmd(nc, in_maps, core_ids=list(range(NCORES)))
    out = np.concatenate([res.results[c]['out_g'] for c in range(NCORES)], 0)
    return out.astype(np.float32)


# revision 11
# speedup vs baseline: 1.3438x; 1.0903x over previous
"""Trainium2 Bass kernel for nn_Discriminator (2x TransformerConv GNN + pool + MLP).

v3 design (raw-feature gathers, no node-table prologue):
- Graphs split 64-per-core (batch sorted => contiguous node ranges per core).
- L1 gathers RAW x_ext rows ([x|1|0pad] -> 128 bf16 cols, 256B) straight from
  a host-built row table; the score projection A1 is applied per dst-block
  (B1_b = A1 @ x_d^T), and the value projection v1 is applied per block AFTER
  aggregation (linearity). The ones column of x_ext yields the softmax
  denominator for free in the aggregation matmul.
- L2 allgathers RAW h1 ([N,128] bf16, half the bytes of a v|p table) and
  applies A2/u per block (R_b = A2 h1_d + u) and v2 after aggregation.
- Per tile: PE transpose of the gathered rows, S = X_sT^T @ B_b (PE), batched
  exp (scalar engine), W = onehot(dst) * exp(S) in one DVE op, aggregation
  via one PE matmul (plus a denominator matmul for L2 only).

Self-contained: hardcodes problem shapes; layout computed from runtime inputs.
"""
import numpy as np
import ml_dtypes

import concourse.bass as bass
import concourse.bacc as bacc
import concourse.mybir as mybir
from concourse.tile import TileContext
from concourse.masks import make_identity
from concourse.bass_utils import run_bass_kernel_spmd

BF = np.float16
N, E, G = 50000, 800000, 512
F_IN, H, SEQ = 64, 128, 256
FE = F_IN + 1                 # x extended with ones column
NCORES = 8
GPC = G // NCORES             # graphs per core
P = 128
SGB = 4                       # blocks per supergroup
GMAX = 8                      # tiles per gather call (1024 idxs; runtime ucode cap)
TTB = 8                       # tiles per transpose/copy batch (one bf16 bank)
SCALE = 1.0 / np.sqrt(np.float32(H))
EPS = 1e-30

FP32 = mybir.dt.float32
BF16 = mybir.dt.float16
I16 = mybir.dt.int16
AF = mybir.ActivationFunctionType
OP = mybir.AluOpType


# ---------------------------------------------------------------- host prep

def _pack_idx(idx_stream):
    """idx_stream [ntot*128] -> [128, ntot*8] int16 (16-partition wrap, x8)."""
    n = idx_stream.shape[0]
    s = n // 16
    out = np.zeros((128, s), dtype=np.int16)
    arr = idx_stream.reshape(s, 16).T.astype(np.int16)
    for g in range(8):
        out[g * 16:(g + 1) * 16, :] = arr
    return out


def preprocess(inputs):
    batch = np.asarray(inputs['batch']).astype(np.int64)
    ei = np.asarray(inputs['edge_index']).astype(np.int64)
    src_g, dst_g = ei[0], ei[1]

    gstart = np.searchsorted(batch, np.arange(NCORES) * GPC)
    gend = np.searchsorted(batch, np.arange(NCORES) * GPC + GPC)
    nloc = gend - gstart
    ncap = int(np.ceil(nloc.max() / (2 * P)) * (2 * P))   # even block count
    NB = ncap // P
    split = (NCORES // 2) * ncap          # lo/hi table split row

    node_core = batch // GPC
    node_local = np.arange(N) - gstart[node_core]
    table_idx = node_core * ncap + node_local            # row in [8*ncap] table
    node_half = (table_idx >= split).astype(np.int64)
    half_idx = table_idx - node_half * split             # row within half

    edge_core = node_core[dst_g]
    per_core = []
    for c in range(NCORES):
        em = np.where(edge_core == c)[0]
        e_src, e_dst = src_g[em], dst_g[em]
        order = np.argsort(e_dst, kind='stable')
        e_src, e_dst = e_src[order], e_dst[order]
        dst_loc = e_dst - gstart[c]
        tsrc = half_idx[e_src]
        is_hi = node_half[e_src]
        blk = dst_loc // P
        buckets = {}
        for b in range(NB):
            bm = np.where(blk == b)[0]
            bh = is_hi[bm]
            for half in (0, 1):
                hm = bm[bh == half]
                buckets[(b, half)] = (tsrc[hm], dst_loc[hm] - b * P)
        per_core.append(buckets)

    # uniform tile counts per (b, half)
    tcount = {}
    for b in range(NB):
        for half in (0, 1):
            mx = max(len(per_core[c][(b, half)][0]) for c in range(NCORES))
            tcount[(b, half)] = (mx + P - 1) // P

    # supergroups and stream layout (shared across cores)
    sgs = []           # (t0, Tlo, Tsg, blocks)
    tile_block = []    # per tile: block id
    t0 = 0
    for s0 in range(0, NB, SGB):
        blocks = list(range(s0, min(s0 + SGB, NB)))
        lo = sum(tcount[(b, 0)] for b in blocks)
        hi = sum(tcount[(b, 1)] for b in blocks)
        for half in (0, 1):
            for b in blocks:
                tile_block += [b] * tcount[(b, half)]
        sgs.append((t0, lo, lo + hi, blocks))
        t0 += lo + hi
    ntot = t0
    tile_block = np.array(tile_block)
    blk_first = {b: int(np.where(tile_block == b)[0][0]) for b in range(NB)}
    blk_last = {b: int(np.where(tile_block == b)[0][-1]) for b in range(NB)}

    # per-core streams
    dls = []
    kvis = []
    for c in range(NCORES):
        kvi = np.zeros(ntot * P, np.int64)
        dl = np.full(ntot * P, -1.0, np.float32)
        pos = 0
        for (t0_, lo, tsg, blocks) in sgs:
            for half in (0, 1):
                for b in blocks:
                    k, d = per_core[c][(b, half)]
                    ntile = tcount[(b, half)]
                    cnt = len(k)
                    kvi[pos:pos + cnt] = k
                    dl[pos:pos + cnt] = d
                    pos += ntile * P
        assert pos == ntot * P
        dls.append(dl.reshape(ntot, P))
        kvis.append(kvi)

    # per-tile dst band: union across cores (program structure is shared)
    bands = []
    for tt in range(ntot):
        lo128, hi = P, -1
        for c in range(NCORES):
            v = dls[c][tt]
            v = v[v >= 0]
            if len(v):
                lo128 = min(lo128, int(v.min()))
                hi = max(hi, int(v.max()))
        if hi < 0:
            lo128, hi = 0, 0
        bands.append((lo128, hi - lo128 + 1))

    cores = []
    for c in range(NCORES):
        dl = dls[c]
        dlb = dl.copy()
        for tt in range(ntot):
            m = dlb[tt] >= 0
            dlb[tt][m] -= bands[tt][0]
        gl = np.full(ncap, -1.0, np.float32)
        gl[:nloc[c]] = (batch[gstart[c]:gend[c]] - c * GPC).astype(np.float32)
        cores.append({
            'kvidx': _pack_idx(kvis[c]),
            'dstl': dlb.T.astype(np.float32),                  # [128, ntot]
            'glocal': gl.reshape(NB, P).T.astype(np.float32),  # [128, NB]
        })

    return {
        'ncap': ncap, 'NB': NB, 'split': split, 'ntot': ntot,
        'gstart': gstart, 'gend': gend, 'nloc': nloc,
        'table_idx': table_idx, 'sgs': sgs,
        'tile_block': tile_block, 'blk_first': blk_first, 'blk_last': blk_last,
        'bands': bands, 'cores': cores,
    }


def make_inputs(inputs, meta):
    ncap = meta['ncap']
    x = np.asarray(inputs['x'], np.float32)
    f32 = lambda v: np.asarray(v, np.float32)

    # raw x_ext row table [8*ncap, 128]: [x | 1 | 0pad], gathered by L1
    xrows = np.zeros((NCORES * ncap, P), np.float32)
    xrows[meta['table_idx'], 0:F_IN] = x
    xrows[:, F_IN] = 1.0
    xrows_bf = xrows.astype(BF)

    # column-major x_ext per core (dst side): [FE, ncap]
    xte = np.zeros((FE, NCORES * ncap), np.float32)
    xte[F_IN, :] = 1.0
    xte[:F_IN, meta['table_idx']] = x.T
    xte_bf = xte.astype(BF)

    # layer-1: score = x_ext[s] A1 x_ext[d]^T; A1 = wk1 wq1^T
    wk1 = np.concatenate([f32(inputs['k1_w']), f32(inputs['k1_b'])[None, :]], 0)
    wq1 = np.concatenate([f32(inputs['q1_w']), f32(inputs['q1_b'])[None, :]], 0)
    A1 = wk1 @ wq1.T                                     # [FE, FE]
    v1ext = np.zeros((FE, H), np.float32)
    v1ext[:F_IN, :] = f32(inputs['v1_w'])
    v1ext[F_IN, :] = f32(inputs['v1_b'])
    ws1ext = np.zeros((FE, H), np.float32)
    ws1ext[:F_IN, :] = f32(inputs['s1_w'])
    ws1ext[F_IN, :] = f32(inputs['s1_b'])

    # layer-2: score = h1[s] A2 h1[d] + u.h1[s] (+ dst-only terms cancel)
    A2 = f32(inputs['k2_w']) @ f32(inputs['q2_w']).T     # [H, H]
    u = f32(inputs['k2_w']) @ f32(inputs['q2_b'])        # [H]
    b2col = (f32(inputs['s2_b']) + f32(inputs['v2_b']))[:, None]  # [H,1]

    seqc = np.asarray(inputs['sequence_character'], np.float32)
    split = meta['split']

    shared = {
        'xrows_lo': np.ascontiguousarray(xrows_bf[:split]),
        'xrows_hi': np.ascontiguousarray(xrows_bf[split:]),
        'a1t': np.ascontiguousarray(A1.T.astype(BF)),
        'v1ext': v1ext.astype(BF),
        'ws1ext': ws1ext.astype(BF),
        'a2t': np.ascontiguousarray(A2.T.astype(BF)),
        'urow': u[None, :].astype(BF),
        'v2w': f32(inputs['v2_w']).astype(BF),
        'ws2': f32(inputs['s2_w']).astype(BF),
        'b2col': b2col.astype(np.float32),
        'seqw': f32(inputs['seq_w']).astype(BF),
        'seqb': f32(inputs['seq_b'])[None, :].astype(BF),
        'fc1w': f32(inputs['fc1_w']).astype(BF),
        'fc1b': f32(inputs['fc1_b'])[None, :].astype(BF),
        'fc2w': f32(inputs['fc2_w']).astype(BF),
        'fc2b': f32(inputs['fc2_b'])[None, :].astype(BF),
        'iota': np.tile(np.arange(P, dtype=np.float32)[None, :], (P, 1)).astype(BF),
    }

    in_maps = []
    for c in range(NCORES):
        m = dict(shared)
        m['xte_own'] = np.ascontiguousarray(xte_bf[:, c * ncap:(c + 1) * ncap])
        m['seqT'] = np.ascontiguousarray(
            seqc[c * GPC:(c + 1) * GPC].T.astype(BF))            # [256, 64]
        mc = meta['cores'][c]
        m['kvidx'] = mc['kvidx']
        m['dstl'] = mc['dstl']
        m['glocal'] = mc['glocal']
        in_maps.append(m)
    return in_maps


# ---------------------------------------------------------------- program

def build_program(meta, dbg=None):
    ncap, NB, ntot = meta['ncap'], meta['NB'], meta['ntot']
    split = meta['split']
    sgs = meta['sgs']
    bands = meta['bands']
    tile_block = meta['tile_block']
    blk_first, blk_last = meta['blk_first'], meta['blk_last']

    nc = bacc.Bacc("TRN2", target_bir_lowering=False, debug=False,
                   enable_asserts=False, num_devices=NCORES,
                   num_swdge_queues=4)

    def din(name, shape, dt):
        return nc.dram_tensor(name, shape, dt, kind="ExternalInput").ap()

    xrows_lo = din('xrows_lo', [split, P], BF16)
    xrows_hi = din('xrows_hi', [NCORES * ncap - split, P], BF16)
    xte_own = din('xte_own', [FE, ncap], BF16)
    a1t = din('a1t', [FE, FE], BF16)
    v1ext = din('v1ext', [FE, H], BF16)
    ws1ext = din('ws1ext', [FE, H], BF16)
    a2t = din('a2t', [H, H], BF16)
    urow = din('urow', [1, H], BF16)
    v2w = din('v2w', [H, H], BF16)
    ws2 = din('ws2', [H, H], BF16)
    b2col = din('b2col', [H, 1], FP32)
    seqw = din('seqw', [SEQ, H], BF16)
    seqb = din('seqb', [1, H], BF16)
    fc1w = din('fc1w', [2 * H, H], BF16)
    fc1b = din('fc1b', [1, H], BF16)
    fc2w = din('fc2w', [H, 1], BF16)
    fc2b = din('fc2b', [1, 1], BF16)
    iota_in = din('iota', [P, P], BF16)
    seqT = din('seqT', [SEQ, GPC], BF16)
    kvidx = din('kvidx', [P, ntot * 8], I16)
    dstl = din('dstl', [P, ntot], FP32)
    glocal = din('glocal', [P, NB], FP32)

    out_g = nc.dram_tensor('out_g', [GPC, 1], FP32, kind="ExternalOutput").ap()
    if dbg:
        dbg_o = nc.dram_tensor('dbg_o', [P, NB * P], FP32,
                               kind="ExternalOutput").ap()

    h1_sh = nc.dram_tensor('h1_sh', [ncap, H], BF16, kind="Internal").ap()
    h1_all = nc.dram_tensor('h1_all', [NCORES * ncap, H], BF16,
                            kind="Internal", addr_space="Shared").ap()

    from contextlib import ExitStack
    with TileContext(nc, num_cores=NCORES) as tc, ExitStack() as _st:
        cpool = _st.enter_context(tc.tile_pool(name="consts", bufs=1))
        pool = _st.enter_context(tc.tile_pool(name="work", bufs=3))
        spool = _st.enter_context(tc.tile_pool(name="stage", bufs=5))
        epool = _st.enter_context(tc.tile_pool(name="exps", bufs=4))
        wpool = _st.enter_context(tc.tile_pool(name="wts", bufs=16))
        persist = _st.enter_context(tc.tile_pool(name="persist", bufs=1))
        psS = _st.enter_context(tc.tile_pool(name="psS", bufs=3, space="PSUM"))
        psT = _st.enter_context(tc.tile_pool(name="psT", bufs=2, space="PSUM"))
        psB = _st.enter_context(tc.tile_pool(name="psB", bufs=3, space="PSUM"))

        # ---------------- constants
        iota = cpool.tile([P, P], BF16)
        nc.sync.dma_start(out=iota[:], in_=iota_in)
        ident = cpool.tile([P, P], BF16)
        make_identity(nc, ident[:])
        ones_col = cpool.tile([P, 1], BF16)
        nc.vector.memset(ones_col[:], 1.0)
        ones_row = cpool.tile([1, P], BF16)
        nc.vector.memset(ones_row[:], 1.0)
        ones_row_f = cpool.tile([1, P], FP32)
        nc.vector.memset(ones_row_f[:], 1.0)
        zero_row = cpool.tile([1, 2 * P], BF16)
        nc.vector.memset(zero_row[:], 0.0)

        _cn = [0]

        def const_tile(ap_, shape, dt=BF16):
            _cn[0] += 1
            t = cpool.tile(shape, dt, tag=f"c{_cn[0]}", name=f"c{_cn[0]}")
            nc.sync.dma_start(out=t[:], in_=ap_)
            return t

        kvidx_t = const_tile(kvidx, [P, ntot * 8], I16)
        a1t_t = const_tile(a1t, [FE, FE])
        v1ext_t = const_tile(v1ext, [FE, H])
        ws1ext_t = const_tile(ws1ext, [FE, H])
        a2t_t = const_tile(a2t, [H, H])
        urow_t = const_tile(urow, [1, H])
        v2w_t = const_tile(v2w, [H, H])
        ws2_t = const_tile(ws2, [H, H])
        b2col_t = const_tile(b2col, [H, 1], FP32)
        seqb_t = const_tile(seqb, [1, H])
        fc1b_t = const_tile(fc1b, [1, H])
        fc2w_t = const_tile(fc2w, [H, 1])
        fc2b_t = const_tile(fc2b, [1, 1])
        dstl_t = const_tile(dstl, [P, ntot], FP32)
        glocal_t = const_tile(glocal, [P, NB], FP32)
        xall = const_tile(xte_own, [FE, ncap])      # full own-x, column-major

        h1T_own = persist.tile([P, NB * P], BF16)   # h1 transposed, own shard

        # ---------------- B1 blocks: B1_b = A1 @ x_d^T  (score rhs, layer 1)
        B1_all = persist.tile([FE, NB * P], BF16)
        for b in range(NB):
            bp = psS.tile([FE, P], FP32, space="PSUM", tag="psS",
                          name=f"b1_{b}")
            nc.tensor.matmul(out=bp[:], lhsT=a1t_t[:],
                             rhs=xall[:, b * P:(b + 1) * P],
                             start=True, stop=True)
            if b % 2 == 0:
                nc.vector.tensor_copy(out=B1_all[:, b * P:(b + 1) * P],
                                      in_=bp[:])
            else:
                nc.scalar.copy(out=B1_all[:, b * P:(b + 1) * P], in_=bp[:])

        # ---------------- edge phase (shared for both layers)
        # Per-block PSUM bank "blk" [P, 4*P] f32:
        #   [:, 0:P]      attention aggregate (raw-feature space)
        #   [0:1, P:2P]   denominator row (layer 2 only; layer 1 uses agg row 64)
        #   [:, 2P:3P]    skip + value-projection accumulator
        #   [:, 3P:4P]    scratch (reciprocal broadcast)
        def edge_phase(layer, tab_lo_ap, tab_hi_ap, rhs_for_block,
                       skip_for_block, on_block_done):
            """rhs_for_block(b) -> (rhs_tile_ap, cp) SBUF [cp,128] dst feats.
            skip_for_block(b, blk) opens blk[:, 2P:3P] accumulation with skip.
            on_block_done(b, h_sb, blk) with h_sb [128,128] = relu'd out^T.

            Software-pipelined with a one-supergroup skew: iteration k emits
            supergroup k's gather/transpose/S/exp (front) and supergroup k-1's
            W/aggregate/epilogue (back), so the in-order engines stream without
            cross-stage stalls."""
            _q = [0]
            _blks = {}
            cp_agg = FE if layer == 1 else H
            SW = 4 * P                     # strip width (full PSUM bank)
            nsg = len(sgs)
            stash = [None] * nsg

            def front(k):
                (t0, lo, tsg, blocks) = sgs[k]
                kv_t = pool.tile([P, tsg, H], BF16, tag="kv_g", bufs=5)
                for (h0, h1, hh) in ((0, lo, 0), (lo, tsg, 1)):
                    for ps0 in range(h0, h1, GMAX):
                        pe0 = min(ps0 + GMAX, h1)
                        tab_h = tab_lo_ap if hh == 0 else tab_hi_ap
                        nc.gpsimd.dma_gather(
                            out_ap=kv_t[:, ps0:pe0, :], in_ap=tab_h,
                            idxs_ap=kvidx_t[:, (t0 + ps0) * 8:(t0 + pe0) * 8],
                            num_idxs=(pe0 - ps0) * P, num_idxs_reg=(pe0 - ps0) * P,
                            elem_size=H, queue_num=_q[0] % 4)
                        _q[0] += 1

                # transposes (PE) + batch copies (scalar engine)
                pT_sbs = {}
                for j0 in range(0, tsg, TTB):
                    j1 = min(j0 + TTB, tsg)
                    psT_t = psT.tile([P, TTB, P], BF16, space="PSUM",
                                     tag="psT")
                    for tl in range(j0, j1):
                        nc.tensor.transpose(
                            out=psT_t[:, tl - j0, :], in_=kv_t[:, tl, :],
                            identity=ident[:])
                    sb = spool.tile([P, TTB, P], BF16, tag="pT_sb")
                    if (j0 // TTB) % 2 == 0:
                        nc.scalar.copy(out=sb[:, 0:j1 - j0, :],
                                       in_=psT_t[:, 0:j1 - j0, :])
                    else:
                        nc.vector.tensor_copy(out=sb[:, 0:j1 - j0, :],
                                              in_=psT_t[:, 0:j1 - j0, :])
                    pT_sbs[j0 // TTB] = sb

                # banded S matmuls packed into full-bank strips + exp (Act)
                tile_se = {}
                strips, strip_cols = [], []
                for tl in range(tsg):
                    tt = t0 + tl
                    b = int(tile_block[tt])
                    rhs_sb, cp = rhs_for_block(b)
                    dlo, w = bands[tt]
                    if not strips or strip_cols[-1] + w > SW:
                        st = psS.tile([P, SW], FP32, space="PSUM", tag="psS",
                                      name=f"st{layer}_{tt}")
                        strips.append(st)
                        strip_cols.append(0)
                    off = strip_cols[-1]
                    strip_cols[-1] += w
                    tile_se[tl] = (len(strips) - 1, off, w)
                    pT_sb = pT_sbs[tl // TTB]
                    nc.tensor.matmul(
                        out=strips[-1][:, off:off + w],
                        lhsT=pT_sb[0:cp, tl % TTB, :],
                        rhs=rhs_sb[:, dlo:dlo + w],
                        start=True, stop=True)

                E_sbs = []
                for si, st in enumerate(strips):
                    esb = epool.tile([P, SW], BF16, tag="E_sb",
                                     name=f"esb{layer}_{t0}_{si}")
                    used = strip_cols[si]
                    nc.scalar.activation(out=esb[:, 0:used], in_=st[:, 0:used],
                                         func=AF.Exp, scale=float(SCALE))
                    E_sbs.append(esb)
                stash[k] = (t0, tsg, kv_t, tile_se, E_sbs)

            def back(k):
                (t0, tsg, kv_t, tile_se, E_sbs) = stash[k]
                stash[k] = None
                for tl in range(tsg):
                    tt = t0 + tl
                    b = int(tile_block[tt])
                    si, off, w = tile_se[tl]
                    E_sb = E_sbs[si]
                    if b not in _blks:
                        blk = psB.tile([P, 4 * P], FP32, space="PSUM",
                                       tag="blk", name=f"blk{layer}_{b}")
                        _blks[b] = blk
                        # zero acc+den regions, open the accumulation group
                        nc.tensor.matmul(out=blk[:, 0:2 * P],
                                         lhsT=zero_row[:1, 0:P],
                                         rhs=zero_row[:1, :],
                                         start=True, stop=False)
                    W = wpool.tile([P, P], BF16, tag="W")
                    nc.vector.scalar_tensor_tensor(
                        out=W[:, 0:w], in0=iota[:, 0:w],
                        scalar=dstl_t[:, tt:tt + 1],
                        in1=E_sb[:, off:off + w],
                        op0=OP.is_equal, op1=OP.mult)
                    blk = _blks[b]
                    dlo = bands[tt][0]
                    nc.tensor.matmul(
                        out=blk[0:cp_agg, dlo:dlo + w],
                        lhsT=kv_t[:, tl, 0:cp_agg],
                        rhs=W[:, 0:w], start=False, stop=False)
                    if layer == 2:
                        nc.tensor.matmul(
                            out=blk[0:1, P + dlo:P + dlo + w], lhsT=ones_col[:],
                            rhs=W[:, 0:w], start=False, stop=False)
                    if tt == blk_last[b]:
                        # close the accumulation group (flush)
                        nc.tensor.matmul(out=blk[:, 0:2 * P],
                                         lhsT=zero_row[:1, 0:P],
                                         rhs=zero_row[:1, :],
                                         start=False, stop=True)
                        # epilogue: normalize + project + skip + relu
                        _blks.pop(b)
                        den_ap = (blk[F_IN:FE, 0:P] if layer == 1
                                  else blk[0:1, P:2 * P])
                        dv = pool.tile([1, P], FP32, tag="dv")
                        nc.vector.tensor_scalar_add(out=dv[:], in0=den_ap,
                                                    scalar1=EPS)
                        rv = pool.tile([1, P], FP32, tag="rv")
                        nc.vector.reciprocal(out=rv[:], in_=dv[:])
                        nc.tensor.matmul(out=blk[0:cp_agg, 3 * P:4 * P],
                                         lhsT=ones_row_f[:1, 0:cp_agg],
                                         rhs=rv[:], start=True, stop=True)
                        dnb = pool.tile([cp_agg, P], FP32, tag="dnb",
                                        name=f"dn{layer}_{b}")
                        nc.scalar.copy(out=dnb[:], in_=blk[0:cp_agg, 3 * P:4 * P])
                        xnorm = pool.tile([cp_agg, P], BF16, tag="xnorm",
                                          name=f"xn{layer}_{b}")
                        nc.vector.tensor_tensor(
                            out=xnorm[:], in0=blk[0:cp_agg, 0:P],
                            in1=dnb[:], op=OP.mult)
                        # skip + value projection accumulate back-to-back
                        skip_for_block(b, blk)
                        vproj = v1ext_t if layer == 1 else v2w_t
                        nc.tensor.matmul(out=blk[:, 2 * P:3 * P],
                                         lhsT=vproj[:], rhs=xnorm[:],
                                         start=False, stop=True)
                        h_sb = spool.tile([P, P], BF16, tag="h_sb")
                        if layer == 1:
                            nc.scalar.activation(out=h_sb[:],
                                                 in_=blk[:, 2 * P:3 * P],
                                                 func=AF.Relu)
                        else:
                            nc.scalar.activation(out=h_sb[:],
                                                 in_=blk[:, 2 * P:3 * P],
                                                 func=AF.Relu,
                                                 bias=b2col_t[:, 0:1])
                        on_block_done(b, h_sb, blk)

            for k in range(nsg + 1):
                if k < nsg:
                    front(k)
                if k >= 1:
                    back(k - 1)

        # ---------------- layer 1 plumbing
        def rhs1(b):
            return (B1_all[:, b * P:(b + 1) * P], FE)

        def skip1(b, blk):
            nc.tensor.matmul(out=blk[:, 2 * P:3 * P], lhsT=ws1ext_t[:],
                             rhs=xall[:, b * P:(b + 1) * P],
                             start=True, stop=False)

        def l1_done(b, h_sb, blk):
            nc.vector.tensor_copy(out=h1T_own[:, b * P:(b + 1) * P], in_=h_sb[:])
            # node-major h1 rows for the allgather + layer-2 gathers
            tp = psT.tile([P, P], BF16, space="PSUM", tag="psT",
                          name=f"h1tp{b}")
            nc.tensor.transpose(out=tp[:], in_=h_sb[:], identity=ident[:])
            tsb = spool.tile([P, H], BF16, tag="t2row")
            nc.scalar.copy(out=tsb[:], in_=tp[:])
            nc.sync.dma_start(out=h1_sh[b * P:(b + 1) * P, :], in_=tsb[:])

        edge_phase(1, xrows_lo, xrows_hi, rhs1, skip1, l1_done)

        if dbg == 'h1':
            d = pool.tile([P, NB * P], FP32, tag="dbg")
            nc.vector.tensor_copy(out=d[:], in_=h1T_own[:])
            nc.sync.dma_start(out=dbg_o, in_=d[:])
            do = pool.tile([GPC, 1], FP32, tag="dbgo")
            nc.vector.memset(do[:], 0.5)
            nc.sync.dma_start(out=out_g, in_=do[:])

        # ---------------- collective: allgather raw h1
        nc.gpsimd.collective_compute(
            kind="AllGather", op=OP.bypass,
            replica_groups=[list(range(NCORES))],
            ins=[h1_sh], outs=[h1_all])

        if dbg is None or dbg == 'full':
            # ---------------- layer 2 plumbing
            pool_sb = persist.tile([GPC, H + 1], FP32)
            nc.vector.memset(pool_sb[:], 0.0)

            # score rhs blocks during the collective: R_b = A2 h1_d^T + u
            R_all = persist.tile([H, NB * P], BF16)
            for b in range(NB):
                rp = psS.tile([H, P], FP32, space="PSUM", tag="psS",
                              name=f"r2_{b}")
                nc.tensor.matmul(out=rp[:], lhsT=a2t_t[:],
                                 rhs=h1T_own[:, b * P:(b + 1) * P],
                                 start=True, stop=False)
                nc.tensor.matmul(out=rp[:], lhsT=urow_t[:1, :],
                                 rhs=ones_row[:1, :], start=False, stop=True)
                if b % 2 == 0:
                    nc.vector.tensor_copy(out=R_all[:, b * P:(b + 1) * P],
                                          in_=rp[:])
                else:
                    nc.scalar.copy(out=R_all[:, b * P:(b + 1) * P], in_=rp[:])

            def rhs2(b):
                return (R_all[:, b * P:(b + 1) * P], H)

            def skip2(b, blk):
                nc.tensor.matmul(out=blk[:, 2 * P:3 * P], lhsT=ws2_t[:],
                                 rhs=h1T_own[:, b * P:(b + 1) * P],
                                 start=True, stop=False)

            def l2_done(b, h_sb, blk):
                # transpose h2^T -> h2 [d, h], then pool matmul
                tp = psT.tile([P, P], BF16, space="PSUM", tag="psT",
                              name=f"h2tp{b}")
                nc.tensor.transpose(out=tp[:], in_=h_sb[:], identity=ident[:])
                h2x = pool.tile([P, H + 1], BF16, tag="h2x")
                nc.scalar.copy(out=h2x[:, 0:H], in_=tp[:])
                nc.vector.memset(h2x[:, H:H + 1], 1.0)
                gh = pool.tile([P, GPC], BF16, tag="gh")
                nc.vector.tensor_scalar(
                    out=gh[:], in0=iota[:, 0:GPC], scalar1=glocal_t[:, b:b + 1],
                    scalar2=None, op0=OP.is_equal)
                nc.tensor.matmul(out=blk[0:GPC, 2 * P:2 * P + H + 1],
                                 lhsT=gh[:], rhs=h2x[:],
                                 start=True, stop=True)
                nc.vector.tensor_tensor(out=pool_sb[:], in0=pool_sb[:],
                                        in1=blk[0:GPC, 2 * P:2 * P + H + 1],
                                        op=OP.add)

            # seq branch computed during the collective window
            seqw_t0 = const_tile(seqw[0:P, :], [P, H])
            seqw_t1 = const_tile(seqw[P:SEQ, :], [P, H])
            fc1w_t0 = const_tile(fc1w[0:P, :], [P, H])
            fc1w_t1 = const_tile(fc1w[P:2 * H, :], [P, H])
            seqT0 = const_tile(seqT[0:P, :], [P, GPC])
            seqT1 = const_tile(seqT[P:SEQ, :], [P, GPC])
            z = pool.tile([GPC, 2 * H], BF16, tag="z")
            pseq = psS.tile([GPC, H], FP32, space="PSUM", tag="psS",
                            name="pseq")
            nc.tensor.matmul(out=pseq[:], lhsT=seqT0[:], rhs=seqw_t0[:],
                             start=True, stop=False)
            nc.tensor.matmul(out=pseq[:], lhsT=seqT1[:], rhs=seqw_t1[:],
                             start=False, stop=False)
            nc.tensor.matmul(out=pseq[:], lhsT=ones_row[:1, 0:GPC],
                             rhs=seqb_t[:1, :], start=False, stop=True)
            nc.scalar.activation(out=z[:, H:2 * H], in_=pseq[:], func=AF.Relu)

            edge_phase(2, h1_all[0:split, :], h1_all[split:NCORES * ncap, :],
                       rhs2, skip2, l2_done)

            # ---------------- tail: pooled -> MLP -> sigmoid
            cnt = pool.tile([GPC, 1], FP32, tag="cnt")
            nc.vector.tensor_scalar_add(out=cnt[:], in0=pool_sb[:, H:H + 1],
                                        scalar1=EPS)
            rc = pool.tile([GPC, 1], FP32, tag="rc")
            nc.vector.reciprocal(out=rc[:], in_=cnt[:])
            nc.vector.tensor_scalar(out=z[:, 0:H], in0=pool_sb[:, 0:H],
                                    scalar1=rc[:, 0:1], scalar2=None,
                                    op0=OP.mult)

            zT = []
            for i in range(2):
                tzp = psT.tile([P, GPC], BF16, space="PSUM", tag="psT",
                               name=f"tzp{i}")
                nc.tensor.transpose(out=tzp[:], in_=z[:, i * H:(i + 1) * H],
                                    identity=ident[0:GPC, 0:GPC])
                zt = pool.tile([P, GPC], BF16, tag=f"zT{i}")
                nc.vector.tensor_copy(out=zt[:], in_=tzp[:])
                zT.append(zt)
            pfc1 = psS.tile([GPC, H], FP32, space="PSUM", tag="psS",
                            name="pfc1")
            nc.tensor.matmul(out=pfc1[:], lhsT=zT[0][:], rhs=fc1w_t0[:],
                             start=True, stop=False)
            nc.tensor.matmul(out=pfc1[:], lhsT=zT[1][:], rhs=fc1w_t1[:],
                             start=False, stop=False)
            nc.tensor.matmul(out=pfc1[:], lhsT=ones_row[:1, 0:GPC],
                             rhs=fc1b_t[:1, :], start=False, stop=True)
            z1 = pool.tile([GPC, H], BF16, tag="z1")
            nc.scalar.activation(out=z1[:], in_=pfc1[:], func=AF.Relu)
            tz1 = psT.tile([P, GPC], BF16, space="PSUM", tag="psT",
                           name="tz1")
            nc.tensor.transpose(out=tz1[:], in_=z1[:],
                                identity=ident[0:GPC, 0:GPC])
            z1T = pool.tile([P, GPC], BF16, tag="z1T")
            nc.vector.tensor_copy(out=z1T[:], in_=tz1[:])
            pfc2 = psS.tile([GPC, 1], FP32, space="PSUM", tag="psS",
                            name="pfc2")
            nc.tensor.matmul(out=pfc2[:], lhsT=z1T[:], rhs=fc2w_t[:],
                             start=True, stop=False)
            nc.tensor.matmul(out=pfc2[:], lhsT=ones_row[:1, 0:GPC],
                             rhs=fc2b_t[:1, :], start=False, stop=True)
            outs = pool.tile([GPC, 1], FP32, tag="outs")
            nc.scalar.activation(out=outs[:], in_=pfc2[:], func=AF.Sigmoid)
            nc.sync.dma_start(out=out_g, in_=outs[:])

    nc.compile()
    return nc


# ---------------------------------------------------------------- entry

_CACHE = {}


def kernel(**inputs):
    meta = preprocess(inputs)
    key = (meta['ncap'], meta['ntot'], tuple(meta['tile_block'].tolist()))
    if key not in _CACHE:
        _CACHE[key] = build_program(meta)
    nc = _CACHE[key]
    in_maps = make_inputs(inputs, meta)
    res = run_bass_kernel_spmd(nc, in_maps, core_ids=list(range(NCORES)))
    out = np.concatenate([res.results[c]['out_g'] for c in range(NCORES)], 0)
    return out.astype(np.float32)


# revision 14
# speedup vs baseline: 1.4073x; 1.0472x over previous
"""Trainium2 Bass kernel for nn_Discriminator (2x TransformerConv GNN + pool + MLP).

v3 design (raw-feature gathers, no node-table prologue):
- Graphs split 64-per-core (batch sorted => contiguous node ranges per core).
- L1 gathers RAW x_ext rows ([x|1|0pad] -> 128 bf16 cols, 256B) straight from
  a host-built row table; the score projection A1 is applied per dst-block
  (B1_b = A1 @ x_d^T), and the value projection v1 is applied per block AFTER
  aggregation (linearity). The ones column of x_ext yields the softmax
  denominator for free in the aggregation matmul.
- L2 allgathers RAW h1 ([N,128] bf16, half the bytes of a v|p table) and
  applies A2/u per block (R_b = A2 h1_d + u) and v2 after aggregation.
- Per tile: PE transpose of the gathered rows, S = X_sT^T @ B_b (PE), batched
  exp (scalar engine), W = onehot(dst) * exp(S) in one DVE op, aggregation
  via one PE matmul (plus a denominator matmul for L2 only).

Self-contained: hardcodes problem shapes; layout computed from runtime inputs.
"""
import numpy as np
import ml_dtypes

import concourse.bass as bass
import concourse.bacc as bacc
import concourse.mybir as mybir
from concourse.tile import TileContext
from concourse.masks import make_identity
from concourse.bass_utils import run_bass_kernel_spmd

BF = np.float16
N, E, G = 50000, 800000, 512
F_IN, H, SEQ = 64, 128, 256
FE = F_IN + 1                 # x extended with ones column
NCORES = 8
GPC = G // NCORES             # graphs per core
P = 128
SGB = 4                       # blocks per supergroup
GMAX = 8                      # tiles per gather call (1024 idxs; runtime ucode cap)
TTB = 8                       # tiles per transpose/copy batch (one bf16 bank)
SCALE = 1.0 / np.sqrt(np.float32(H))
EPS = 1e-30

FP32 = mybir.dt.float32
BF16 = mybir.dt.float16
I16 = mybir.dt.int16
AF = mybir.ActivationFunctionType
OP = mybir.AluOpType


# ---------------------------------------------------------------- host prep

def _pack_idx(idx_stream):
    """idx_stream [ntot*128] -> [128, ntot*8] int16 (16-partition wrap, x8)."""
    n = idx_stream.shape[0]
    s = n // 16
    out = np.zeros((128, s), dtype=np.int16)
    arr = idx_stream.reshape(s, 16).T.astype(np.int16)
    for g in range(8):
        out[g * 16:(g + 1) * 16, :] = arr
    return out


def preprocess(inputs):
    batch = np.asarray(inputs['batch']).astype(np.int64)
    ei = np.asarray(inputs['edge_index']).astype(np.int64)
    src_g, dst_g = ei[0], ei[1]

    gstart = np.searchsorted(batch, np.arange(NCORES) * GPC)
    gend = np.searchsorted(batch, np.arange(NCORES) * GPC + GPC)
    nloc = gend - gstart
    ncap = int(np.ceil(nloc.max() / (2 * P)) * (2 * P))   # even block count
    NB = ncap // P
    split = (NCORES // 2) * ncap          # lo/hi table split row

    node_core = batch // GPC
    node_local = np.arange(N) - gstart[node_core]
    table_idx = node_core * ncap + node_local            # row in [8*ncap] table
    node_half = (table_idx >= split).astype(np.int64)
    half_idx = table_idx - node_half * split             # row within half

    edge_core = node_core[dst_g]
    per_core = []
    for c in range(NCORES):
        em = np.where(edge_core == c)[0]
        e_src, e_dst = src_g[em], dst_g[em]
        order = np.argsort(e_dst, kind='stable')
        e_src, e_dst = e_src[order], e_dst[order]
        dst_loc = e_dst - gstart[c]
        tsrc = half_idx[e_src]
        is_hi = node_half[e_src]
        blk = dst_loc // P
        buckets = {}
        for b in range(NB):
            bm = np.where(blk == b)[0]
            bh = is_hi[bm]
            for half in (0, 1):
                hm = bm[bh == half]
                buckets[(b, half)] = (tsrc[hm], dst_loc[hm] - b * P)
        per_core.append(buckets)

    # uniform tile counts per (b, half)
    tcount = {}
    for b in range(NB):
        for half in (0, 1):
            mx = max(len(per_core[c][(b, half)][0]) for c in range(NCORES))
            tcount[(b, half)] = (mx + P - 1) // P

    # supergroups and stream layout (shared across cores)
    sgs = []           # (t0, Tlo, Tsg, blocks)
    tile_block = []    # per tile: block id
    t0 = 0
    for s0 in range(0, NB, SGB):
        blocks = list(range(s0, min(s0 + SGB, NB)))
        lo = sum(tcount[(b, 0)] for b in blocks)
        hi = sum(tcount[(b, 1)] for b in blocks)
        for half in (0, 1):
            for b in blocks:
                tile_block += [b] * tcount[(b, half)]
        sgs.append((t0, lo, lo + hi, blocks))
        t0 += lo + hi
    ntot = t0
    tile_block = np.array(tile_block)
    blk_first = {b: int(np.where(tile_block == b)[0][0]) for b in range(NB)}
    blk_last = {b: int(np.where(tile_block == b)[0][-1]) for b in range(NB)}

    # per-core streams
    dls = []
    kvis = []
    for c in range(NCORES):
        kvi = np.zeros(ntot * P, np.int64)
        dl = np.full(ntot * P, -1.0, np.float32)
        pos = 0
        for (t0_, lo, tsg, blocks) in sgs:
            for half in (0, 1):
                for b in blocks:
                    k, d = per_core[c][(b, half)]
                    ntile = tcount[(b, half)]
                    cnt = len(k)
                    kvi[pos:pos + cnt] = k
                    dl[pos:pos + cnt] = d
                    pos += ntile * P
        assert pos == ntot * P
        dls.append(dl.reshape(ntot, P))
        kvis.append(kvi)

    # per-tile dst band: union across cores (program structure is shared)
    bands = []
    for tt in range(ntot):
        lo128, hi = P, -1
        for c in range(NCORES):
            v = dls[c][tt]
            v = v[v >= 0]
            if len(v):
                lo128 = min(lo128, int(v.min()))
                hi = max(hi, int(v.max()))
        if hi < 0:
            lo128, hi = 0, 0
        bands.append((lo128, hi - lo128 + 1))

    # global strip packing (must match edge_phase): strips restart per sg
    SW = 4 * P
    tile_se = [None] * ntot       # tt -> (strip gid, off, w)
    strip_used = []               # gid -> used cols
    strip_sg = []                 # gid -> sg index
    for si_sg, (t0_, lo, tsg, blocks) in enumerate(sgs):
        cur = -1
        for tl in range(tsg):
            tt = t0_ + tl
            w = bands[tt][1]
            if cur < 0 or strip_used[cur] + w > SW:
                strip_used.append(0)
                strip_sg.append(si_sg)
                cur = len(strip_used) - 1
            tile_se[tt] = (cur, strip_used[cur], w)
            strip_used[cur] += w
    strip_base = np.concatenate([[0], np.cumsum(strip_used)]).astype(int)
    pm_cols = int(strip_base[-1])

    cores = []
    for c in range(NCORES):
        dl = dls[c]
        dlb = dl.copy()
        for tt in range(ntot):
            m = dlb[tt] >= 0
            dlb[tt][m] -= bands[tt][0]
        pm = np.zeros((P, pm_cols), np.float16)
        for tt in range(ntot):
            gid, off, w = tile_se[tt]
            col = strip_base[gid] + off
            d = dlb[tt]
            j = d.astype(np.int64)
            e = np.where(d >= 0)[0]
            pm[e, col + j[e]] = 1.0
        gl = np.full(ncap, -1.0, np.float32)
        gl[:nloc[c]] = (batch[gstart[c]:gend[c]] - c * GPC).astype(np.float32)
        cores.append({
            'kvidx': _pack_idx(kvis[c]),
            'premask': pm,                                     # [128, pm_cols]
            'glocal': gl.reshape(NB, P).T.astype(np.float32),  # [128, NB]
        })

    return {
        'ncap': ncap, 'NB': NB, 'split': split, 'ntot': ntot,
        'gstart': gstart, 'gend': gend, 'nloc': nloc,
        'table_idx': table_idx, 'sgs': sgs,
        'tile_block': tile_block, 'blk_first': blk_first, 'blk_last': blk_last,
        'bands': bands, 'cores': cores,
        'tile_se': tile_se, 'strip_used': strip_used, 'strip_sg': strip_sg,
        'strip_base': strip_base, 'pm_cols': pm_cols,
    }


def make_inputs(inputs, meta):
    ncap = meta['ncap']
    x = np.asarray(inputs['x'], np.float32)
    f32 = lambda v: np.asarray(v, np.float32)

    # raw x_ext row table [8*ncap, 128]: [x | 1 | 0pad], gathered by L1
    xrows = np.zeros((NCORES * ncap, P), np.float32)
    xrows[meta['table_idx'], 0:F_IN] = x
    xrows[:, F_IN] = 1.0
    xrows_bf = xrows.astype(BF)

    # column-major x_ext per core (dst side): [FE, ncap]
    xte = np.zeros((FE, NCORES * ncap), np.float32)
    xte[F_IN, :] = 1.0
    xte[:F_IN, meta['table_idx']] = x.T
    xte_bf = xte.astype(BF)

    # layer-1: score = x_ext[s] A1 x_ext[d]^T; A1 = wk1 wq1^T
    wk1 = np.concatenate([f32(inputs['k1_w']), f32(inputs['k1_b'])[None, :]], 0)
    wq1 = np.concatenate([f32(inputs['q1_w']), f32(inputs['q1_b'])[None, :]], 0)
    A1 = wk1 @ wq1.T                                     # [FE, FE]
    v1ext = np.zeros((FE, H), np.float32)
    v1ext[:F_IN, :] = f32(inputs['v1_w'])
    v1ext[F_IN, :] = f32(inputs['v1_b'])
    ws1ext = np.zeros((FE, H), np.float32)
    ws1ext[:F_IN, :] = f32(inputs['s1_w'])
    ws1ext[F_IN, :] = f32(inputs['s1_b'])

    # layer-2: score = h1[s] A2 h1[d] + u.h1[s] (+ dst-only terms cancel)
    A2 = f32(inputs['k2_w']) @ f32(inputs['q2_w']).T     # [H, H]
    u = f32(inputs['k2_w']) @ f32(inputs['q2_b'])        # [H]
    b2col = (f32(inputs['s2_b']) + f32(inputs['v2_b']))[:, None]  # [H,1]

    seqc = np.asarray(inputs['sequence_character'], np.float32)
    split = meta['split']

    shared = {
        'xrows_lo': np.ascontiguousarray(xrows_bf[:split]),
        'xrows_hi': np.ascontiguousarray(xrows_bf[split:]),
        'a1t': np.ascontiguousarray(A1.T.astype(BF)),
        'v1ext': v1ext.astype(BF),
        'ws1ext': ws1ext.astype(BF),
        'a2t': np.ascontiguousarray(A2.T.astype(BF)),
        'urow': u[None, :].astype(BF),
        'v2w': f32(inputs['v2_w']).astype(BF),
        'ws2': f32(inputs['s2_w']).astype(BF),
        'b2col': b2col.astype(np.float32),
        'seqw': f32(inputs['seq_w']).astype(BF),
        'seqb': f32(inputs['seq_b'])[None, :].astype(BF),
        'fc1w': f32(inputs['fc1_w']).astype(BF),
        'fc1b': f32(inputs['fc1_b'])[None, :].astype(BF),
        'fc2w': f32(inputs['fc2_w']).astype(BF),
        'fc2b': f32(inputs['fc2_b'])[None, :].astype(BF),
        'iota': np.tile(np.arange(P, dtype=np.float32)[None, :], (P, 1)).astype(BF),
    }

    in_maps = []
    for c in range(NCORES):
        m = dict(shared)
        m['xte_own'] = np.ascontiguousarray(xte_bf[:, c * ncap:(c + 1) * ncap])
        m['seqT'] = np.ascontiguousarray(
            seqc[c * GPC:(c + 1) * GPC].T.astype(BF))            # [256, 64]
        mc = meta['cores'][c]
        m['kvidx'] = mc['kvidx']
        m['premask'] = mc['premask']
        m['glocal'] = mc['glocal']
        in_maps.append(m)
    return in_maps


# ---------------------------------------------------------------- program

def build_program(meta, dbg=None):
    ncap, NB, ntot = meta['ncap'], meta['NB'], meta['ntot']
    split = meta['split']
    sgs = meta['sgs']
    bands = meta['bands']
    tile_block = meta['tile_block']
    blk_first, blk_last = meta['blk_first'], meta['blk_last']

    nc = bacc.Bacc("TRN2", target_bir_lowering=False, debug=False,
                   enable_asserts=False, num_devices=NCORES,
                   num_swdge_queues=4)

    def din(name, shape, dt):
        return nc.dram_tensor(name, shape, dt, kind="ExternalInput").ap()

    xrows_lo = din('xrows_lo', [split, P], BF16)
    xrows_hi = din('xrows_hi', [NCORES * ncap - split, P], BF16)
    xte_own = din('xte_own', [FE, ncap], BF16)
    a1t = din('a1t', [FE, FE], BF16)
    v1ext = din('v1ext', [FE, H], BF16)
    ws1ext = din('ws1ext', [FE, H], BF16)
    a2t = din('a2t', [H, H], BF16)
    urow = din('urow', [1, H], BF16)
    v2w = din('v2w', [H, H], BF16)
    ws2 = din('ws2', [H, H], BF16)
    b2col = din('b2col', [H, 1], FP32)
    seqw = din('seqw', [SEQ, H], BF16)
    seqb = din('seqb', [1, H], BF16)
    fc1w = din('fc1w', [2 * H, H], BF16)
    fc1b = din('fc1b', [1, H], BF16)
    fc2w = din('fc2w', [H, 1], BF16)
    fc2b = din('fc2b', [1, 1], BF16)
    iota_in = din('iota', [P, P], BF16)
    seqT = din('seqT', [SEQ, GPC], BF16)
    kvidx = din('kvidx', [P, ntot * 8], I16)
    premask = din('premask', [P, meta['pm_cols']], BF16)
    glocal = din('glocal', [P, NB], FP32)

    out_g = nc.dram_tensor('out_g', [GPC, 1], FP32, kind="ExternalOutput").ap()
    if dbg:
        dbg_o = nc.dram_tensor('dbg_o', [P, NB * P], FP32,
                               kind="ExternalOutput").ap()

    h1_sh = nc.dram_tensor('h1_sh', [ncap, H], BF16, kind="Internal").ap()
    h1_all = nc.dram_tensor('h1_all', [NCORES * ncap, H], BF16,
                            kind="Internal", addr_space="Shared").ap()

    from contextlib import ExitStack
    with TileContext(nc, num_cores=NCORES) as tc, ExitStack() as _st:
        cpool = _st.enter_context(tc.tile_pool(name="consts", bufs=1))
        pool = _st.enter_context(tc.tile_pool(name="work", bufs=3))
        spool = _st.enter_context(tc.tile_pool(name="stage", bufs=5))
        epool = _st.enter_context(tc.tile_pool(name="exps", bufs=4))
        wpool = _st.enter_context(tc.tile_pool(name="wts", bufs=6))
        pmpool = _st.enter_context(tc.tile_pool(name="pmask", bufs=4))
        persist = _st.enter_context(tc.tile_pool(name="persist", bufs=1))
        psS = _st.enter_context(tc.tile_pool(name="psS", bufs=3, space="PSUM"))
        psT = _st.enter_context(tc.tile_pool(name="psT", bufs=2, space="PSUM"))
        psB = _st.enter_context(tc.tile_pool(name="psB", bufs=3, space="PSUM"))

        # ---------------- constants
        iota = cpool.tile([P, P], BF16)
        nc.sync.dma_start(out=iota[:], in_=iota_in)
        ident = cpool.tile([P, P], BF16)
        make_identity(nc, ident[:])
        ones_col = cpool.tile([P, 1], BF16)
        nc.vector.memset(ones_col[:], 1.0)
        ones_row = cpool.tile([1, P], BF16)
        nc.vector.memset(ones_row[:], 1.0)
        ones_row_f = cpool.tile([1, P], FP32)
        nc.vector.memset(ones_row_f[:], 1.0)
        zero_row = cpool.tile([1, 2 * P], BF16)
        nc.vector.memset(zero_row[:], 0.0)

        _cn = [0]

        def const_tile(ap_, shape, dt=BF16):
            _cn[0] += 1
            t = cpool.tile(shape, dt, tag=f"c{_cn[0]}", name=f"c{_cn[0]}")
            nc.sync.dma_start(out=t[:], in_=ap_)
            return t

        kvidx_t = const_tile(kvidx, [P, ntot * 8], I16)
        a1t_t = const_tile(a1t, [FE, FE])
        v1ext_t = const_tile(v1ext, [FE, H])
        ws1ext_t = const_tile(ws1ext, [FE, H])
        a2t_t = const_tile(a2t, [H, H])
        urow_t = const_tile(urow, [1, H])
        v2w_t = const_tile(v2w, [H, H])
        ws2_t = const_tile(ws2, [H, H])
        b2col_t = const_tile(b2col, [H, 1], FP32)
        seqb_t = const_tile(seqb, [1, H])
        fc1b_t = const_tile(fc1b, [1, H])
        fc2w_t = const_tile(fc2w, [H, 1])
        fc2b_t = const_tile(fc2b, [1, 1])
        glocal_t = const_tile(glocal, [P, NB], FP32)
        xall = const_tile(xte_own, [FE, ncap])      # full own-x, column-major

        h1T_own = persist.tile([P, NB * P], BF16)   # h1 transposed, own shard

        # ---------------- B1 blocks: B1_b = A1 @ x_d^T  (score rhs, layer 1)
        B1_all = persist.tile([FE, NB * P], BF16)
        for b in range(NB):
            bp = psS.tile([FE, P], FP32, space="PSUM", tag="psS",
                          name=f"b1_{b}")
            nc.tensor.matmul(out=bp[:], lhsT=a1t_t[:],
                             rhs=xall[:, b * P:(b + 1) * P],
                             start=True, stop=True)
            if b % 2 == 0:
                nc.vector.tensor_copy(out=B1_all[:, b * P:(b + 1) * P],
                                      in_=bp[:])
            else:
                nc.scalar.copy(out=B1_all[:, b * P:(b + 1) * P], in_=bp[:])

        # ---------------- edge phase (shared for both layers)
        # Per-block PSUM bank "blk" [P, 4*P] f32:
        #   [:, 0:P]      attention aggregate (raw-feature space)
        #   [0:1, P:2P]   denominator row (layer 2 only; layer 1 uses agg row 64)
        #   [:, 2P:3P]    skip + value-projection accumulator
        #   [:, 3P:4P]    scratch (reciprocal broadcast)
        def edge_phase(layer, tab_lo_ap, tab_hi_ap, rhs_for_block,
                       skip_for_block, on_block_done):
            """rhs_for_block(b) -> (rhs_tile_ap, cp) SBUF [cp,128] dst feats.
            skip_for_block(b, blk) opens blk[:, 2P:3P] accumulation with skip.
            on_block_done(b, h_sb, blk) with h_sb [128,128] = relu'd out^T.

            Software-pipelined with a one-supergroup skew: iteration k emits
            supergroup k's gather/transpose/S/exp (front) and supergroup k-1's
            W/aggregate/epilogue (back), so the in-order engines stream without
            cross-stage stalls."""
            _q = [0]
            _blks = {}
            cp_agg = FE if layer == 1 else H
            SW = 4 * P                     # strip width (full PSUM bank)
            nsg = len(sgs)
            stash = [None] * nsg
            tile_se = meta['tile_se']
            strip_used = meta['strip_used']
            strip_base = meta['strip_base']

            def front(k):
                (t0, lo, tsg, blocks) = sgs[k]
                kv_t = pool.tile([P, tsg, H], BF16, tag="kv_g", bufs=5)
                for (h0, h1, hh) in ((0, lo, 0), (lo, tsg, 1)):
                    for ps0 in range(h0, h1, GMAX):
                        pe0 = min(ps0 + GMAX, h1)
                        tab_h = tab_lo_ap if hh == 0 else tab_hi_ap
                        nc.gpsimd.dma_gather(
                            out_ap=kv_t[:, ps0:pe0, :], in_ap=tab_h,
                            idxs_ap=kvidx_t[:, (t0 + ps0) * 8:(t0 + pe0) * 8],
                            num_idxs=(pe0 - ps0) * P, num_idxs_reg=(pe0 - ps0) * P,
                            elem_size=H, queue_num=_q[0] % 4)
                        _q[0] += 1

                # transposes (PE) + batch copies (scalar engine)
                pT_sbs = {}
                for j0 in range(0, tsg, TTB):
                    j1 = min(j0 + TTB, tsg)
                    psT_t = psT.tile([P, TTB, P], BF16, space="PSUM",
                                     tag="psT")
                    for tl in range(j0, j1):
                        nc.tensor.transpose(
                            out=psT_t[:, tl - j0, :], in_=kv_t[:, tl, :],
                            identity=ident[:])
                    sb = spool.tile([P, TTB, P], BF16, tag="pT_sb")
                    if (j0 // TTB) % 2 == 0:
                        nc.scalar.copy(out=sb[:, 0:j1 - j0, :],
                                       in_=psT_t[:, 0:j1 - j0, :])
                    else:
                        nc.vector.tensor_copy(out=sb[:, 0:j1 - j0, :],
                                              in_=psT_t[:, 0:j1 - j0, :])
                    pT_sbs[j0 // TTB] = sb

                # banded S matmuls packed into full-bank strips + exp (Act)
                strips = {}
                for tl in range(tsg):
                    tt = t0 + tl
                    b = int(tile_block[tt])
                    rhs_sb, cp = rhs_for_block(b)
                    dlo, w = bands[tt]
                    gid, off, w2 = tile_se[tt]
                    if gid not in strips:
                        strips[gid] = psS.tile([P, SW], FP32, space="PSUM",
                                               tag="psS",
                                               name=f"st{layer}_{tt}")
                    pT_sb = pT_sbs[tl // TTB]
                    nc.tensor.matmul(
                        out=strips[gid][:, off:off + w],
                        lhsT=pT_sb[0:cp, tl % TTB, :],
                        rhs=rhs_sb[:, dlo:dlo + w],
                        start=True, stop=True)

                E_sbs = {}
                pm_sbs = {}
                for gid, st in strips.items():
                    esb = epool.tile([P, SW], BF16, tag="E_sb",
                                     name=f"esb{layer}_{gid}")
                    used = strip_used[gid]
                    nc.scalar.activation(out=esb[:, 0:used], in_=st[:, 0:used],
                                         func=AF.Exp, scale=float(SCALE))
                    E_sbs[gid] = esb
                    pm_sb = pmpool.tile([P, SW], BF16, tag="pm")
                    nc.sync.dma_start(
                        out=pm_sb[:, 0:used],
                        in_=premask[:, int(strip_base[gid]):
                                    int(strip_base[gid]) + used])
                    pm_sbs[gid] = pm_sb
                stash[k] = (t0, tsg, kv_t, E_sbs, pm_sbs)

            def back(k):
                (t0, tsg, kv_t, E_sbs, pm_sbs) = stash[k]
                stash[k] = None
                Ws = {}
                for tl in range(tsg):
                    tt = t0 + tl
                    b = int(tile_block[tt])
                    gid, off, w = tile_se[tt]
                    if gid not in Ws:
                        used = strip_used[gid]
                        Wt = wpool.tile([P, SW], BF16, tag="W")
                        nc.vector.tensor_tensor(
                            out=Wt[:, 0:used],
                            in0=pm_sbs[gid][:, 0:used],
                            in1=E_sbs[gid][:, 0:used], op=OP.mult)
                        Ws[gid] = Wt
                    W = Ws[gid]
                    if b not in _blks:
                        blk = psB.tile([P, 4 * P], FP32, space="PSUM",
                                       tag="blk", name=f"blk{layer}_{b}")
                        _blks[b] = blk
                        # zero acc+den regions, open the accumulation group
                        nc.tensor.matmul(out=blk[:, 0:2 * P],
                                         lhsT=zero_row[:1, 0:P],
                                         rhs=zero_row[:1, :],
                                         start=True, stop=False)
                    blk = _blks[b]
                    dlo = bands[tt][0]
                    nc.tensor.matmul(
                        out=blk[0:cp_agg, dlo:dlo + w],
                        lhsT=kv_t[:, tl, 0:cp_agg],
                        rhs=W[:, off:off + w], start=False, stop=False)
                    if layer == 2:
                        nc.tensor.matmul(
                            out=blk[0:1, P + dlo:P + dlo + w], lhsT=ones_col[:],
                            rhs=W[:, off:off + w], start=False, stop=False)
                    if tt == blk_last[b]:
                        # close the accumulation group (flush)
                        nc.tensor.matmul(out=blk[:, 0:2 * P],
                                         lhsT=zero_row[:1, 0:P],
                                         rhs=zero_row[:1, :],
                                         start=False, stop=True)
                        # epilogue: normalize + project + skip + relu
                        _blks.pop(b)
                        den_ap = (blk[F_IN:FE, 0:P] if layer == 1
                                  else blk[0:1, P:2 * P])
                        dv = pool.tile([1, P], FP32, tag="dv")
                        nc.vector.tensor_scalar_add(out=dv[:], in0=den_ap,
                                                    scalar1=EPS)
                        rv = pool.tile([1, P], FP32, tag="rv")
                        nc.vector.reciprocal(out=rv[:], in_=dv[:])
                        nc.tensor.matmul(out=blk[0:cp_agg, 3 * P:4 * P],
                                         lhsT=ones_row_f[:1, 0:cp_agg],
                                         rhs=rv[:], start=True, stop=True)
                        dnb = pool.tile([cp_agg, P], FP32, tag="dnb",
                                        name=f"dn{layer}_{b}")
                        nc.scalar.copy(out=dnb[:], in_=blk[0:cp_agg, 3 * P:4 * P])
                        xnorm = pool.tile([cp_agg, P], BF16, tag="xnorm",
                                          name=f"xn{layer}_{b}")
                        nc.vector.tensor_tensor(
                            out=xnorm[:], in0=blk[0:cp_agg, 0:P],
                            in1=dnb[:], op=OP.mult)
                        # skip + value projection accumulate back-to-back
                        skip_for_block(b, blk)
                        vproj = v1ext_t if layer == 1 else v2w_t
                        nc.tensor.matmul(out=blk[:, 2 * P:3 * P],
                                         lhsT=vproj[:], rhs=xnorm[:],
                                         start=False, stop=True)
                        h_sb = spool.tile([P, P], BF16, tag="h_sb")
                        if layer == 1:
                            nc.scalar.activation(out=h_sb[:],
                                                 in_=blk[:, 2 * P:3 * P],
                                                 func=AF.Relu)
                        else:
                            nc.scalar.activation(out=h_sb[:],
                                                 in_=blk[:, 2 * P:3 * P],
                                                 func=AF.Relu,
                                                 bias=b2col_t[:, 0:1])
                        on_block_done(b, h_sb, blk)

            for k in range(nsg + 1):
                if k < nsg:
                    front(k)
                if k >= 1:
                    back(k - 1)

        # ---------------- layer 1 plumbing
        def rhs1(b):
            return (B1_all[:, b * P:(b + 1) * P], FE)

        def skip1(b, blk):
            nc.tensor.matmul(out=blk[:, 2 * P:3 * P], lhsT=ws1ext_t[:],
                             rhs=xall[:, b * P:(b + 1) * P],
                             start=True, stop=False)

        def l1_done(b, h_sb, blk):
            nc.vector.tensor_copy(out=h1T_own[:, b * P:(b + 1) * P], in_=h_sb[:])
            # node-major h1 rows for the allgather + layer-2 gathers
            tp = psT.tile([P, P], BF16, space="PSUM", tag="psT",
                          name=f"h1tp{b}")
            nc.tensor.transpose(out=tp[:], in_=h_sb[:], identity=ident[:])
            tsb = spool.tile([P, H], BF16, tag="t2row")
            nc.scalar.copy(out=tsb[:], in_=tp[:])
            nc.sync.dma_start(out=h1_sh[b * P:(b + 1) * P, :], in_=tsb[:])

        edge_phase(1, xrows_lo, xrows_hi, rhs1, skip1, l1_done)

        if dbg == 'h1':
            d = pool.tile([P, NB * P], FP32, tag="dbg")
            nc.vector.tensor_copy(out=d[:], in_=h1T_own[:])
            nc.sync.dma_start(out=dbg_o, in_=d[:])
            do = pool.tile([GPC, 1], FP32, tag="dbgo")
            nc.vector.memset(do[:], 0.5)
            nc.sync.dma_start(out=out_g, in_=do[:])

        # ---------------- collective: allgather raw h1
        nc.gpsimd.collective_compute(
            kind="AllGather", op=OP.bypass,
            replica_groups=[list(range(NCORES))],
            ins=[h1_sh], outs=[h1_all])

        if dbg is None or dbg == 'full':
            # ---------------- layer 2 plumbing
            pool_sb = persist.tile([GPC, H + 1], FP32)
            nc.vector.memset(pool_sb[:], 0.0)

            # score rhs blocks during the collective: R_b = A2 h1_d^T + u
            R_all = persist.tile([H, NB * P], BF16)
            for b in range(NB):
                rp = psS.tile([H, P], FP32, space="PSUM", tag="psS",
                              name=f"r2_{b}")
                nc.tensor.matmul(out=rp[:], lhsT=a2t_t[:],
                                 rhs=h1T_own[:, b * P:(b + 1) * P],
                                 start=True, stop=False)
                nc.tensor.matmul(out=rp[:], lhsT=urow_t[:1, :],
                                 rhs=ones_row[:1, :], start=False, stop=True)
                if b % 2 == 0:
                    nc.vector.tensor_copy(out=R_all[:, b * P:(b + 1) * P],
                                          in_=rp[:])
                else:
                    nc.scalar.copy(out=R_all[:, b * P:(b + 1) * P], in_=rp[:])

            def rhs2(b):
                return (R_all[:, b * P:(b + 1) * P], H)

            def skip2(b, blk):
                nc.tensor.matmul(out=blk[:, 2 * P:3 * P], lhsT=ws2_t[:],
                                 rhs=h1T_own[:, b * P:(b + 1) * P],
                                 start=True, stop=False)

            def l2_done(b, h_sb, blk):
                # transpose h2^T -> h2 [d, h], then pool matmul
                tp = psT.tile([P, P], BF16, space="PSUM", tag="psT",
                              name=f"h2tp{b}")
                nc.tensor.transpose(out=tp[:], in_=h_sb[:], identity=ident[:])
                h2x = pool.tile([P, H + 1], BF16, tag="h2x")
                nc.scalar.copy(out=h2x[:, 0:H], in_=tp[:])
                nc.vector.memset(h2x[:, H:H + 1], 1.0)
                gh = pool.tile([P, GPC], BF16, tag="gh")
                nc.vector.tensor_scalar(
                    out=gh[:], in0=iota[:, 0:GPC], scalar1=glocal_t[:, b:b + 1],
                    scalar2=None, op0=OP.is_equal)
                nc.tensor.matmul(out=blk[0:GPC, 2 * P:2 * P + H + 1],
                                 lhsT=gh[:], rhs=h2x[:],
                                 start=True, stop=True)
                nc.vector.tensor_tensor(out=pool_sb[:], in0=pool_sb[:],
                                        in1=blk[0:GPC, 2 * P:2 * P + H + 1],
                                        op=OP.add)

            # seq branch computed during the collective window
            seqw_t0 = const_tile(seqw[0:P, :], [P, H])
            seqw_t1 = const_tile(seqw[P:SEQ, :], [P, H])
            fc1w_t0 = const_tile(fc1w[0:P, :], [P, H])
            fc1w_t1 = const_tile(fc1w[P:2 * H, :], [P, H])
            seqT0 = const_tile(seqT[0:P, :], [P, GPC])
            seqT1 = const_tile(seqT[P:SEQ, :], [P, GPC])
            z = pool.tile([GPC, 2 * H], BF16, tag="z")
            pseq = psS.tile([GPC, H], FP32, space="PSUM", tag="psS",
                            name="pseq")
            nc.tensor.matmul(out=pseq[:], lhsT=seqT0[:], rhs=seqw_t0[:],
                             start=True, stop=False)
            nc.tensor.matmul(out=pseq[:], lhsT=seqT1[:], rhs=seqw_t1[:],
                             start=False, stop=False)
            nc.tensor.matmul(out=pseq[:], lhsT=ones_row[:1, 0:GPC],
                             rhs=seqb_t[:1, :], start=False, stop=True)
            nc.scalar.activation(out=z[:, H:2 * H], in_=pseq[:], func=AF.Relu)

            edge_phase(2, h1_all[0:split, :], h1_all[split:NCORES * ncap, :],
                       rhs2, skip2, l2_done)

            # ---------------- tail: pooled -> MLP -> sigmoid
            cnt = pool.tile([GPC, 1], FP32, tag="cnt")
            nc.vector.tensor_scalar_add(out=cnt[:], in0=pool_sb[:, H:H + 1],
                                        scalar1=EPS)
            rc = pool.tile([GPC, 1], FP32, tag="rc")
            nc.vector.reciprocal(out=rc[:], in_=cnt[:])
            nc.vector.tensor_scalar(out=z[:, 0:H], in0=pool_sb[:, 0:H],
                                    scalar1=rc[:, 0:1], scalar2=None,
                                    op0=OP.mult)

            zT = []
            for i in range(2):
                tzp = psT.tile([P, GPC], BF16, space="PSUM", tag="psT",
                               name=f"tzp{i}")
                nc.tensor.transpose(out=tzp[:], in_=z[:, i * H:(i + 1) * H],
                                    identity=ident[0:GPC, 0:GPC])
                zt = pool.tile([P, GPC], BF16, tag=f"zT{i}")
                nc.vector.tensor_copy(out=zt[:], in_=tzp[:])
                zT.append(zt)
            pfc1 = psS.tile([GPC, H], FP32, space="PSUM", tag="psS",
                            name="pfc1")
            nc.tensor.matmul(out=pfc1[:], lhsT=zT[0][:], rhs=fc1w_t0[:],
                             start=True, stop=False)
            nc.tensor.matmul(out=pfc1[:], lhsT=zT[1][:], rhs=fc1w_t1[:],
                             start=False, stop=False)
            nc.tensor.matmul(out=pfc1[:], lhsT=ones_row[:1, 0:GPC],
                             rhs=fc1b_t[:1, :], start=False, stop=True)
            z1 = pool.tile([GPC, H], BF16, tag="z1")
            nc.scalar.activation(out=z1[:], in_=pfc1[:], func=AF.Relu)
            tz1 = psT.tile([P, GPC], BF16, space="PSUM", tag="psT",
                           name="tz1")
            nc.tensor.transpose(out=tz1[:], in_=z1[:],
                                identity=ident[0:GPC, 0:GPC])
            z1T = pool.tile([P, GPC], BF16, tag="z1T")
            nc.vector.tensor_copy(out=z1T[:], in_=tz1[:])
            pfc2 = psS.tile([GPC, 1], FP32, space="PSUM", tag="psS",
                            name="pfc2")
            nc.tensor.matmul(out=pfc2[:], lhsT=z1T[:], rhs=fc2w_t[:],
                             start=True, stop=False)
            nc.tensor.matmul(out=pfc2[:], lhsT=ones_row[:1, 0:GPC],
                             rhs=fc2b_t[:1, :], start=False, stop=True)
            outs = pool.tile([GPC, 1], FP32, tag="outs")
            nc.scalar.activation(out=outs[:], in_=pfc2[:], func=AF.Sigmoid)
            nc.sync.dma_start(out=out_g, in_=outs[:])

    nc.compile()
    return nc


# ---------------------------------------------------------------- entry

_CACHE = {}


def kernel(**inputs):
    meta = preprocess(inputs)
    key = (meta['ncap'], meta['ntot'], tuple(meta['tile_block'].tolist()))
    if key not in _CACHE:
        _CACHE[key] = build_program(meta)
    nc = _CACHE[key]
    in_maps = make_inputs(inputs, meta)
    res = run_bass_kernel_spmd(nc, in_maps, core_ids=list(range(NCORES)))
    out = np.concatenate([res.results[c]['out_g'] for c in range(NCORES)], 0)
    return out.astype(np.float32)


# revision 15
# speedup vs baseline: 1.4910x; 1.0595x over previous
"""Trainium2 Bass kernel for nn_Discriminator (2x TransformerConv GNN + pool + MLP).

v3 design (raw-feature gathers, no node-table prologue):
- Graphs split 64-per-core (batch sorted => contiguous node ranges per core).
- L1 gathers RAW x_ext rows ([x|1|0pad] -> 128 bf16 cols, 256B) straight from
  a host-built row table; the score projection A1 is applied per dst-block
  (B1_b = A1 @ x_d^T), and the value projection v1 is applied per block AFTER
  aggregation (linearity). The ones column of x_ext yields the softmax
  denominator for free in the aggregation matmul.
- L2 allgathers RAW h1 ([N,128] bf16, half the bytes of a v|p table) and
  applies A2/u per block (R_b = A2 h1_d + u) and v2 after aggregation.
- Per tile: PE transpose of the gathered rows, S = X_sT^T @ B_b (PE), batched
  exp (scalar engine), W = onehot(dst) * exp(S) in one DVE op, aggregation
  via one PE matmul (plus a denominator matmul for L2 only).

Self-contained: hardcodes problem shapes; layout computed from runtime inputs.
"""
import numpy as np
import ml_dtypes

import concourse.bass as bass
import concourse.bacc as bacc
import concourse.mybir as mybir
from concourse.tile import TileContext
from concourse.masks import make_identity
from concourse.bass_utils import run_bass_kernel_spmd

BF = np.float16
N, E, G = 50000, 800000, 512
F_IN, H, SEQ = 64, 128, 256
FE = F_IN + 1                 # x extended with ones column
NCORES = 8
GPC = G // NCORES             # graphs per core
P = 128
SGB = 4                       # blocks per supergroup
GMAX = 8                      # tiles per gather call (1024 idxs; runtime ucode cap)
TTB = 8                       # tiles per transpose/copy batch (one bf16 bank)
SCALE = 1.0 / np.sqrt(np.float32(H))
EPS = 1e-30

FP32 = mybir.dt.float32
BF16 = mybir.dt.float16
F8 = mybir.dt.float8e4
NCHUNK = 4
I16 = mybir.dt.int16
AF = mybir.ActivationFunctionType
OP = mybir.AluOpType


# ---------------------------------------------------------------- host prep

def _pack_idx(idx_stream):
    """idx_stream [ntot*128] -> [128, ntot*8] int16 (16-partition wrap, x8)."""
    n = idx_stream.shape[0]
    s = n // 16
    out = np.zeros((128, s), dtype=np.int16)
    arr = idx_stream.reshape(s, 16).T.astype(np.int16)
    for g in range(8):
        out[g * 16:(g + 1) * 16, :] = arr
    return out


def preprocess(inputs):
    batch = np.asarray(inputs['batch']).astype(np.int64)
    ei = np.asarray(inputs['edge_index']).astype(np.int64)
    src_g, dst_g = ei[0], ei[1]

    gstart = np.searchsorted(batch, np.arange(NCORES) * GPC)
    gend = np.searchsorted(batch, np.arange(NCORES) * GPC + GPC)
    nloc = gend - gstart
    ncap = int(np.ceil(nloc.max() / (2 * P)) * (2 * P))   # even block count
    NB = ncap // P
    split = (NCORES // 2) * ncap          # lo/hi table split row

    node_core = batch // GPC
    node_local = np.arange(N) - gstart[node_core]
    table_idx = node_core * ncap + node_local            # row in [8*ncap] table
    node_half = (table_idx >= split).astype(np.int64)
    half_idx = table_idx - node_half * split             # row within half

    edge_core = node_core[dst_g]
    per_core = []
    for c in range(NCORES):
        em = np.where(edge_core == c)[0]
        e_src, e_dst = src_g[em], dst_g[em]
        order = np.argsort(e_dst, kind='stable')
        e_src, e_dst = e_src[order], e_dst[order]
        dst_loc = e_dst - gstart[c]
        tsrc = half_idx[e_src]
        is_hi = node_half[e_src]
        blk = dst_loc // P
        buckets = {}
        for b in range(NB):
            bm = np.where(blk == b)[0]
            bh = is_hi[bm]
            for half in (0, 1):
                hm = bm[bh == half]
                buckets[(b, half)] = (tsrc[hm], dst_loc[hm] - b * P)
        per_core.append(buckets)

    # uniform tile counts per (b, half)
    tcount = {}
    for b in range(NB):
        for half in (0, 1):
            mx = max(len(per_core[c][(b, half)][0]) for c in range(NCORES))
            tcount[(b, half)] = (mx + P - 1) // P

    # supergroups and stream layout (shared across cores)
    sgs = []           # (t0, Tlo, Tsg, blocks)
    tile_block = []    # per tile: block id
    t0 = 0
    for s0 in range(0, NB, SGB):
        blocks = list(range(s0, min(s0 + SGB, NB)))
        lo = sum(tcount[(b, 0)] for b in blocks)
        hi = sum(tcount[(b, 1)] for b in blocks)
        for half in (0, 1):
            for b in blocks:
                tile_block += [b] * tcount[(b, half)]
        sgs.append((t0, lo, lo + hi, blocks))
        t0 += lo + hi
    ntot = t0
    tile_block = np.array(tile_block)
    blk_first = {b: int(np.where(tile_block == b)[0][0]) for b in range(NB)}
    blk_last = {b: int(np.where(tile_block == b)[0][-1]) for b in range(NB)}

    # per-core streams
    dls = []
    kvis = []
    for c in range(NCORES):
        kvi = np.zeros(ntot * P, np.int64)
        dl = np.full(ntot * P, -1.0, np.float32)
        pos = 0
        for (t0_, lo, tsg, blocks) in sgs:
            for half in (0, 1):
                for b in blocks:
                    k, d = per_core[c][(b, half)]
                    ntile = tcount[(b, half)]
                    cnt = len(k)
                    kvi[pos:pos + cnt] = k
                    dl[pos:pos + cnt] = d
                    pos += ntile * P
        assert pos == ntot * P
        dls.append(dl.reshape(ntot, P))
        kvis.append(kvi)

    # per-tile dst band: union across cores (program structure is shared)
    bands = []
    for tt in range(ntot):
        lo128, hi = P, -1
        for c in range(NCORES):
            v = dls[c][tt]
            v = v[v >= 0]
            if len(v):
                lo128 = min(lo128, int(v.min()))
                hi = max(hi, int(v.max()))
        if hi < 0:
            lo128, hi = 0, 0
        bands.append((lo128, hi - lo128 + 1))

    # global strip packing (must match edge_phase): strips restart per sg
    SW = 4 * P
    tile_se = [None] * ntot       # tt -> (strip gid, off, w)
    strip_used = []               # gid -> used cols
    strip_sg = []                 # gid -> sg index
    for si_sg, (t0_, lo, tsg, blocks) in enumerate(sgs):
        cur = -1
        for tl in range(tsg):
            tt = t0_ + tl
            w = bands[tt][1]
            if cur < 0 or strip_used[cur] + w > SW:
                strip_used.append(0)
                strip_sg.append(si_sg)
                cur = len(strip_used) - 1
            tile_se[tt] = (cur, strip_used[cur], w)
            strip_used[cur] += w
    strip_base = np.concatenate([[0], np.cumsum(strip_used)]).astype(int)
    pm_cols = int(strip_base[-1])

    cores = []
    for c in range(NCORES):
        dl = dls[c]
        dlb = dl.copy()
        for tt in range(ntot):
            m = dlb[tt] >= 0
            dlb[tt][m] -= bands[tt][0]
        pm = np.zeros((P, pm_cols), np.float16)
        for tt in range(ntot):
            gid, off, w = tile_se[tt]
            col = strip_base[gid] + off
            d = dlb[tt]
            j = d.astype(np.int64)
            e = np.where(d >= 0)[0]
            pm[e, col + j[e]] = 1.0
        gl = np.full(ncap, -1.0, np.float32)
        gl[:nloc[c]] = (batch[gstart[c]:gend[c]] - c * GPC).astype(np.float32)
        cores.append({
            'kvidx': _pack_idx(kvis[c]),
            'premask': pm,                                     # [128, pm_cols]
            'glocal': gl.reshape(NB, P).T.astype(np.float32),  # [128, NB]
        })

    return {
        'ncap': ncap, 'NB': NB, 'split': split, 'ntot': ntot,
        'gstart': gstart, 'gend': gend, 'nloc': nloc,
        'table_idx': table_idx, 'sgs': sgs,
        'tile_block': tile_block, 'blk_first': blk_first, 'blk_last': blk_last,
        'bands': bands, 'cores': cores,
        'tile_se': tile_se, 'strip_used': strip_used, 'strip_sg': strip_sg,
        'strip_base': strip_base, 'pm_cols': pm_cols,
    }


def make_inputs(inputs, meta):
    ncap = meta['ncap']
    x = np.asarray(inputs['x'], np.float32)
    f32 = lambda v: np.asarray(v, np.float32)

    # raw x_ext row table [8*ncap, 128]: [x | 1 | 0pad], gathered by L1
    xrows = np.zeros((NCORES * ncap, P), np.float32)
    xrows[meta['table_idx'], 0:F_IN] = x
    xrows[:, F_IN] = 1.0
    xrows_bf = xrows.astype(BF)

    # column-major x_ext per core (dst side): [FE, ncap]
    xte = np.zeros((FE, NCORES * ncap), np.float32)
    xte[F_IN, :] = 1.0
    xte[:F_IN, meta['table_idx']] = x.T
    xte_bf = xte.astype(BF)

    # layer-1: score = x_ext[s] A1 x_ext[d]^T; A1 = wk1 wq1^T
    wk1 = np.concatenate([f32(inputs['k1_w']), f32(inputs['k1_b'])[None, :]], 0)
    wq1 = np.concatenate([f32(inputs['q1_w']), f32(inputs['q1_b'])[None, :]], 0)
    A1 = wk1 @ wq1.T                                     # [FE, FE]
    v1ext = np.zeros((FE, H), np.float32)
    v1ext[:F_IN, :] = f32(inputs['v1_w'])
    v1ext[F_IN, :] = f32(inputs['v1_b'])
    ws1ext = np.zeros((FE, H), np.float32)
    ws1ext[:F_IN, :] = f32(inputs['s1_w'])
    ws1ext[F_IN, :] = f32(inputs['s1_b'])

    # layer-2: score = h1[s] A2 h1[d] + u.h1[s] (+ dst-only terms cancel)
    A2 = f32(inputs['k2_w']) @ f32(inputs['q2_w']).T     # [H, H]
    u = f32(inputs['k2_w']) @ f32(inputs['q2_b'])        # [H]
    b2col = (f32(inputs['s2_b']) + f32(inputs['v2_b']))[:, None]  # [H,1]

    seqc = np.asarray(inputs['sequence_character'], np.float32)
    split = meta['split']

    shared = {
        'xrows_lo': np.ascontiguousarray(xrows_bf[:split]),
        'xrows_hi': np.ascontiguousarray(xrows_bf[split:]),
        'a1t': np.ascontiguousarray(A1.T.astype(BF)),
        'v1ext': v1ext.astype(BF),
        'ws1ext': ws1ext.astype(BF),
        'a2t': np.ascontiguousarray(A2.T.astype(BF)),
        'urow': u[None, :].astype(BF),
        'v2w': f32(inputs['v2_w']).astype(BF),
        'ws2': f32(inputs['s2_w']).astype(BF),
        'b2col': b2col.astype(np.float32),
        'seqw': f32(inputs['seq_w']).astype(BF),
        'seqb': f32(inputs['seq_b'])[None, :].astype(BF),
        'fc1w': f32(inputs['fc1_w']).astype(BF),
        'fc1b': f32(inputs['fc1_b'])[None, :].astype(BF),
        'fc2w': f32(inputs['fc2_w']).astype(BF),
        'fc2b': f32(inputs['fc2_b'])[None, :].astype(BF),
        'iota': np.tile(np.arange(P, dtype=np.float32)[None, :], (P, 1)).astype(BF),
    }

    in_maps = []
    for c in range(NCORES):
        m = dict(shared)
        m['xte_own'] = np.ascontiguousarray(xte_bf[:, c * ncap:(c + 1) * ncap])
        m['seqT'] = np.ascontiguousarray(
            seqc[c * GPC:(c + 1) * GPC].T.astype(BF))            # [256, 64]
        mc = meta['cores'][c]
        m['kvidx'] = mc['kvidx']
        m['premask'] = mc['premask']
        m['glocal'] = mc['glocal']
        in_maps.append(m)
    return in_maps


# ---------------------------------------------------------------- program

def build_program(meta, dbg=None):
    ncap, NB, ntot = meta['ncap'], meta['NB'], meta['ntot']
    split = meta['split']
    sgs = meta['sgs']
    bands = meta['bands']
    tile_block = meta['tile_block']
    blk_first, blk_last = meta['blk_first'], meta['blk_last']

    nc = bacc.Bacc("TRN2", target_bir_lowering=False, debug=False,
                   enable_asserts=False, num_devices=NCORES,
                   num_swdge_queues=4)

    def din(name, shape, dt):
        return nc.dram_tensor(name, shape, dt, kind="ExternalInput").ap()

    xrows_lo = din('xrows_lo', [split, P], BF16)
    xrows_hi = din('xrows_hi', [NCORES * ncap - split, P], BF16)
    xte_own = din('xte_own', [FE, ncap], BF16)
    a1t = din('a1t', [FE, FE], BF16)
    v1ext = din('v1ext', [FE, H], BF16)
    ws1ext = din('ws1ext', [FE, H], BF16)
    a2t = din('a2t', [H, H], BF16)
    urow = din('urow', [1, H], BF16)
    v2w = din('v2w', [H, H], BF16)
    ws2 = din('ws2', [H, H], BF16)
    b2col = din('b2col', [H, 1], FP32)
    seqw = din('seqw', [SEQ, H], BF16)
    seqb = din('seqb', [1, H], BF16)
    fc1w = din('fc1w', [2 * H, H], BF16)
    fc1b = din('fc1b', [1, H], BF16)
    fc2w = din('fc2w', [H, 1], BF16)
    fc2b = din('fc2b', [1, 1], BF16)
    iota_in = din('iota', [P, P], BF16)
    seqT = din('seqT', [SEQ, GPC], BF16)
    kvidx = din('kvidx', [P, ntot * 8], I16)
    premask = din('premask', [P, meta['pm_cols']], BF16)
    glocal = din('glocal', [P, NB], FP32)

    out_g = nc.dram_tensor('out_g', [GPC, 1], FP32, kind="ExternalOutput").ap()
    if dbg:
        dbg_o = nc.dram_tensor('dbg_o', [P, NB * P], FP32,
                               kind="ExternalOutput").ap()

    cbase = NB // NCHUNK
    crem = NB % NCHUNK
    csizes = [cbase + 1] * crem + [cbase] * (NCHUNK - crem)   # blocks per chunk
    cstart = np.concatenate([[0], np.cumsum(csizes)]).astype(int)
    chunk_of = np.repeat(np.arange(NCHUNK), csizes)
    h1f8_sh = [nc.dram_tensor(f'h1f8_sh{i}', [csizes[i] * P, H], F8,
                              kind="Internal").ap() for i in range(NCHUNK)]
    h1f8_ag = [nc.dram_tensor(f'h1f8_ag{i}', [NCORES, csizes[i] * P, H], F8,
                              kind="Internal", addr_space="Shared").ap()
               for i in range(NCHUNK)]
    h1x_all = nc.dram_tensor('h1x_all', [NCORES * ncap, H], BF16,
                             kind="Internal").ap()

    from contextlib import ExitStack
    with TileContext(nc, num_cores=NCORES) as tc, ExitStack() as _st:
        cpool = _st.enter_context(tc.tile_pool(name="consts", bufs=1))
        pool = _st.enter_context(tc.tile_pool(name="work", bufs=3))
        spool = _st.enter_context(tc.tile_pool(name="stage", bufs=5))
        epool = _st.enter_context(tc.tile_pool(name="exps", bufs=4))
        wpool = _st.enter_context(tc.tile_pool(name="wts", bufs=6))
        pmpool = _st.enter_context(tc.tile_pool(name="pmask", bufs=4))
        xpool = _st.enter_context(tc.tile_pool(name="expand", bufs=2))
        persist = _st.enter_context(tc.tile_pool(name="persist", bufs=1))
        psS = _st.enter_context(tc.tile_pool(name="psS", bufs=3, space="PSUM"))
        psT = _st.enter_context(tc.tile_pool(name="psT", bufs=2, space="PSUM"))
        psB = _st.enter_context(tc.tile_pool(name="psB", bufs=3, space="PSUM"))

        # ---------------- constants
        iota = cpool.tile([P, P], BF16)
        nc.sync.dma_start(out=iota[:], in_=iota_in)
        ident = cpool.tile([P, P], BF16)
        make_identity(nc, ident[:])
        ones_col = cpool.tile([P, 1], BF16)
        nc.vector.memset(ones_col[:], 1.0)
        ones_row = cpool.tile([1, P], BF16)
        nc.vector.memset(ones_row[:], 1.0)
        ones_row_f = cpool.tile([1, P], FP32)
        nc.vector.memset(ones_row_f[:], 1.0)
        zero_row = cpool.tile([1, 2 * P], BF16)
        nc.vector.memset(zero_row[:], 0.0)

        _cn = [0]

        def const_tile(ap_, shape, dt=BF16):
            _cn[0] += 1
            t = cpool.tile(shape, dt, tag=f"c{_cn[0]}", name=f"c{_cn[0]}")
            nc.sync.dma_start(out=t[:], in_=ap_)
            return t

        kvidx_t = const_tile(kvidx, [P, ntot * 8], I16)
        a1t_t = const_tile(a1t, [FE, FE])
        v1ext_t = const_tile(v1ext, [FE, H])
        ws1ext_t = const_tile(ws1ext, [FE, H])
        a2t_t = const_tile(a2t, [H, H])
        urow_t = const_tile(urow, [1, H])
        v2w_t = const_tile(v2w, [H, H])
        ws2_t = const_tile(ws2, [H, H])
        b2col_t = const_tile(b2col, [H, 1], FP32)
        seqb_t = const_tile(seqb, [1, H])
        fc1b_t = const_tile(fc1b, [1, H])
        fc2w_t = const_tile(fc2w, [H, 1])
        fc2b_t = const_tile(fc2b, [1, 1])
        glocal_t = const_tile(glocal, [P, NB], FP32)
        xall = const_tile(xte_own, [FE, ncap])      # full own-x, column-major

        h1T_own = persist.tile([P, NB * P], BF16)   # h1 transposed, own shard

        # ---------------- B1 blocks: B1_b = A1 @ x_d^T  (score rhs, layer 1)
        B1_all = persist.tile([FE, NB * P], BF16)
        for b in range(NB):
            bp = psS.tile([FE, P], FP32, space="PSUM", tag="psS",
                          name=f"b1_{b}")
            nc.tensor.matmul(out=bp[:], lhsT=a1t_t[:],
                             rhs=xall[:, b * P:(b + 1) * P],
                             start=True, stop=True)
            if b % 2 == 0:
                nc.vector.tensor_copy(out=B1_all[:, b * P:(b + 1) * P],
                                      in_=bp[:])
            else:
                nc.scalar.copy(out=B1_all[:, b * P:(b + 1) * P], in_=bp[:])

        # ---------------- edge phase (shared for both layers)
        # Per-block PSUM bank "blk" [P, 4*P] f32:
        #   [:, 0:P]      attention aggregate (raw-feature space)
        #   [0:1, P:2P]   denominator row (layer 2 only; layer 1 uses agg row 64)
        #   [:, 2P:3P]    skip + value-projection accumulator
        #   [:, 3P:4P]    scratch (reciprocal broadcast)
        def edge_phase(layer, tab_lo_ap, tab_hi_ap, rhs_for_block,
                       skip_for_block, on_block_done):
            """rhs_for_block(b) -> (rhs_tile_ap, cp) SBUF [cp,128] dst feats.
            skip_for_block(b, blk) opens blk[:, 2P:3P] accumulation with skip.
            on_block_done(b, h_sb, blk) with h_sb [128,128] = relu'd out^T.

            Software-pipelined with a one-supergroup skew: iteration k emits
            supergroup k's gather/transpose/S/exp (front) and supergroup k-1's
            W/aggregate/epilogue (back), so the in-order engines stream without
            cross-stage stalls."""
            _q = [0]
            _blks = {}
            cp_agg = FE if layer == 1 else H
            SW = 4 * P                     # strip width (full PSUM bank)
            nsg = len(sgs)
            stash = [None] * nsg
            tile_se = meta['tile_se']
            strip_used = meta['strip_used']
            strip_base = meta['strip_base']

            def front(k):
                (t0, lo, tsg, blocks) = sgs[k]
                kv_t = pool.tile([P, tsg, H], BF16, tag="kv_g", bufs=5)
                for (h0, h1, hh) in ((0, lo, 0), (lo, tsg, 1)):
                    for ps0 in range(h0, h1, GMAX):
                        pe0 = min(ps0 + GMAX, h1)
                        tab_h = tab_lo_ap if hh == 0 else tab_hi_ap
                        nc.gpsimd.dma_gather(
                            out_ap=kv_t[:, ps0:pe0, :], in_ap=tab_h,
                            idxs_ap=kvidx_t[:, (t0 + ps0) * 8:(t0 + pe0) * 8],
                            num_idxs=(pe0 - ps0) * P, num_idxs_reg=(pe0 - ps0) * P,
                            elem_size=H, queue_num=_q[0] % 4)
                        _q[0] += 1

                # transposes (PE) + batch copies (scalar engine)
                pT_sbs = {}
                for j0 in range(0, tsg, TTB):
                    j1 = min(j0 + TTB, tsg)
                    psT_t = psT.tile([P, TTB, P], BF16, space="PSUM",
                                     tag="psT")
                    for tl in range(j0, j1):
                        nc.tensor.transpose(
                            out=psT_t[:, tl - j0, :], in_=kv_t[:, tl, :],
                            identity=ident[:])
                    sb = spool.tile([P, TTB, P], BF16, tag="pT_sb")
                    if (j0 // TTB) % 2 == 0:
                        nc.scalar.copy(out=sb[:, 0:j1 - j0, :],
                                       in_=psT_t[:, 0:j1 - j0, :])
                    else:
                        nc.vector.tensor_copy(out=sb[:, 0:j1 - j0, :],
                                              in_=psT_t[:, 0:j1 - j0, :])
                    pT_sbs[j0 // TTB] = sb

                # banded S matmuls packed into full-bank strips + exp (Act)
                strips = {}
                for tl in range(tsg):
                    tt = t0 + tl
                    b = int(tile_block[tt])
                    rhs_sb, cp = rhs_for_block(b)
                    dlo, w = bands[tt]
                    gid, off, w2 = tile_se[tt]
                    if gid not in strips:
                        strips[gid] = psS.tile([P, SW], FP32, space="PSUM",
                                               tag="psS",
                                               name=f"st{layer}_{tt}")
                    pT_sb = pT_sbs[tl // TTB]
                    nc.tensor.matmul(
                        out=strips[gid][:, off:off + w],
                        lhsT=pT_sb[0:cp, tl % TTB, :],
                        rhs=rhs_sb[:, dlo:dlo + w],
                        start=True, stop=True)

                E_sbs = {}
                pm_sbs = {}
                for gid, st in strips.items():
                    esb = epool.tile([P, SW], BF16, tag="E_sb",
                                     name=f"esb{layer}_{gid}")
                    used = strip_used[gid]
                    nc.scalar.activation(out=esb[:, 0:used], in_=st[:, 0:used],
                                         func=AF.Exp, scale=float(SCALE))
                    E_sbs[gid] = esb
                    pm_sb = pmpool.tile([P, SW], BF16, tag="pm")
                    nc.sync.dma_start(
                        out=pm_sb[:, 0:used],
                        in_=premask[:, int(strip_base[gid]):
                                    int(strip_base[gid]) + used])
                    pm_sbs[gid] = pm_sb
                stash[k] = (t0, tsg, kv_t, E_sbs, pm_sbs)

            def back(k):
                (t0, tsg, kv_t, E_sbs, pm_sbs) = stash[k]
                stash[k] = None
                Ws = {}
                for tl in range(tsg):
                    tt = t0 + tl
                    b = int(tile_block[tt])
                    gid, off, w = tile_se[tt]
                    if gid not in Ws:
                        used = strip_used[gid]
                        Wt = wpool.tile([P, SW], BF16, tag="W")
                        nc.vector.tensor_tensor(
                            out=Wt[:, 0:used],
                            in0=pm_sbs[gid][:, 0:used],
                            in1=E_sbs[gid][:, 0:used], op=OP.mult)
                        Ws[gid] = Wt
                    W = Ws[gid]
                    if b not in _blks:
                        blk = psB.tile([P, 4 * P], FP32, space="PSUM",
                                       tag="blk", name=f"blk{layer}_{b}")
                        _blks[b] = blk
                        # zero acc+den regions, open the accumulation group
                        nc.tensor.matmul(out=blk[:, 0:2 * P],
                                         lhsT=zero_row[:1, 0:P],
                                         rhs=zero_row[:1, :],
                                         start=True, stop=False)
                    blk = _blks[b]
                    dlo = bands[tt][0]
                    nc.tensor.matmul(
                        out=blk[0:cp_agg, dlo:dlo + w],
                        lhsT=kv_t[:, tl, 0:cp_agg],
                        rhs=W[:, off:off + w], start=False, stop=False)
                    if layer == 2:
                        nc.tensor.matmul(
                            out=blk[0:1, P + dlo:P + dlo + w], lhsT=ones_col[:],
                            rhs=W[:, off:off + w], start=False, stop=False)
                    if tt == blk_last[b]:
                        # close the accumulation group (flush)
                        nc.tensor.matmul(out=blk[:, 0:2 * P],
                                         lhsT=zero_row[:1, 0:P],
                                         rhs=zero_row[:1, :],
                                         start=False, stop=True)
                        # epilogue: normalize + project + skip + relu
                        _blks.pop(b)
                        den_ap = (blk[F_IN:FE, 0:P] if layer == 1
                                  else blk[0:1, P:2 * P])
                        dv = pool.tile([1, P], FP32, tag="dv")
                        nc.vector.tensor_scalar_add(out=dv[:], in0=den_ap,
                                                    scalar1=EPS)
                        rv = pool.tile([1, P], FP32, tag="rv")
                        nc.vector.reciprocal(out=rv[:], in_=dv[:])
                        nc.tensor.matmul(out=blk[0:cp_agg, 3 * P:4 * P],
                                         lhsT=ones_row_f[:1, 0:cp_agg],
                                         rhs=rv[:], start=True, stop=True)
                        dnb = pool.tile([cp_agg, P], FP32, tag="dnb",
                                        name=f"dn{layer}_{b}")
                        nc.scalar.copy(out=dnb[:], in_=blk[0:cp_agg, 3 * P:4 * P])
                        xnorm = pool.tile([cp_agg, P], BF16, tag="xnorm",
                                          name=f"xn{layer}_{b}")
                        nc.vector.tensor_tensor(
                            out=xnorm[:], in0=blk[0:cp_agg, 0:P],
                            in1=dnb[:], op=OP.mult)
                        # skip + value projection accumulate back-to-back
                        skip_for_block(b, blk)
                        vproj = v1ext_t if layer == 1 else v2w_t
                        nc.tensor.matmul(out=blk[:, 2 * P:3 * P],
                                         lhsT=vproj[:], rhs=xnorm[:],
                                         start=False, stop=True)
                        h_sb = spool.tile([P, P], BF16, tag="h_sb")
                        if layer == 1:
                            nc.scalar.activation(out=h_sb[:],
                                                 in_=blk[:, 2 * P:3 * P],
                                                 func=AF.Relu)
                        else:
                            nc.scalar.activation(out=h_sb[:],
                                                 in_=blk[:, 2 * P:3 * P],
                                                 func=AF.Relu,
                                                 bias=b2col_t[:, 0:1])
                        on_block_done(b, h_sb, blk)

            for k in range(nsg + 1):
                if k < nsg:
                    front(k)
                if k >= 1:
                    back(k - 1)

        # ---------------- layer 1 plumbing
        def rhs1(b):
            return (B1_all[:, b * P:(b + 1) * P], FE)

        def skip1(b, blk):
            nc.tensor.matmul(out=blk[:, 2 * P:3 * P], lhsT=ws1ext_t[:],
                             rhs=xall[:, b * P:(b + 1) * P],
                             start=True, stop=False)

        def l1_done(b, h_sb, blk):
            nc.vector.tensor_copy(out=h1T_own[:, b * P:(b + 1) * P], in_=h_sb[:])
            # node-major fp8 h1 rows for the chunked allgather
            tp = psT.tile([P, P], BF16, space="PSUM", tag="psT",
                          name=f"h1tp{b}")
            nc.tensor.transpose(out=tp[:], in_=h_sb[:], identity=ident[:])
            tsb = spool.tile([P, H], F8, tag="t2row")
            nc.scalar.copy(out=tsb[:], in_=tp[:])
            ci = int(chunk_of[b])
            lb = b - int(cstart[ci])
            nc.sync.dma_start(out=h1f8_sh[ci][lb * P:(lb + 1) * P, :],
                              in_=tsb[:])
            if b == int(cstart[ci + 1]) - 1:
                nc.gpsimd.collective_compute(
                    kind="AllGather", op=OP.bypass,
                    replica_groups=[list(range(NCORES))],
                    ins=[h1f8_sh[ci]], outs=[h1f8_ag[ci]])

        edge_phase(1, xrows_lo, xrows_hi, rhs1, skip1, l1_done)

        # expand fp8 chunks to the f16 gather table (flat-order views)
        for ci in range(NCHUNK):
            rows = csizes[ci] * P
            for c in range(NCORES):
                e8 = xpool.tile([P, rows], F8, tag="e8")
                src_v = h1f8_ag[ci][c, :, :].flatten().rearrange(
                    "(p j) -> p j", p=P)
                nc.sync.dma_start(out=e8[:], in_=src_v)
                e16 = xpool.tile([P, rows], BF16, tag="e16")
                if c % 2 == 0:
                    nc.vector.tensor_copy(out=e16[:], in_=e8[:])
                else:
                    nc.scalar.copy(out=e16[:], in_=e8[:])
                g0 = c * ncap + int(cstart[ci]) * P
                dst_v = h1x_all[g0:g0 + rows, :].flatten().rearrange(
                    "(p j) -> p j", p=P)
                nc.sync.dma_start(out=dst_v, in_=e16[:])

        if dbg == 'h1':
            d = pool.tile([P, NB * P], FP32, tag="dbg")
            nc.vector.tensor_copy(out=d[:], in_=h1T_own[:])
            nc.sync.dma_start(out=dbg_o, in_=d[:])
            do = pool.tile([GPC, 1], FP32, tag="dbgo")
            nc.vector.memset(do[:], 0.5)
            nc.sync.dma_start(out=out_g, in_=do[:])

        if dbg is None or dbg == 'full':
            # ---------------- layer 2 plumbing
            pool_sb = persist.tile([GPC, H + 1], FP32)
            nc.vector.memset(pool_sb[:], 0.0)

            # score rhs blocks during the collective: R_b = A2 h1_d^T + u
            R_all = persist.tile([H, NB * P], BF16)
            for b in range(NB):
                rp = psS.tile([H, P], FP32, space="PSUM", tag="psS",
                              name=f"r2_{b}")
                nc.tensor.matmul(out=rp[:], lhsT=a2t_t[:],
                                 rhs=h1T_own[:, b * P:(b + 1) * P],
                                 start=True, stop=False)
                nc.tensor.matmul(out=rp[:], lhsT=urow_t[:1, :],
                                 rhs=ones_row[:1, :], start=False, stop=True)
                if b % 2 == 0:
                    nc.vector.tensor_copy(out=R_all[:, b * P:(b + 1) * P],
                                          in_=rp[:])
                else:
                    nc.scalar.copy(out=R_all[:, b * P:(b + 1) * P], in_=rp[:])

            def rhs2(b):
                return (R_all[:, b * P:(b + 1) * P], H)

            def skip2(b, blk):
                nc.tensor.matmul(out=blk[:, 2 * P:3 * P], lhsT=ws2_t[:],
                                 rhs=h1T_own[:, b * P:(b + 1) * P],
                                 start=True, stop=False)

            def l2_done(b, h_sb, blk):
                # transpose h2^T -> h2 [d, h], then pool matmul
                tp = psT.tile([P, P], BF16, space="PSUM", tag="psT",
                              name=f"h2tp{b}")
                nc.tensor.transpose(out=tp[:], in_=h_sb[:], identity=ident[:])
                h2x = pool.tile([P, H + 1], BF16, tag="h2x")
                nc.scalar.copy(out=h2x[:, 0:H], in_=tp[:])
                nc.vector.memset(h2x[:, H:H + 1], 1.0)
                gh = pool.tile([P, GPC], BF16, tag="gh")
                nc.vector.tensor_scalar(
                    out=gh[:], in0=iota[:, 0:GPC], scalar1=glocal_t[:, b:b + 1],
                    scalar2=None, op0=OP.is_equal)
                nc.tensor.matmul(out=blk[0:GPC, 2 * P:2 * P + H + 1],
                                 lhsT=gh[:], rhs=h2x[:],
                                 start=True, stop=True)
                nc.vector.tensor_tensor(out=pool_sb[:], in0=pool_sb[:],
                                        in1=blk[0:GPC, 2 * P:2 * P + H + 1],
                                        op=OP.add)

            # seq branch computed during the collective window
            seqw_t0 = const_tile(seqw[0:P, :], [P, H])
            seqw_t1 = const_tile(seqw[P:SEQ, :], [P, H])
            fc1w_t0 = const_tile(fc1w[0:P, :], [P, H])
            fc1w_t1 = const_tile(fc1w[P:2 * H, :], [P, H])
            seqT0 = const_tile(seqT[0:P, :], [P, GPC])
            seqT1 = const_tile(seqT[P:SEQ, :], [P, GPC])
            z = pool.tile([GPC, 2 * H], BF16, tag="z")
            pseq = psS.tile([GPC, H], FP32, space="PSUM", tag="psS",
                            name="pseq")
            nc.tensor.matmul(out=pseq[:], lhsT=seqT0[:], rhs=seqw_t0[:],
                             start=True, stop=False)
            nc.tensor.matmul(out=pseq[:], lhsT=seqT1[:], rhs=seqw_t1[:],
                             start=False, stop=False)
            nc.tensor.matmul(out=pseq[:], lhsT=ones_row[:1, 0:GPC],
                             rhs=seqb_t[:1, :], start=False, stop=True)
            nc.scalar.activation(out=z[:, H:2 * H], in_=pseq[:], func=AF.Relu)

            edge_phase(2, h1x_all[0:split, :], h1x_all[split:NCORES * ncap, :],
                       rhs2, skip2, l2_done)

            # ---------------- tail: pooled -> MLP -> sigmoid
            cnt = pool.tile([GPC, 1], FP32, tag="cnt")
            nc.vector.tensor_scalar_add(out=cnt[:], in0=pool_sb[:, H:H + 1],
                                        scalar1=EPS)
            rc = pool.tile([GPC, 1], FP32, tag="rc")
            nc.vector.reciprocal(out=rc[:], in_=cnt[:])
            nc.vector.tensor_scalar(out=z[:, 0:H], in0=pool_sb[:, 0:H],
                                    scalar1=rc[:, 0:1], scalar2=None,
                                    op0=OP.mult)

            zT = []
            for i in range(2):
                tzp = psT.tile([P, GPC], BF16, space="PSUM", tag="psT",
                               name=f"tzp{i}")
                nc.tensor.transpose(out=tzp[:], in_=z[:, i * H:(i + 1) * H],
                                    identity=ident[0:GPC, 0:GPC])
                zt = pool.tile([P, GPC], BF16, tag=f"zT{i}")
                nc.vector.tensor_copy(out=zt[:], in_=tzp[:])
                zT.append(zt)
            pfc1 = psS.tile([GPC, H], FP32, space="PSUM", tag="psS",
                            name="pfc1")
            nc.tensor.matmul(out=pfc1[:], lhsT=zT[0][:], rhs=fc1w_t0[:],
                             start=True, stop=False)
            nc.tensor.matmul(out=pfc1[:], lhsT=zT[1][:], rhs=fc1w_t1[:],
                             start=False, stop=False)
            nc.tensor.matmul(out=pfc1[:], lhsT=ones_row[:1, 0:GPC],
                             rhs=fc1b_t[:1, :], start=False, stop=True)
            z1 = pool.tile([GPC, H], BF16, tag="z1")
            nc.scalar.activation(out=z1[:], in_=pfc1[:], func=AF.Relu)
            tz1 = psT.tile([P, GPC], BF16, space="PSUM", tag="psT",
                           name="tz1")
            nc.tensor.transpose(out=tz1[:], in_=z1[:],
                                identity=ident[0:GPC, 0:GPC])
            z1T = pool.tile([P, GPC], BF16, tag="z1T")
            nc.vector.tensor_copy(out=z1T[:], in_=tz1[:])
            pfc2 = psS.tile([GPC, 1], FP32, space="PSUM", tag="psS",
                            name="pfc2")
            nc.tensor.matmul(out=pfc2[:], lhsT=z1T[:], rhs=fc2w_t[:],
                             start=True, stop=False)
            nc.tensor.matmul(out=pfc2[:], lhsT=ones_row[:1, 0:GPC],
                             rhs=fc2b_t[:1, :], start=False, stop=True)
            outs = pool.tile([GPC, 1], FP32, tag="outs")
            nc.scalar.activation(out=outs[:], in_=pfc2[:], func=AF.Sigmoid)
            nc.sync.dma_start(out=out_g, in_=outs[:])

    nc.compile()
    return nc


# ---------------------------------------------------------------- entry

_CACHE = {}


def kernel(**inputs):
    meta = preprocess(inputs)
    key = (meta['ncap'], meta['ntot'], tuple(meta['tile_block'].tolist()))
    if key not in _CACHE:
        _CACHE[key] = build_program(meta)
    nc = _CACHE[key]
    in_maps = make_inputs(inputs, meta)
    res = run_bass_kernel_spmd(nc, in_maps, core_ids=list(range(NCORES)))
    out = np.concatenate([res.results[c]['out_g'] for c in range(NCORES)], 0)
    return out.astype(np.float32)
